# revision 11
# baseline (speedup 1.0000x reference)
"""Trainium2 Bass kernel for MoE-LoRA GQA attention (nn_Attention_57389353009692).

V2 strategy (8 NeuronCores, one SPMD launch):
  - Tensor-parallel over heads: core c owns q-heads 4c..4c+3 and kv-head c.
  - Phase A (per 512-token block): QKV projections (+ MoE-LoRA) with packed
    matmul chains (wk|wv fused; LoRA-A for q/k/v + all three routers fused
    into two chains of 128/88 rows), router softmax done with
    exp -> ones-matmul row-sum -> reciprocal -> broadcast-matmul (no
    transposes, no DRAM bounce), RoPE in bf16 on 128 partitions.
  - Phase C: flash-style attention per 512-query block; causal mask applied
    as a 0/1 multiply after exp (bf16); output normalized PRE-collective via
    reciprocal-of-denominator broadcast matmuls fused into the PSUM->SBUF
    cast.
  - AllToAll is chunked per query block (4 collectives) and overlaps the
    remaining attention compute. Output tokens are interleaved at
    64-granularity: core c owns tokens {t : (t//64) % 8 == c} so every chunk
    is a uniform 8-way exchange.
  - Phase D: o-projection + o-LoRA for the core's 256 tokens with the full
    (prefetched) wo.

Numerics: bf16 operands, fp32 PSUM accumulation, fp32->exp softmax without
max subtraction (scores are O(1) here; masked entries are zeroed exactly by
the 0/1 multiply). Scale 1/sqrt(64) folded into wq and q-LoRA-B on host.
RoPE trick: interleaved even/odd pairs are made contiguous by permuting
wq/wk output features on host (per 2-head "page": [h0e|h1e|h0o|h1o]).
"""

import sys

for _p in ("/opt/trn_rl_repo", "/root/.axon_site/_ro/trn_rl_repo"):
    if _p not in sys.path:
        sys.path.insert(0, _p)

import numpy as np
import ml_dtypes

import concourse.bass as bass
import concourse.tile as tile
from concourse import bacc, mybir
from concourse.masks import make_identity
from concourse.alu_op_type import AluOpType

F32 = mybir.dt.float32
BF16 = mybir.dt.bfloat16
AF = mybir.ActivationFunctionType
AX = mybir.AxisListType
BF16NP = ml_dtypes.bfloat16

B, S, D = 1, 2048, 2048
H, KVH, HD = 32, 8, 64
NREP = H // KVH
R, E = 8, 8
SCALING = 32.0 / 8.0
NCORES = 8
QH = H // NCORES          # 4 q heads per core
QF = QH * HD              # 256 q feats per core
KF = HD                   # 64 kv feats per core
TSH = S // NCORES         # 256 tokens per core for o-proj
NKT = S // 128            # 16 key tiles
NQB = S // 512            # 4 query blocks
NIF = D // 128            # 16 contraction tiles

MASK_NEG = -1e30

# mask tile classes
M_SKIP, M_ZERO, M_ADD = 0, 1, 2


def _build_perm():
    """Per-core feature permutations for rope-friendly layout."""
    idx_q = np.zeros(QF, dtype=np.int64)
    for f in range(QF):
        page, w = divmod(f, 128)
        if w < 32:
            hl, j, odd = 2 * page, w, 0
        elif w < 64:
            hl, j, odd = 2 * page + 1, w - 32, 0
        elif w < 96:
            hl, j, odd = 2 * page, w - 64, 1
        else:
            hl, j, odd = 2 * page + 1, w - 96, 1
        idx_q[f] = 64 * hl + 2 * j + odd
    idx_k = np.zeros(KF, dtype=np.int64)
    for w in range(KF):
        if w < 32:
            idx_k[w] = 2 * w
        else:
            idx_k[w] = 2 * (w - 32) + 1
    return idx_q, idx_k


IDX_Q, IDX_K = _build_perm()


def _a_pack(A):
    """[E,R,D] -> [D, 64] with col r*8+e."""
    return np.transpose(A, (1, 0, 2)).reshape(E * R, -1).T


def _b_flat(Bw, scale):
    """[E, OF, R] -> [64, OF] with row r*8+e."""
    return (np.transpose(Bw, (2, 0, 1)).reshape(E * R, -1) * scale)


def _bf(x):
    return np.ascontiguousarray(x, dtype=np.float32).astype(BF16NP)


def _f32(x):
    return np.ascontiguousarray(x, dtype=np.float32)


def classify_mask(maskT):
    """maskT: [S(k), S(q)] clamped fp32. Returns [NKT, NQB] class map."""
    cls = np.zeros((NKT, NQB), dtype=np.int64)
    for kt in range(NKT):
        blk_rows = maskT[kt * 128:(kt + 1) * 128]
        for qb in range(NQB):
            blk = blk_rows[:, qb * 512:(qb + 1) * 512]
            if np.all(blk <= MASK_NEG * 0.5):
                cls[kt, qb] = M_SKIP
            elif np.all(blk == 0.0):
                cls[kt, qb] = M_ZERO
            else:
                cls[kt, qb] = M_ADD
    return cls


# constants tensor layout (bf16, [24, 344]):
#  [:, 0:128]   E_A: row e, col j -> 1 if (j<64 and e==j%8) or (j>=64 and e-8==j%8)
#  [:, 128:192] E_v: row e, col j -> 1 if e-16 == j%8
#  [:, 192:216] ones24: block-diag 3x(8x8 ones)
#  [0:8, 216:280] E8o: row e, col j -> 1 if e == j%8
#  [0:1, 280:344] ones64 row
CST_W = 344


def _build_cst():
    cst = np.zeros((24, CST_W), dtype=np.float32)
    for j in range(64):
        cst[j % 8, j] = 1.0           # E_A q half
        cst[8 + j % 8, 64 + j] = 1.0  # E_A k half
        cst[16 + j % 8, 128 + j] = 1.0  # E_v
        cst[j % 8, 216 + j] = 1.0     # E8o
        cst[0, 280 + j] = 1.0         # ones64
    for b in range(3):
        cst[8 * b:8 * b + 8, 192 + 8 * b:200 + 8 * b] = 1.0  # ones24
    return _bf(cst)


def build(mask_cls):
    """Build the SPMD Bass graph. mask_cls: [NKT, NQB] int array."""
    nc = bacc.Bacc(None, target_bir_lowering=False)

    # ---- DRAM I/O (per-core shards prepared on host) ----
    xT = nc.declare_dram_parameter("xT", [D, S], BF16, isOutput=False)
    wqT = nc.declare_dram_parameter("wqT", [D, QF], BF16, isOutput=False)
    wkvT = nc.declare_dram_parameter("wkvT", [D, 2 * KF], BF16, isOutput=False)
    aA = nc.declare_dram_parameter("aA", [D, 128], BF16, isOutput=False)
    aB = nc.declare_dram_parameter("aB", [D, 88], BF16, isOutput=False)
    ao = nc.declare_dram_parameter("ao", [D, 72], BF16, isOutput=False)
    bq = nc.declare_dram_parameter("bq", [E * R, QF], BF16, isOutput=False)
    bk = nc.declare_dram_parameter("bk", [E * R, KF], BF16, isOutput=False)
    bv = nc.declare_dram_parameter("bv", [E * R, KF], BF16, isOutput=False)
    bo = nc.declare_dram_parameter("bo", [E * R, D], BF16, isOutput=False)
    woT = nc.declare_dram_parameter("woT", [D, D], BF16, isOutput=False)
    cs4 = nc.declare_dram_parameter("cs4", [128, S], BF16, isOutput=False)
    sn4 = nc.declare_dram_parameter("sn4", [128, S], BF16, isOutput=False)
    mask01 = nc.declare_dram_parameter("mask01", [S, S], BF16, isOutput=False)
    cst = nc.declare_dram_parameter("cst", [24, CST_W], BF16, isOutput=False)
    y = nc.declare_dram_parameter("y", [TSH, D], F32, isOutput=True)

    # internal DRAM for the chunked collectives: [dest/src, 4h*64 feat, 64 tok]
    cc_in = [nc.dram_tensor("cc_in%d" % q, [NCORES, QF, 64], BF16)
             for q in range(NQB)]
    cc_out = [nc.dram_tensor("cc_out%d" % q, [NCORES, QF, 64], BF16)
              for q in range(NQB)]

    with tile.TileContext(nc) as tc:
        _emit(nc, tc, locals(), mask_cls)
    nc.finalize()
    return nc


def _emit(nc, tc, t, mask_cls):
    xT, wqT, wkvT = t["xT"], t["wqT"], t["wkvT"]
    aA, aB, ao = t["aA"], t["aB"], t["ao"]
    bq, bk, bv, bo = t["bq"], t["bk"], t["bv"], t["bo"]
    woT, cs4, sn4, mask01, y = t["woT"], t["cs4"], t["sn4"], t["mask01"], t["y"]
    cst = t["cst"]
    cc_in, cc_out = t["cc_in"], t["cc_out"]

    import contextlib
    ctx = contextlib.ExitStack()
    with ctx:
        persist = ctx.enter_context(tc.tile_pool(name="persist", bufs=1))
        ps = ctx.enter_context(tc.tile_pool(name="ps", bufs=1, space="PSUM"))

        # ---- persistent weights, split in k-groups of 4 for early start ----
        NSP = 4
        KG = NIF // NSP
        aA_sb, aB_sb, wq_sb, wkv_sb = [], [], [], []
        for g in range(NSP):
            ksl = slice(g * KG * 128, (g + 1) * KG * 128)
            tl = persist.tile([128, KG, 128], BF16, name="aA%d" % g)
            nc.scalar.dma_start(
                out=tl, in_=aA[ksl].rearrange("(n p) f -> p n f", p=128))
            aA_sb.append(tl)
            tl = persist.tile([128, KG, 88], BF16, name="aB%d" % g)
            nc.scalar.dma_start(
                out=tl, in_=aB[ksl].rearrange("(n p) f -> p n f", p=128))
            aB_sb.append(tl)
            tl = persist.tile([128, KG, QF], BF16, name="wq%d" % g)
            nc.sync.dma_start(
                out=tl, in_=wqT[ksl].rearrange("(n p) f -> p n f", p=128))
            wq_sb.append(tl)
            tl = persist.tile([128, KG, 2 * KF], BF16, name="wkv%d" % g)
            nc.gpsimd.dma_start(
                out=tl, in_=wkvT[ksl].rearrange("(n p) f -> p n f", p=128))
            wkv_sb.append(tl)

        def A_AT(k):  # aA chain lhsT for contraction tile k
            return aA_sb[k // KG][:, k % KG, :]

        def A_BT(k):
            return aB_sb[k // KG][:, k % KG, :]

        def W_Q(k):
            return wq_sb[k // KG][:, k % KG, :]

        def W_KV(k):
            return wkv_sb[k // KG][:, k % KG, :]

        cst_sb = persist.tile([24, CST_W], BF16)
        nc.gpsimd.dma_start(out=cst_sb, in_=cst[:])
        E_A = cst_sb[:, 0:128]
        E_v = cst_sb[0:24, 128:192]
        ones24 = cst_sb[:, 192:216]
        ones8 = cst_sb[0:8, 192:200]
        E8o = cst_sb[0:8, 216:280]
        ones64 = cst_sb[0:1, 280:344]

        bq_sb = persist.tile([64, QF], BF16)
        nc.gpsimd.dma_start(out=bq_sb, in_=bq[:])
        bk_sb = persist.tile([128, KF], BF16)   # bk lives at partitions 64:128
        nc.gpsimd.dma_start(out=bk_sb[64:128, :], in_=bk[:])
        bv_sb = persist.tile([64, KF], BF16)
        nc.gpsimd.dma_start(out=bv_sb, in_=bv[:])
        bo_sb = persist.tile([64, D], BF16)
        nc.gpsimd.dma_start(out=bo_sb, in_=bo[:])
        ao_sb = persist.tile([128, NIF, 72], BF16)
        nc.scalar.dma_start(out=ao_sb,
                            in_=ao.rearrange("(n p) f -> p n f", p=128))
        cs_sb = persist.tile([128, S], BF16)
        nc.scalar.dma_start(out=cs_sb, in_=cs4[:])
        sn_sb = persist.tile([128, S], BF16)
        nc.scalar.dma_start(out=sn_sb, in_=sn4[:])

        ident_b = persist.tile([128, 128], BF16)
        make_identity(nc, ident_b)

        # attention operands (persist across phases)
        qh_sb = persist.tile([128, 2, S], BF16)   # [2 heads x 64, page, S]
        kh_sb = persist.tile([128, S], BF16)      # kv head duplicated 2x
        vtok = persist.tile([128, NKT, 65], BF16)  # token-major v + ones col
        nc.vector.memset(vtok[:, :, 64:65], 1.0)
        g_sb = persist.tile([128, NIF, TSH], BF16)  # gathered out (post-A2A)

        # ================= Phase A: QKV + LoRA + RoPE =================
        with tc.tile_pool(name="pA", bufs=1) as pA:
            for tb in range(4):
                tsl = slice(tb * 512, (tb + 1) * 512)
                xq = pA.tile([128, NIF, 512], BF16, name="xq", tag="xq",
                             bufs=2)
                if tb == 0:
                    # split first block's load so matmuls start early
                    for g in range(NSP):
                        ksl = slice(g * KG * 128, (g + 1) * KG * 128)
                        nc.sync.dma_start(
                            out=xq[:, g * KG:(g + 1) * KG, :],
                            in_=xT[ksl].rearrange(
                                "(n p) t -> p n t", p=128)[:, :, tsl])
                else:
                    nc.sync.dma_start(
                        out=xq,
                        in_=xT.rearrange("(n p) t -> p n t", p=128)[:, :, tsl])

                # ---- main projection chains ----
                hA = ps.tile([128, 512], F32, name="hA", tag="p_hA")
                hB = ps.tile([88, 512], F32, name="hB", tag="p_hB")
                q0 = ps.tile([128, 512], F32, name="q0", tag="p_q0")
                q1 = ps.tile([128, 512], F32, name="q1", tag="p_q1")
                kv = ps.tile([128, 512], F32, name="kv", tag="p_kv")
                for k in range(NIF):
                    st = k == 0
                    sp = k == NIF - 1
                    rhs = xq[:, k, :]
                    nc.tensor.matmul(hA, A_AT(k), rhs, start=st, stop=sp)
                    nc.tensor.matmul(hB, A_BT(k), rhs, start=st, stop=sp)
                for k in range(NIF):
                    rhs = xq[:, k, :]
                    st = k == 0
                    nc.tensor.matmul(q0, W_Q(k)[:, 0:128], rhs,
                                     start=st, stop=False)
                    nc.tensor.matmul(q1, W_Q(k)[:, 128:256], rhs,
                                     start=st, stop=False)
                    nc.tensor.matmul(kv, W_KV(k), rhs, start=st, stop=False)

                # ---- router softmax (q,k,v fused; no transposes) ----
                ex3 = pA.tile([24, 512], BF16, name="ex3", tag="ex3", bufs=2)
                nc.scalar.activation(ex3, hB[64:88, :], AF.Exp)
                s3 = ps.tile([24, 512], F32, name="s3", tag="p_s3")
                nc.tensor.matmul(s3, ones24, ex3, start=True, stop=True)
                rec3 = pA.tile([24, 512], BF16, name="rec3", tag="rec3",
                               bufs=2)
                with nc.allow_low_precision(reason="router softmax denom"):
                    nc.vector.reciprocal(rec3, s3)
                rw3 = pA.tile([24, 512], BF16, name="rw3", tag="rw3", bufs=2)
                nc.vector.tensor_tensor(rw3, ex3, rec3, AluOpType.mult)
                rwbA = ps.tile([128, 512], F32, name="rwbA", tag="p_rwA")
                nc.tensor.matmul(rwbA, E_A, rw3, start=True, stop=True)
                rwbV = ps.tile([64, 512], F32, name="rwbV", tag="p_rwV")
                nc.tensor.matmul(rwbV, E_v, rw3, start=True, stop=True)
                rwbA_s = pA.tile([128, 512], BF16, name="rwbA_s",
                                 tag="rwbA_s", bufs=2)
                nc.scalar.activation(rwbA_s, rwbA, AF.Copy)
                rwbV_s = pA.tile([64, 512], BF16, name="rwbV_s",
                                 tag="rwbV_s", bufs=2)
                nc.scalar.activation(rwbV_s, rwbV, AF.Copy)
                hpA = pA.tile([128, 512], BF16, name="hpA", tag="hpA", bufs=2)
                nc.vector.tensor_tensor(hpA, hA, rwbA_s, AluOpType.mult)
                hpV = pA.tile([64, 512], BF16, name="hpV", tag="hpV", bufs=2)
                nc.vector.tensor_tensor(hpV, hB[0:64, :], rwbV_s,
                                        AluOpType.mult)

                # ---- LoRA-B closes the accumulations ----
                nc.tensor.matmul(q0, bq_sb[:, 0:128], hpA[0:64, :],
                                 start=False, stop=True)
                nc.tensor.matmul(q1, bq_sb[:, 128:256], hpA[0:64, :],
                                 start=False, stop=True)
                nc.tensor.matmul(kv[0:64, :], bk_sb[64:128, :],
                                 hpA[64:128, :], start=False, stop=True,
                                 tile_position=(64, 0))
                nc.tensor.matmul(kv[64:128, :], bv_sb, hpV,
                                 start=False, stop=True,
                                 tile_position=(0, 64))

                # ---- RoPE (bf16, 128 partitions) ----
                qe = pA.tile([128, 512], BF16, name="qe", tag="qe", bufs=2)
                qo = pA.tile([128, 512], BF16, name="qo", tag="qo", bufs=2)
                nc.scalar.activation(qe[0:64, :], q0[0:64, :], AF.Copy)
                nc.scalar.activation(qe[64:128, :], q1[0:64, :], AF.Copy)
                nc.scalar.activation(qo[0:64, :], q0[64:128, :], AF.Copy)
                nc.scalar.activation(qo[64:128, :], q1[64:128, :], AF.Copy)
                cs_t = cs_sb[:, tsl]
                sn_t = sn_sb[:, tsl]
                t1 = pA.tile([128, 512], BF16, name="t1", tag="t1", bufs=2)
                t2 = pA.tile([128, 512], BF16, name="t2", tag="t2", bufs=2)
                rote = pA.tile([128, 512], BF16, name="rote", tag="rote",
                               bufs=2)
                roto = pA.tile([128, 512], BF16, name="roto", tag="roto",
                               bufs=2)
                nc.vector.tensor_tensor(t1, qe, cs_t, AluOpType.mult)
                nc.vector.tensor_tensor(t2, qo, sn_t, AluOpType.mult)
                nc.vector.tensor_tensor(rote, t1, t2, AluOpType.subtract)
                nc.vector.tensor_tensor(t1, qe, sn_t, AluOpType.mult)
                nc.vector.tensor_tensor(t2, qo, cs_t, AluOpType.mult)
                nc.vector.tensor_tensor(roto, t1, t2, AluOpType.add)

                # k RoPE on gpsimd ([32, 512])
                ke = pA.tile([32, 512], BF16, name="ke", tag="ke", bufs=2)
                ko = pA.tile([32, 512], BF16, name="ko", tag="ko", bufs=2)
                nc.scalar.activation(ke, kv[0:32, :], AF.Copy)
                nc.scalar.activation(ko, kv[32:64, :], AF.Copy)
                k1 = pA.tile([32, 512], BF16, name="k1", tag="k1", bufs=2)
                k2 = pA.tile([32, 512], BF16, name="k2", tag="k2", bufs=2)
                csk = cs_sb[0:32, tsl]
                snk = sn_sb[0:32, tsl]
                nc.gpsimd.tensor_tensor(k1, ke, csk, AluOpType.mult)
                nc.gpsimd.tensor_tensor(k2, ko, snk, AluOpType.mult)
                nc.gpsimd.tensor_tensor(kh_sb[0:32, tsl], k1, k2,
                                        AluOpType.subtract)
                nc.gpsimd.tensor_tensor(k1, ke, snk, AluOpType.mult)
                nc.gpsimd.tensor_tensor(k2, ko, csk, AluOpType.mult)
                nc.gpsimd.tensor_tensor(kh_sb[32:64, tsl], k1, k2,
                                        AluOpType.add)
                nc.gpsimd.tensor_copy(kh_sb[64:96, tsl], kh_sb[0:32, tsl])
                nc.gpsimd.tensor_copy(kh_sb[96:128, tsl], kh_sb[32:64, tsl])

                # q head rearrange: head h=2*page+i at partitions
                # [64i, 64i+32)=evens, [64i+32, 64i+64)=odds, page index.
                for h in range(QH):
                    page, i = h // 2, h % 2
                    nc.vector.tensor_copy(
                        qh_sb[64 * i:64 * i + 32, page, tsl],
                        rote[32 * h:32 * h + 32, :])
                    nc.vector.tensor_copy(
                        qh_sb[64 * i + 32:64 * i + 64, page, tsl],
                        roto[32 * h:32 * h + 32, :])

                # token-major v (+ denominator ones col kept from memset)
                vT = pA.tile([64, 512], BF16, name="vT", tag="vT", bufs=2)
                nc.scalar.activation(vT, kv[64:128, :], AF.Copy)
                for j in range(4):
                    kt = 4 * tb + j
                    v_ps = ps.tile([128, 64], BF16, name="v_ps", tag="p_s3")
                    nc.tensor.transpose(v_ps, vT[:, 128 * j:128 * j + 128],
                                        ident_b[0:64, 0:64])
                    nc.vector.tensor_copy(vtok[:, kt, 0:64], v_ps)

        # prefetch the full output-projection weight during attention
        wo_ctx = tc.tile_pool(name="wo_pool", bufs=4)
        wo_pool = wo_ctx.__enter__()
        wo_tiles = []
        for ob in range(4):
            osl = slice(ob * 512, (ob + 1) * 512)
            wo_sb = wo_pool.tile([128, NIF, 512], BF16, name="wo_sb",
                                 tag="wo", bufs=4)
            nc.gpsimd.dma_start(
                out=wo_sb,
                in_=woT.rearrange("(n p) f -> p n f", p=128)[:, :, osl])
            wo_tiles.append(wo_sb)

        # ============ Phase C: attention + chunked A2A ============
        SC_TAGS = ["p_q0", "p_q1", "p_hA", "p_hB"]
        OUT_TAGS = ["p_kv", "p_s3", "p_rwA", "p_rwV"]
        with tc.tile_pool(name="pC", bufs=1) as pC:
            for qb in range(NQB):
                qsl = slice(qb * 512, (qb + 1) * 512)
                active = [kt for kt in range(NKT)
                          if mask_cls[kt, qb] != M_SKIP]
                assert active, f"fully masked query block qb={qb}"
                outps = [ps.tile([65, 512], F32, name="outp%d" % h,
                                 tag=OUT_TAGS[h]) for h in range(QH)]
                for kt in active:
                    c = mask_cls[kt, qb]
                    mt = None
                    if c == M_ADD:
                        mt = pC.tile([128, 512], BF16, name="mt",
                                     tag="mt", bufs=4)
                        nc.sync.dma_start(
                            out=mt,
                            in_=mask01[128 * kt:128 * kt + 128, qsl])
                    ksl = slice(128 * kt, 128 * kt + 128)
                    prs = []
                    for h in range(QH):
                        page, i = h // 2, h % 2
                        sc = ps.tile([128, 512], F32, name="sc%d" % h,
                                     tag=SC_TAGS[h])
                        nc.tensor.matmul(sc,
                                         kh_sb[64 * i:64 * i + 64, ksl],
                                         qh_sb[64 * i:64 * i + 64, page, qsl],
                                         start=True, stop=True,
                                         tile_position=(64 * i, 0))
                        pr = pC.tile([128, 512], BF16, name="pr%d" % h,
                                     tag="pr%d" % h, bufs=2)
                        nc.scalar.activation(pr, sc, AF.Exp)
                        if mt is not None:
                            nc.vector.tensor_tensor(pr, pr, mt,
                                                    AluOpType.mult)
                        prs.append(pr)
                    for h in range(QH):
                        nc.tensor.matmul(outps[h], vtok[:, kt, :], prs[h],
                                         start=(kt == active[0]),
                                         stop=(kt == active[-1]))
                # normalize + ship this query block's chunk
                for h in range(QH):
                    den = pC.tile([1, 512], F32, name="den%d" % h,
                                  tag="den%d" % h, bufs=2)
                    nc.vector.tensor_copy(den, outps[h][64:65, :])
                    rec = pC.tile([1, 512], BF16, name="rec%d" % h,
                                  tag="rec%d" % h, bufs=2)
                    with nc.allow_low_precision(reason="attn denom"):
                        nc.vector.reciprocal(rec, den)
                    rb = ps.tile([64, 512], F32, name="rb", tag=SC_TAGS[h])
                    nc.tensor.matmul(rb, ones64, rec, start=True, stop=True)
                    rb_s = pC.tile([64, 512], BF16, name="rb_s%d" % h,
                                   tag="rb_s%d" % h, bufs=2)
                    nc.vector.tensor_copy(rb_s, rb)
                    o65 = pC.tile([64, 512], BF16, name="o65%d" % h,
                                  tag="o65%d" % h, bufs=2)
                    nc.vector.tensor_tensor(o65, outps[h][0:64, :], rb_s,
                                            AluOpType.mult)
                    # [64, 512] -> cc_in[qb][dest, 64h:64h+64, 0:64]
                    nc.gpsimd.dma_start(
                        out=cc_in[qb][:, 64 * h:64 * h + 64, :]
                            .rearrange("d p t -> p d t"),
                        in_=o65)
                nc.gpsimd.collective_compute(
                    "AllToAll",
                    AluOpType.bypass,
                    ins=[cc_in[qb][:]],
                    outs=[cc_out[qb][:]],
                    replica_groups=[list(range(NCORES))],
                )
                # gather this chunk into g_sb[:, :, 64qb:64qb+64]
                nc.sync.dma_start(
                    out=g_sb[:, :, 64 * qb:64 * qb + 64],
                    in_=cc_out[qb].rearrange("s (k p) t -> p (s k) t",
                                             k=2, p=128))

        # ================= Phase D: o-proj =================
        with tc.tile_pool(name="pD", bufs=1) as pD:
            ho = ps.tile([72, TSH], F32, name="ho", tag="p_hA")
            for k in range(NIF):
                nc.tensor.matmul(ho, ao_sb[:, k, :], g_sb[:, k, :],
                                 start=(k == 0), stop=(k == NIF - 1))
            exo = pD.tile([8, TSH], BF16, name="exo")
            nc.scalar.activation(exo, ho[64:72, :], AF.Exp)
            so = ps.tile([8, TSH], F32, name="so", tag="p_s3")
            nc.tensor.matmul(so, ones8, exo, start=True, stop=True)
            reco = pD.tile([8, TSH], BF16, name="reco")
            with nc.allow_low_precision(reason="o-router softmax denom"):
                nc.vector.reciprocal(reco, so)
            rwo = pD.tile([8, TSH], BF16, name="rwo")
            nc.vector.tensor_tensor(rwo, exo, reco, AluOpType.mult)
            rwbo = ps.tile([64, TSH], F32, name="rwbo", tag="p_hB")
            nc.tensor.matmul(rwbo, E8o, rwo, start=True, stop=True)
            rwbo_s = pD.tile([64, TSH], BF16, name="rwbo_s")
            nc.vector.tensor_copy(rwbo_s, rwbo)
            hpo = pD.tile([64, TSH], BF16, name="hpo")
            nc.vector.tensor_tensor(hpo, ho[0:64, :], rwbo_s, AluOpType.mult)

            for ob in range(4):
                osl = slice(ob * 512, (ob + 1) * 512)
                wo_sb = wo_tiles[ob]
                for tt in range(2):
                    yp = ps.tile([128, 512], F32, name="yp",
                                 tag="p_q0" if (2 * ob + tt) % 2 == 0
                                 else "p_q1")
                    for k in range(NIF):
                        nc.tensor.matmul(
                            yp, g_sb[:, k, 128 * tt:128 * tt + 128],
                            wo_sb[:, k, :], start=(k == 0), stop=False)
                    nc.tensor.matmul(yp, hpo[:, 128 * tt:128 * tt + 128],
                                     bo_sb[:, osl], start=False, stop=True)
                    yt = pD.tile([128, 512], F32, name="yt", tag="yt",
                                 bufs=3)
                    if (2 * ob + tt) % 2 == 0:
                        nc.scalar.activation(yt, yp, AF.Copy)
                    else:
                        nc.vector.tensor_copy(yt, yp)
                    nc.sync.dma_start(out=y[128 * tt:128 * tt + 128, osl],
                                      in_=yt)
        wo_ctx.__exit__(None, None, None)


# ======================= host side =======================

_CACHE = {}


def _prep_inputs(x, mask, freqs_cos, freqs_sin, wq, wk, wv, wo,
                 lq_router, lq_A, lq_B, lk_router, lk_A, lk_B,
                 lv_router, lv_A, lv_B, lo_router, lo_A, lo_B):
    scale = 1.0 / np.sqrt(HD)
    x = _f32(np.asarray(x)).reshape(S, D)
    maskf = _f32(np.asarray(mask)).reshape(S, S)
    maskT = np.maximum(maskf, MASK_NEG).T.copy()
    mask_cls = classify_mask(maskT)
    mask01 = _bf((maskT > MASK_NEG * 0.5).astype(np.float32))

    xT = _bf(x.T)
    cs4 = _bf(np.tile(_f32(freqs_cos).T, (4, 1)))      # [128, S]
    sn4 = _bf(np.tile(_f32(freqs_sin).T, (4, 1)))
    woT = _bf(_f32(wo).T)
    ao_p = _bf(np.concatenate([_a_pack(_f32(lo_A)), _f32(lo_router).T],
                              axis=1))                 # [D, 72]
    bo_f = _bf(_b_flat(_f32(lo_B), SCALING))

    # fused LoRA-A stationaries: [D, 128] = [aq|ak], [D, 88] = [av|rq|rk|rv]
    aA_p = _bf(np.concatenate(
        [_a_pack(_f32(lq_A)), _a_pack(_f32(lk_A))], axis=1))
    aB_p = _bf(np.concatenate(
        [_a_pack(_f32(lv_A)), _f32(lq_router).T, _f32(lk_router).T,
         _f32(lv_router).T], axis=1))

    shared = dict(xT=xT, cs4=cs4, sn4=sn4, woT=woT, mask01=mask01,
                  ao=ao_p, bo=bo_f, cst=_build_cst(), aA=aA_p, aB=aB_p)

    wqf, wkf, wvf = _f32(wq), _f32(wk), _f32(wv)
    lqB, lkB, lvB = _f32(lq_B), _f32(lk_B), _f32(lv_B)

    in_maps = []
    for c in range(NCORES):
        wq_c = wqf[c * QF:(c + 1) * QF][IDX_Q] * scale
        wk_c = wkf[c * KF:(c + 1) * KF][IDX_K]
        wv_c = wvf[c * KF:(c + 1) * KF]
        bq_c = _b_flat(lqB[:, c * QF:(c + 1) * QF, :][:, IDX_Q, :],
                       SCALING * scale)
        bk_c = _b_flat(lkB[:, c * KF:(c + 1) * KF, :][:, IDX_K, :], SCALING)
        bv_c = _b_flat(lvB[:, c * KF:(c + 1) * KF, :], SCALING)
        m = dict(shared)
        m.update(wqT=_bf(wq_c.T),
                 wkvT=_bf(np.concatenate([wk_c.T, wv_c.T], axis=1)),
                 bq=_bf(bq_c), bk=_bf(bk_c), bv=_bf(bv_c))
        in_maps.append(m)
    return in_maps, mask_cls


def get_graph(mask_cls):
    key = mask_cls.tobytes()
    if key not in _CACHE:
        _CACHE[key] = build(mask_cls)
    return _CACHE[key]


def kernel(x, start_pos, mask, freqs_cos, freqs_sin, wq, wk, wv, wo,
           lq_router, lq_A, lq_B, lk_router, lk_A, lk_B,
           lv_router, lv_A, lv_B, lo_router, lo_A, lo_B,
           _trace=False):
    from concourse.bass_utils import run_bass_kernel_spmd
    in_maps, mask_cls = _prep_inputs(
        x, mask, freqs_cos, freqs_sin, wq, wk, wv, wo,
        lq_router, lq_A, lq_B, lk_router, lk_A, lk_B,
        lv_router, lv_A, lv_B, lo_router, lo_A, lo_B)
    nc = get_graph(mask_cls)
    res = run_bass_kernel_spmd(nc, in_maps, list(range(NCORES)), trace=_trace)
    # core c's y rows: group g (0..3) covers tokens [512g + 64c, 512g + 64c + 64)
    ys = np.stack([res.results[c]["y"] for c in range(NCORES)], axis=0)
    ys = ys.reshape(NCORES, 4, 64, D).transpose(1, 0, 2, 3).reshape(S, D)
    out = ys.reshape(B, S, H * HD).astype(np.float32)
    if _trace:
        return out, res
    return out


# revision 13
# speedup vs baseline: 1.2458x; 1.2458x over previous
"""Trainium2 Bass kernel for MoE-LoRA GQA attention (nn_Attention_57389353009692).

V2 strategy (8 NeuronCores, one SPMD launch):
  - Tensor-parallel over heads: core c owns q-heads 4c..4c+3 and kv-head c.
  - Phase A (per 512-token block): QKV projections (+ MoE-LoRA) with packed
    matmul chains (wk|wv fused; LoRA-A for q/k/v + all three routers fused
    into two chains of 128/88 rows), router softmax done with
    exp -> ones-matmul row-sum -> reciprocal -> broadcast-matmul (no
    transposes, no DRAM bounce), RoPE in bf16 on 128 partitions.
  - Phase C: flash-style attention per 512-query block; causal mask applied
    as a 0/1 multiply after exp (bf16); output normalized PRE-collective via
    reciprocal-of-denominator broadcast matmuls fused into the PSUM->SBUF
    cast.
  - AllToAll is chunked per query block (4 collectives) and overlaps the
    remaining attention compute. Output tokens are interleaved at
    64-granularity: core c owns tokens {t : (t//64) % 8 == c} so every chunk
    is a uniform 8-way exchange.
  - Phase D: o-projection + o-LoRA for the core's 256 tokens with the full
    (prefetched) wo.

Numerics: bf16 operands, fp32 PSUM accumulation, fp32->exp softmax without
max subtraction (scores are O(1) here; masked entries are zeroed exactly by
the 0/1 multiply). Scale 1/sqrt(64) folded into wq and q-LoRA-B on host.
RoPE trick: interleaved even/odd pairs are made contiguous by permuting
wq/wk output features on host (per 2-head "page": [h0e|h1e|h0o|h1o]).
"""

import sys

for _p in ("/opt/trn_rl_repo", "/root/.axon_site/_ro/trn_rl_repo"):
    if _p not in sys.path:
        sys.path.insert(0, _p)

import numpy as np
import ml_dtypes

import concourse.bass as bass
import concourse.tile as tile
from concourse import bacc, mybir
from concourse.masks import make_identity
from concourse.alu_op_type import AluOpType

F32 = mybir.dt.float32
BF16 = mybir.dt.bfloat16
AF = mybir.ActivationFunctionType
AX = mybir.AxisListType
BF16NP = ml_dtypes.bfloat16

B, S, D = 1, 2048, 2048
H, KVH, HD = 32, 8, 64
NREP = H // KVH
R, E = 8, 8
SCALING = 32.0 / 8.0
NCORES = 8
QH = H // NCORES          # 4 q heads per core
QF = QH * HD              # 256 q feats per core
KF = HD                   # 64 kv feats per core
TSH = S // NCORES         # 256 tokens per core for o-proj
NKT = S // 128            # 16 key tiles
NQB = S // 512            # 4 query blocks
NIF = D // 128            # 16 contraction tiles

MASK_NEG = -1e30

# mask tile classes
M_SKIP, M_ZERO, M_ADD = 0, 1, 2


def _build_perm():
    """Per-core feature permutations for rope-friendly layout."""
    idx_q = np.zeros(QF, dtype=np.int64)
    for f in range(QF):
        page, w = divmod(f, 128)
        if w < 32:
            hl, j, odd = 2 * page, w, 0
        elif w < 64:
            hl, j, odd = 2 * page + 1, w - 32, 0
        elif w < 96:
            hl, j, odd = 2 * page, w - 64, 1
        else:
            hl, j, odd = 2 * page + 1, w - 96, 1
        idx_q[f] = 64 * hl + 2 * j + odd
    idx_k = np.zeros(KF, dtype=np.int64)
    for w in range(KF):
        if w < 32:
            idx_k[w] = 2 * w
        else:
            idx_k[w] = 2 * (w - 32) + 1
    return idx_q, idx_k


IDX_Q, IDX_K = _build_perm()


def _a_pack(A):
    """[E,R,D] -> [D, 64] with col r*8+e."""
    return np.transpose(A, (1, 0, 2)).reshape(E * R, -1).T


def _b_flat(Bw, scale):
    """[E, OF, R] -> [64, OF] with row r*8+e."""
    return (np.transpose(Bw, (2, 0, 1)).reshape(E * R, -1) * scale)


def _bf(x):
    return np.ascontiguousarray(x, dtype=np.float32).astype(BF16NP)


def _f32(x):
    return np.ascontiguousarray(x, dtype=np.float32)


def classify_mask(maskT):
    """maskT: [S(k), S(q)] clamped fp32. Returns [NKT, NQB] class map."""
    cls = np.zeros((NKT, NQB), dtype=np.int64)
    for kt in range(NKT):
        blk_rows = maskT[kt * 128:(kt + 1) * 128]
        for qb in range(NQB):
            blk = blk_rows[:, qb * 512:(qb + 1) * 512]
            if np.all(blk <= MASK_NEG * 0.5):
                cls[kt, qb] = M_SKIP
            elif np.all(blk == 0.0):
                cls[kt, qb] = M_ZERO
            else:
                cls[kt, qb] = M_ADD
    return cls


# constants tensor layout (bf16, [24, 344]):
#  [:, 0:128]   E_A: row e, col j -> 1 if (j<64 and e==j%8) or (j>=64 and e-8==j%8)
#  [:, 128:192] E_v: row e, col j -> 1 if e-16 == j%8
#  [:, 192:216] ones24: block-diag 3x(8x8 ones)
#  [0:8, 216:280] E8o: row e, col j -> 1 if e == j%8
#  [0:1, 280:344] ones64 row
CST_W = 344


def _build_cst():
    cst = np.zeros((24, CST_W), dtype=np.float32)
    for j in range(64):
        cst[j % 8, j] = 1.0           # E_A q half
        cst[8 + j % 8, 64 + j] = 1.0  # E_A k half
        cst[16 + j % 8, 128 + j] = 1.0  # E_v
        cst[j % 8, 216 + j] = 1.0     # E8o
        cst[0, 280 + j] = 1.0         # ones64
    for b in range(3):
        cst[8 * b:8 * b + 8, 192 + 8 * b:200 + 8 * b] = 1.0  # ones24
    return _bf(cst)


def build(mask_cls):
    """Build the SPMD Bass graph. mask_cls: [NKT, NQB] int array."""
    nc = bacc.Bacc(None, target_bir_lowering=False)

    # ---- DRAM I/O (per-core shards prepared on host) ----
    xT = nc.declare_dram_parameter("xT", [D, S], BF16, isOutput=False)
    wqT = nc.declare_dram_parameter("wqT", [D, QF], BF16, isOutput=False)
    wkvT = nc.declare_dram_parameter("wkvT", [D, 2 * KF], BF16, isOutput=False)
    aA = nc.declare_dram_parameter("aA", [D, 128], BF16, isOutput=False)
    aB = nc.declare_dram_parameter("aB", [D, 88], BF16, isOutput=False)
    ao = nc.declare_dram_parameter("ao", [D, 72], BF16, isOutput=False)
    bq = nc.declare_dram_parameter("bq", [E * R, QF], BF16, isOutput=False)
    bk = nc.declare_dram_parameter("bk", [E * R, KF], BF16, isOutput=False)
    bv = nc.declare_dram_parameter("bv", [E * R, KF], BF16, isOutput=False)
    bo = nc.declare_dram_parameter("bo", [E * R, D], BF16, isOutput=False)
    woT = nc.declare_dram_parameter("woT", [D, D], BF16, isOutput=False)
    cs4 = nc.declare_dram_parameter("cs4", [128, S], BF16, isOutput=False)
    sn4 = nc.declare_dram_parameter("sn4", [128, S], BF16, isOutput=False)
    mask01 = nc.declare_dram_parameter("mask01", [S, S], BF16, isOutput=False)
    cst = nc.declare_dram_parameter("cst", [24, CST_W], BF16, isOutput=False)
    y = nc.declare_dram_parameter("y", [TSH, D], F32, isOutput=True)

    # internal DRAM for the chunked collectives: [dest/src, 4h*64 feat, 64 tok]
    cc_in = [nc.dram_tensor("cc_in%d" % q, [NCORES, QF, 64], BF16)
             for q in range(NQB)]
    cc_out = [nc.dram_tensor("cc_out%d" % q, [NCORES, QF, 64], BF16)
              for q in range(NQB)]

    with tile.TileContext(nc) as tc:
        _emit(nc, tc, locals(), mask_cls)
    nc.finalize()
    return nc


def _emit(nc, tc, t, mask_cls):
    xT, wqT, wkvT = t["xT"], t["wqT"], t["wkvT"]
    aA, aB, ao = t["aA"], t["aB"], t["ao"]
    bq, bk, bv, bo = t["bq"], t["bk"], t["bv"], t["bo"]
    woT, cs4, sn4, mask01, y = t["woT"], t["cs4"], t["sn4"], t["mask01"], t["y"]
    cst = t["cst"]
    cc_in, cc_out = t["cc_in"], t["cc_out"]

    import contextlib
    ctx = contextlib.ExitStack()
    with ctx:
        persist = ctx.enter_context(tc.tile_pool(name="persist", bufs=1))
        ps = ctx.enter_context(tc.tile_pool(name="ps", bufs=1, space="PSUM"))

        # ---- persistent weights, split in k-groups of 4 for early start ----
        NSP = 4
        KG = NIF // NSP
        aA_sb, aB_sb, wq_sb, wkv_sb = [], [], [], []
        for g in range(NSP):
            ksl = slice(g * KG * 128, (g + 1) * KG * 128)
            tl = persist.tile([128, KG, 128], BF16, name="aA%d" % g)
            nc.scalar.dma_start(
                out=tl, in_=aA[ksl].rearrange("(n p) f -> p n f", p=128))
            aA_sb.append(tl)
            tl = persist.tile([128, KG, 88], BF16, name="aB%d" % g)
            nc.scalar.dma_start(
                out=tl, in_=aB[ksl].rearrange("(n p) f -> p n f", p=128))
            aB_sb.append(tl)
            tl = persist.tile([128, KG, QF], BF16, name="wq%d" % g)
            nc.sync.dma_start(
                out=tl, in_=wqT[ksl].rearrange("(n p) f -> p n f", p=128))
            wq_sb.append(tl)
            tl = persist.tile([128, KG, 2 * KF], BF16, name="wkv%d" % g)
            nc.gpsimd.dma_start(
                out=tl, in_=wkvT[ksl].rearrange("(n p) f -> p n f", p=128))
            wkv_sb.append(tl)

        def A_AT(k):  # aA chain lhsT for contraction tile k
            return aA_sb[k // KG][:, k % KG, :]

        def A_BT(k):
            return aB_sb[k // KG][:, k % KG, :]

        def W_Q(k):
            return wq_sb[k // KG][:, k % KG, :]

        def W_KV(k):
            return wkv_sb[k // KG][:, k % KG, :]

        cst_sb = persist.tile([24, CST_W], BF16)
        nc.gpsimd.dma_start(out=cst_sb, in_=cst[:])
        E_A = cst_sb[:, 0:128]
        E_v = cst_sb[0:24, 128:192]
        ones24 = cst_sb[:, 192:216]
        ones8 = cst_sb[0:8, 192:200]
        E8o = cst_sb[0:8, 216:280]
        ones64 = cst_sb[0:1, 280:344]

        bq_sb = persist.tile([64, QF], BF16)
        nc.gpsimd.dma_start(out=bq_sb, in_=bq[:])
        bk_sb = persist.tile([128, KF], BF16)   # bk lives at partitions 64:128
        nc.gpsimd.dma_start(out=bk_sb[64:128, :], in_=bk[:])
        bv_sb = persist.tile([64, KF], BF16)
        nc.gpsimd.dma_start(out=bv_sb, in_=bv[:])
        bo_sb = persist.tile([64, D], BF16)
        nc.gpsimd.dma_start(out=bo_sb, in_=bo[:])
        ao_sb = persist.tile([128, NIF, 72], BF16)
        nc.scalar.dma_start(out=ao_sb,
                            in_=ao.rearrange("(n p) f -> p n f", p=128))
        cs_sb = persist.tile([128, S], BF16)
        nc.scalar.dma_start(out=cs_sb, in_=cs4[:])
        sn_sb = persist.tile([128, S], BF16)
        nc.scalar.dma_start(out=sn_sb, in_=sn4[:])

        ident_b = persist.tile([128, 128], BF16)
        make_identity(nc, ident_b)

        # attention operands (persist across phases)
        qh_sb = persist.tile([128, 2, S], BF16)   # [2 heads x 64, page, S]
        kh_sb = persist.tile([128, S], BF16)      # kv head duplicated 2x
        vtok = persist.tile([128, NKT, 65], BF16)  # token-major v + ones col
        nc.vector.memset(vtok[:, :, 64:65], 1.0)
        g_sb = persist.tile([128, NIF, TSH], BF16)  # gathered out (post-A2A)

        # ================= Phase A: QKV + LoRA + RoPE =================
        vT_all = persist.tile([64, S], BF16)      # v (feat-major) staging
        with tc.tile_pool(name="pA", bufs=1) as pA:
            for tb in range(4):
                tsl = slice(tb * 512, (tb + 1) * 512)
                xq = pA.tile([128, NIF, 512], BF16, name="xq", tag="xq",
                             bufs=2)
                if tb == 0:
                    # split first block's load so matmuls start early
                    for g in range(NSP):
                        ksl = slice(g * KG * 128, (g + 1) * KG * 128)
                        nc.sync.dma_start(
                            out=xq[:, g * KG:(g + 1) * KG, :],
                            in_=xT[ksl].rearrange(
                                "(n p) t -> p n t", p=128)[:, :, tsl])
                else:
                    nc.sync.dma_start(
                        out=xq,
                        in_=xT.rearrange("(n p) t -> p n t", p=128)[:, :, tsl])

                # ---- main projection chains ----
                hA = ps.tile([128, 512], F32, name="hA", tag="p_hA")
                hB = ps.tile([88, 512], F32, name="hB", tag="p_hB")
                q0 = ps.tile([128, 512], F32, name="q0", tag="p_q0")
                q1 = ps.tile([128, 512], F32, name="q1", tag="p_q1")
                kv = ps.tile([128, 512], F32, name="kv", tag="p_kv")
                for k in range(NIF):
                    st = k == 0
                    sp = k == NIF - 1
                    rhs = xq[:, k, :]
                    nc.tensor.matmul(hA, A_AT(k), rhs, start=st, stop=sp)
                    nc.tensor.matmul(hB, A_BT(k), rhs, start=st, stop=sp)
                for k in range(NIF):
                    rhs = xq[:, k, :]
                    st = k == 0
                    nc.tensor.matmul(q0, W_Q(k)[:, 0:128], rhs,
                                     start=st, stop=False)
                    nc.tensor.matmul(q1, W_Q(k)[:, 128:256], rhs,
                                     start=st, stop=False)
                    nc.tensor.matmul(kv, W_KV(k), rhs, start=st, stop=False)

                # ---- router softmax (q,k,v fused; no transposes) ----
                ex3 = pA.tile([24, 512], BF16, name="ex3", tag="ex3", bufs=2)
                nc.scalar.activation(ex3, hB[64:88, :], AF.Exp)
                s3 = ps.tile([24, 512], F32, name="s3", tag="p_s3")
                nc.tensor.matmul(s3, ones24, ex3, start=True, stop=True)
                rec3 = pA.tile([24, 512], BF16, name="rec3", tag="rec3",
                               bufs=2)
                with nc.allow_low_precision(reason="router softmax denom"):
                    nc.vector.reciprocal(rec3, s3)
                rw3 = pA.tile([24, 512], BF16, name="rw3", tag="rw3", bufs=2)
                nc.vector.tensor_tensor(rw3, ex3, rec3, AluOpType.mult)
                rwbA = ps.tile([128, 512], F32, name="rwbA", tag="p_rwA")
                nc.tensor.matmul(rwbA, E_A, rw3, start=True, stop=True)
                rwbV = ps.tile([64, 512], F32, name="rwbV", tag="p_rwV")
                nc.tensor.matmul(rwbV, E_v, rw3, start=True, stop=True)
                rwbA_s = pA.tile([128, 512], BF16, name="rwbA_s",
                                 tag="rwbA_s", bufs=2)
                nc.scalar.activation(rwbA_s, rwbA, AF.Copy)
                rwbV_s = pA.tile([64, 512], BF16, name="rwbV_s",
                                 tag="rwbV_s", bufs=2)
                nc.scalar.activation(rwbV_s, rwbV, AF.Copy)
                hpA = pA.tile([128, 512], BF16, name="hpA", tag="hpA", bufs=2)
                nc.vector.tensor_tensor(hpA, hA, rwbA_s, AluOpType.mult)
                hpV = pA.tile([64, 512], BF16, name="hpV", tag="hpV", bufs=2)
                nc.vector.tensor_tensor(hpV, hB[0:64, :], rwbV_s,
                                        AluOpType.mult)

                # ---- LoRA-B closes the accumulations ----
                nc.tensor.matmul(q0, bq_sb[:, 0:128], hpA[0:64, :],
                                 start=False, stop=True)
                nc.tensor.matmul(q1, bq_sb[:, 128:256], hpA[0:64, :],
                                 start=False, stop=True)
                nc.tensor.matmul(kv[0:64, :], bk_sb[64:128, :],
                                 hpA[64:128, :], start=False, stop=True,
                                 tile_position=(64, 0))
                nc.tensor.matmul(kv[64:128, :], bv_sb, hpV,
                                 start=False, stop=True,
                                 tile_position=(0, 64))

                # ---- PSUM extraction (scalar engine) ----
                qe = pA.tile([128, 512], BF16, name="qe", tag="qe", bufs=2)
                qo = pA.tile([128, 512], BF16, name="qo", tag="qo", bufs=2)
                nc.scalar.activation(qe[0:64, :], q0[0:64, :], AF.Copy)
                nc.scalar.activation(qe[64:128, :], q1[0:64, :], AF.Copy)
                nc.scalar.activation(qo[0:64, :], q0[64:128, :], AF.Copy)
                nc.scalar.activation(qo[64:128, :], q1[64:128, :], AF.Copy)
                ke = pA.tile([32, 512], BF16, name="ke", tag="ke", bufs=2)
                ko = pA.tile([32, 512], BF16, name="ko", tag="ko", bufs=2)
                nc.scalar.activation(ke, kv[0:32, :], AF.Copy)
                nc.scalar.activation(ko, kv[32:64, :], AF.Copy)
                nc.scalar.activation(vT_all[:, tsl], kv[64:128, :], AF.Copy)

                # ---- RoPE (bf16, vector engine) ----
                cs_t = cs_sb[:, tsl]
                sn_t = sn_sb[:, tsl]
                t1 = pA.tile([128, 512], BF16, name="t1", tag="t1", bufs=2)
                t2 = pA.tile([128, 512], BF16, name="t2", tag="t2", bufs=2)
                rote = pA.tile([128, 512], BF16, name="rote", tag="rote",
                               bufs=2)
                roto = pA.tile([128, 512], BF16, name="roto", tag="roto",
                               bufs=2)
                nc.vector.tensor_tensor(t1, qe, cs_t, AluOpType.mult)
                nc.vector.tensor_tensor(t2, qo, sn_t, AluOpType.mult)
                nc.vector.tensor_tensor(rote, t1, t2, AluOpType.subtract)
                nc.vector.tensor_tensor(t1, qe, sn_t, AluOpType.mult)
                nc.vector.tensor_tensor(t2, qo, cs_t, AluOpType.mult)
                nc.vector.tensor_tensor(roto, t1, t2, AluOpType.add)
                k1 = pA.tile([32, 512], BF16, name="k1", tag="k1", bufs=2)
                k2 = pA.tile([32, 512], BF16, name="k2", tag="k2", bufs=2)
                csk = cs_sb[0:32, tsl]
                snk = sn_sb[0:32, tsl]
                nc.vector.tensor_tensor(k1, ke, csk, AluOpType.mult)
                nc.vector.tensor_tensor(k2, ko, snk, AluOpType.mult)
                nc.vector.tensor_tensor(kh_sb[0:32, tsl], k1, k2,
                                        AluOpType.subtract)
                nc.vector.tensor_tensor(k1, ke, snk, AluOpType.mult)
                nc.vector.tensor_tensor(k2, ko, csk, AluOpType.mult)
                nc.vector.tensor_tensor(kh_sb[32:64, tsl], k1, k2,
                                        AluOpType.add)

                # head rearrange via SBUF->SBUF DMA (off the engines)
                nc.gpsimd.dma_start(out=kh_sb[64:128, tsl],
                                    in_=kh_sb[0:64, tsl])
                for h in range(QH):
                    page, i = h // 2, h % 2
                    nc.gpsimd.dma_start(
                        out=qh_sb[64 * i:64 * i + 32, page, tsl],
                        in_=rote[32 * h:32 * h + 32, :])
                    nc.gpsimd.dma_start(
                        out=qh_sb[64 * i + 32:64 * i + 64, page, tsl],
                        in_=roto[32 * h:32 * h + 32, :])

            # token-major v, built after the per-block pipeline drains
            for kt in range(NKT):
                v_ps = ps.tile([128, 64], BF16, name="v_ps", tag="p_s3")
                nc.tensor.transpose(v_ps,
                                    vT_all[:, 128 * kt:128 * kt + 128],
                                    ident_b[0:64, 0:64])
                nc.vector.tensor_copy(vtok[:, kt, 0:64], v_ps)

        # prefetch the full output-projection weight during attention
        wo_ctx = tc.tile_pool(name="wo_pool", bufs=4)
        wo_pool = wo_ctx.__enter__()
        wo_tiles = []
        for ob in range(4):
            osl = slice(ob * 512, (ob + 1) * 512)
            wo_sb = wo_pool.tile([128, NIF, 512], BF16, name="wo_sb",
                                 tag="wo", bufs=4)
            nc.gpsimd.dma_start(
                out=wo_sb,
                in_=woT.rearrange("(n p) f -> p n f", p=128)[:, :, osl])
            wo_tiles.append(wo_sb)

        # ============ Phase C: attention + chunked A2A ============
        SC_TAGS = ["p_q0", "p_q1", "p_hA", "p_hB"]
        OUT_TAGS = ["p_kv", "p_s3", "p_rwA", "p_rwV"]
        with tc.tile_pool(name="pC", bufs=1) as pC:
            def emit_tail(qb, outps):
                """Normalize + ship chunk qb. Emitted after the next query
                block's first score/exp round so the reciprocal latency
                hides under attention compute."""
                for h in range(QH):
                    rec = pC.tile([1, 512], BF16, name="rec%d" % h,
                                  tag="rec%d" % h, bufs=2)
                    with nc.allow_low_precision(reason="attn denom"):
                        nc.vector.reciprocal(rec, outps[h][64:65, :])
                    rb = ps.tile([64, 512], F32, name="rb", tag=SC_TAGS[h])
                    nc.tensor.matmul(rb, ones64, rec, start=True, stop=True)
                    rb_s = pC.tile([64, 512], BF16, name="rb_s%d" % h,
                                   tag="rb_s%d" % h, bufs=2)
                    nc.vector.tensor_copy(rb_s, rb)
                    o65 = pC.tile([64, 512], BF16, name="o65%d" % h,
                                  tag="o65%d" % h, bufs=2)
                    nc.vector.tensor_tensor(o65, outps[h][0:64, :], rb_s,
                                            AluOpType.mult)
                    # [64, 512] -> cc_in[qb][dest, 64h:64h+64, 0:64]
                    nc.gpsimd.dma_start(
                        out=cc_in[qb][:, 64 * h:64 * h + 64, :]
                            .rearrange("d p t -> p d t"),
                        in_=o65)
                nc.gpsimd.collective_compute(
                    "AllToAll",
                    AluOpType.bypass,
                    ins=[cc_in[qb][:]],
                    outs=[cc_out[qb][:]],
                    replica_groups=[list(range(NCORES))],
                )
                # gather this chunk into g_sb[:, :, 64qb:64qb+64]
                nc.sync.dma_start(
                    out=g_sb[:, :, 64 * qb:64 * qb + 64],
                    in_=cc_out[qb].rearrange("s (k p) t -> p (s k) t",
                                             k=2, p=128))

            pending = None
            for qb in range(NQB):
                qsl = slice(qb * 512, (qb + 1) * 512)
                active = [kt for kt in range(NKT)
                          if mask_cls[kt, qb] != M_SKIP]
                assert active, f"fully masked query block qb={qb}"
                outps = [ps.tile([65, 512], F32, name="outp%d" % h,
                                 tag=OUT_TAGS[h]) for h in range(QH)]
                for idx, kt in enumerate(active):
                    c = mask_cls[kt, qb]
                    mt = None
                    if c == M_ADD:
                        mt = pC.tile([128, 512], BF16, name="mt",
                                     tag="mt", bufs=4)
                        nc.sync.dma_start(
                            out=mt,
                            in_=mask01[128 * kt:128 * kt + 128, qsl])
                    ksl = slice(128 * kt, 128 * kt + 128)
                    prs = []
                    for h in range(QH):
                        page, i = h // 2, h % 2
                        sc = ps.tile([128, 512], F32, name="sc%d" % h,
                                     tag=SC_TAGS[h])
                        nc.tensor.matmul(sc,
                                         kh_sb[64 * i:64 * i + 64, ksl],
                                         qh_sb[64 * i:64 * i + 64, page, qsl],
                                         start=True, stop=True,
                                         tile_position=(64 * i, 0))
                        pr = pC.tile([128, 512], BF16, name="pr%d" % h,
                                     tag="pr%d" % h, bufs=2)
                        nc.scalar.activation(pr, sc, AF.Exp)
                        if mt is not None:
                            nc.vector.tensor_tensor(pr, pr, mt,
                                                    AluOpType.mult)
                        prs.append(pr)
                    if idx == 0 and pending is not None:
                        # previous block's normalization rides behind this
                        # round's score matmuls
                        emit_tail(qb - 1, pending)
                        pending = None
                    for h in range(QH):
                        nc.tensor.matmul(outps[h], vtok[:, kt, :], prs[h],
                                         start=(kt == active[0]),
                                         stop=(kt == active[-1]))
                pending = outps
            emit_tail(NQB - 1, pending)

        # ================= Phase D: o-proj =================
        with tc.tile_pool(name="pD", bufs=1) as pD:
            ho = ps.tile([72, TSH], F32, name="ho", tag="p_hA")
            for k in range(NIF):
                nc.tensor.matmul(ho, ao_sb[:, k, :], g_sb[:, k, :],
                                 start=(k == 0), stop=(k == NIF - 1))
            exo = pD.tile([8, TSH], BF16, name="exo")
            nc.scalar.activation(exo, ho[64:72, :], AF.Exp)
            so = ps.tile([8, TSH], F32, name="so", tag="p_s3")
            nc.tensor.matmul(so, ones8, exo, start=True, stop=True)
            reco = pD.tile([8, TSH], BF16, name="reco")
            with nc.allow_low_precision(reason="o-router softmax denom"):
                nc.vector.reciprocal(reco, so)
            rwo = pD.tile([8, TSH], BF16, name="rwo")
            nc.vector.tensor_tensor(rwo, exo, reco, AluOpType.mult)
            rwbo = ps.tile([64, TSH], F32, name="rwbo", tag="p_hB")
            nc.tensor.matmul(rwbo, E8o, rwo, start=True, stop=True)
            rwbo_s = pD.tile([64, TSH], BF16, name="rwbo_s")
            nc.vector.tensor_copy(rwbo_s, rwbo)
            hpo = pD.tile([64, TSH], BF16, name="hpo")
            nc.vector.tensor_tensor(hpo, ho[0:64, :], rwbo_s, AluOpType.mult)

            for ob in range(4):
                osl = slice(ob * 512, (ob + 1) * 512)
                wo_sb = wo_tiles[ob]
                for tt in range(2):
                    yp = ps.tile([128, 512], F32, name="yp",
                                 tag="p_q0" if (2 * ob + tt) % 2 == 0
                                 else "p_q1")
                    for k in range(NIF):
                        nc.tensor.matmul(
                            yp, g_sb[:, k, 128 * tt:128 * tt + 128],
                            wo_sb[:, k, :], start=(k == 0), stop=False)
                    nc.tensor.matmul(yp, hpo[:, 128 * tt:128 * tt + 128],
                                     bo_sb[:, osl], start=False, stop=True)
                    yt = pD.tile([128, 512], F32, name="yt", tag="yt",
                                 bufs=3)
                    if (2 * ob + tt) % 2 == 0:
                        nc.scalar.activation(yt, yp, AF.Copy)
                    else:
                        nc.vector.tensor_copy(yt, yp)
                    nc.sync.dma_start(out=y[128 * tt:128 * tt + 128, osl],
                                      in_=yt)
        wo_ctx.__exit__(None, None, None)


# ======================= host side =======================

_CACHE = {}


def _prep_inputs(x, mask, freqs_cos, freqs_sin, wq, wk, wv, wo,
                 lq_router, lq_A, lq_B, lk_router, lk_A, lk_B,
                 lv_router, lv_A, lv_B, lo_router, lo_A, lo_B):
    scale = 1.0 / np.sqrt(HD)
    x = _f32(np.asarray(x)).reshape(S, D)
    maskf = _f32(np.asarray(mask)).reshape(S, S)
    maskT = np.maximum(maskf, MASK_NEG).T.copy()
    mask_cls = classify_mask(maskT)
    mask01 = _bf((maskT > MASK_NEG * 0.5).astype(np.float32))

    xT = _bf(x.T)
    cs4 = _bf(np.tile(_f32(freqs_cos).T, (4, 1)))      # [128, S]
    sn4 = _bf(np.tile(_f32(freqs_sin).T, (4, 1)))
    woT = _bf(_f32(wo).T)
    ao_p = _bf(np.concatenate([_a_pack(_f32(lo_A)), _f32(lo_router).T],
                              axis=1))                 # [D, 72]
    bo_f = _bf(_b_flat(_f32(lo_B), SCALING))

    # fused LoRA-A stationaries: [D, 128] = [aq|ak], [D, 88] = [av|rq|rk|rv]
    aA_p = _bf(np.concatenate(
        [_a_pack(_f32(lq_A)), _a_pack(_f32(lk_A))], axis=1))
    aB_p = _bf(np.concatenate(
        [_a_pack(_f32(lv_A)), _f32(lq_router).T, _f32(lk_router).T,
         _f32(lv_router).T], axis=1))

    shared = dict(xT=xT, cs4=cs4, sn4=sn4, woT=woT, mask01=mask01,
                  ao=ao_p, bo=bo_f, cst=_build_cst(), aA=aA_p, aB=aB_p)

    wqf, wkf, wvf = _f32(wq), _f32(wk), _f32(wv)
    lqB, lkB, lvB = _f32(lq_B), _f32(lk_B), _f32(lv_B)

    in_maps = []
    for c in range(NCORES):
        wq_c = wqf[c * QF:(c + 1) * QF][IDX_Q] * scale
        wk_c = wkf[c * KF:(c + 1) * KF][IDX_K]
        wv_c = wvf[c * KF:(c + 1) * KF]
        bq_c = _b_flat(lqB[:, c * QF:(c + 1) * QF, :][:, IDX_Q, :],
                       SCALING * scale)
        bk_c = _b_flat(lkB[:, c * KF:(c + 1) * KF, :][:, IDX_K, :], SCALING)
        bv_c = _b_flat(lvB[:, c * KF:(c + 1) * KF, :], SCALING)
        m = dict(shared)
        m.update(wqT=_bf(wq_c.T),
                 wkvT=_bf(np.concatenate([wk_c.T, wv_c.T], axis=1)),
                 bq=_bf(bq_c), bk=_bf(bk_c), bv=_bf(bv_c))
        in_maps.append(m)
    return in_maps, mask_cls


def get_graph(mask_cls):
    key = mask_cls.tobytes()
    if key not in _CACHE:
        _CACHE[key] = build(mask_cls)
    return _CACHE[key]


def kernel(x, start_pos, mask, freqs_cos, freqs_sin, wq, wk, wv, wo,
           lq_router, lq_A, lq_B, lk_router, lk_A, lk_B,
           lv_router, lv_A, lv_B, lo_router, lo_A, lo_B,
           _trace=False):
    from concourse.bass_utils import run_bass_kernel_spmd
    in_maps, mask_cls = _prep_inputs(
        x, mask, freqs_cos, freqs_sin, wq, wk, wv, wo,
        lq_router, lq_A, lq_B, lk_router, lk_A, lk_B,
        lv_router, lv_A, lv_B, lo_router, lo_A, lo_B)
    nc = get_graph(mask_cls)
    res = run_bass_kernel_spmd(nc, in_maps, list(range(NCORES)), trace=_trace)
    # core c's y rows: group g (0..3) covers tokens [512g + 64c, 512g + 64c + 64)
    ys = np.stack([res.results[c]["y"] for c in range(NCORES)], axis=0)
    ys = ys.reshape(NCORES, 4, 64, D).transpose(1, 0, 2, 3).reshape(S, D)
    out = ys.reshape(B, S, H * HD).astype(np.float32)
    if _trace:
        return out, res
    return out


# revision 15
# speedup vs baseline: 1.2784x; 1.0261x over previous
"""Trainium2 Bass kernel for MoE-LoRA GQA attention (nn_Attention_57389353009692).

V2 strategy (8 NeuronCores, one SPMD launch):
  - Tensor-parallel over heads: core c owns q-heads 4c..4c+3 and kv-head c.
  - Phase A (per 512-token block): QKV projections (+ MoE-LoRA) with packed
    matmul chains (wk|wv fused; LoRA-A for q/k/v + all three routers fused
    into two chains of 128/88 rows), router softmax done with
    exp -> ones-matmul row-sum -> reciprocal -> broadcast-matmul (no
    transposes, no DRAM bounce), RoPE in bf16 on 128 partitions.
  - Phase C: flash-style attention per 512-query block; causal mask applied
    as a 0/1 multiply after exp (bf16); output normalized PRE-collective via
    reciprocal-of-denominator broadcast matmuls fused into the PSUM->SBUF
    cast.
  - AllToAll is chunked per query block (4 collectives) and overlaps the
    remaining attention compute. Output tokens are interleaved at
    64-granularity: core c owns tokens {t : (t//64) % 8 == c} so every chunk
    is a uniform 8-way exchange.
  - Phase D: o-projection + o-LoRA for the core's 256 tokens with the full
    (prefetched) wo.

Numerics: bf16 operands, fp32 PSUM accumulation, fp32->exp softmax without
max subtraction (scores are O(1) here; masked entries are zeroed exactly by
the 0/1 multiply). Scale 1/sqrt(64) folded into wq and q-LoRA-B on host.
RoPE trick: interleaved even/odd pairs are made contiguous by permuting
wq/wk output features on host (per 2-head "page": [h0e|h1e|h0o|h1o]).
"""

import sys

for _p in ("/opt/trn_rl_repo", "/root/.axon_site/_ro/trn_rl_repo"):
    if _p not in sys.path:
        sys.path.insert(0, _p)

import numpy as np
import ml_dtypes

import concourse.bass as bass
import concourse.tile as tile
from concourse import bacc, mybir
from concourse.masks import make_identity
from concourse.alu_op_type import AluOpType

F32 = mybir.dt.float32
BF16 = mybir.dt.bfloat16
AF = mybir.ActivationFunctionType
AX = mybir.AxisListType
BF16NP = ml_dtypes.bfloat16

B, S, D = 1, 2048, 2048
H, KVH, HD = 32, 8, 64
NREP = H // KVH
R, E = 8, 8
SCALING = 32.0 / 8.0
NCORES = 8
QH = H // NCORES          # 4 q heads per core
QF = QH * HD              # 256 q feats per core
KF = HD                   # 64 kv feats per core
TSH = S // NCORES         # 256 tokens per core for o-proj
NKT = S // 128            # 16 key tiles
NQB = S // 512            # 4 query blocks
NIF = D // 128            # 16 contraction tiles

MASK_NEG = -1e30

# mask tile classes
M_SKIP, M_ZERO, M_ADD = 0, 1, 2


def _build_perm():
    """Per-core feature permutations for rope-friendly layout."""
    idx_q = np.zeros(QF, dtype=np.int64)
    for f in range(QF):
        page, w = divmod(f, 128)
        if w < 32:
            hl, j, odd = 2 * page, w, 0
        elif w < 64:
            hl, j, odd = 2 * page + 1, w - 32, 0
        elif w < 96:
            hl, j, odd = 2 * page, w - 64, 1
        else:
            hl, j, odd = 2 * page + 1, w - 96, 1
        idx_q[f] = 64 * hl + 2 * j + odd
    idx_k = np.zeros(KF, dtype=np.int64)
    for w in range(KF):
        if w < 32:
            idx_k[w] = 2 * w
        else:
            idx_k[w] = 2 * (w - 32) + 1
    return idx_q, idx_k


IDX_Q, IDX_K = _build_perm()


def _a_pack(A):
    """[E,R,D] -> [D, 64] with col r*8+e."""
    return np.transpose(A, (1, 0, 2)).reshape(E * R, -1).T


def _b_flat(Bw, scale):
    """[E, OF, R] -> [64, OF] with row r*8+e."""
    return (np.transpose(Bw, (2, 0, 1)).reshape(E * R, -1) * scale)


def _bf(x):
    return np.ascontiguousarray(x, dtype=np.float32).astype(BF16NP)


def _f32(x):
    return np.ascontiguousarray(x, dtype=np.float32)


def classify_mask(maskT):
    """maskT: [S(k), S(q)] clamped fp32. Returns [NKT, NQB] class map."""
    cls = np.zeros((NKT, NQB), dtype=np.int64)
    for kt in range(NKT):
        blk_rows = maskT[kt * 128:(kt + 1) * 128]
        for qb in range(NQB):
            blk = blk_rows[:, qb * 512:(qb + 1) * 512]
            if np.all(blk <= MASK_NEG * 0.5):
                cls[kt, qb] = M_SKIP
            elif np.all(blk == 0.0):
                cls[kt, qb] = M_ZERO
            else:
                cls[kt, qb] = M_ADD
    return cls


# constants tensor layout (bf16, [24, 344]):
#  [:, 0:128]   E_A: row e, col j -> 1 if (j<64 and e==j%8) or (j>=64 and e-8==j%8)
#  [:, 128:192] E_v: row e, col j -> 1 if e-16 == j%8
#  [:, 192:216] ones24: block-diag 3x(8x8 ones)
#  [0:8, 216:280] E8o: row e, col j -> 1 if e == j%8
#  [0:1, 280:344] ones64 row
CST_W = 344


def _build_cst():
    cst = np.zeros((24, CST_W), dtype=np.float32)
    for j in range(64):
        cst[j % 8, j] = 1.0           # E_A q half
        cst[8 + j % 8, 64 + j] = 1.0  # E_A k half
        cst[16 + j % 8, 128 + j] = 1.0  # E_v
        cst[j % 8, 216 + j] = 1.0     # E8o
        cst[0, 280 + j] = 1.0         # ones64
    for b in range(3):
        cst[8 * b:8 * b + 8, 192 + 8 * b:200 + 8 * b] = 1.0  # ones24
    return _bf(cst)


def build(mask_cls):
    """Build the SPMD Bass graph. mask_cls: [NKT, NQB] int array."""
    nc = bacc.Bacc(None, target_bir_lowering=False)

    # ---- DRAM I/O (per-core shards prepared on host) ----
    xT = nc.declare_dram_parameter("xT", [D, S], BF16, isOutput=False)
    wqT = nc.declare_dram_parameter("wqT", [D, QF], BF16, isOutput=False)
    wkvT = nc.declare_dram_parameter("wkvT", [D, 2 * KF], BF16, isOutput=False)
    aA = nc.declare_dram_parameter("aA", [D, 128], BF16, isOutput=False)
    aB = nc.declare_dram_parameter("aB", [D, 88], BF16, isOutput=False)
    ao = nc.declare_dram_parameter("ao", [D, 72], BF16, isOutput=False)
    bq = nc.declare_dram_parameter("bq", [E * R, QF], BF16, isOutput=False)
    bk = nc.declare_dram_parameter("bk", [E * R, KF], BF16, isOutput=False)
    bv = nc.declare_dram_parameter("bv", [E * R, KF], BF16, isOutput=False)
    bo = nc.declare_dram_parameter("bo", [E * R, D], BF16, isOutput=False)
    woT = nc.declare_dram_parameter("woT", [D, D], BF16, isOutput=False)
    cs4 = nc.declare_dram_parameter("cs4", [128, S], BF16, isOutput=False)
    sn4 = nc.declare_dram_parameter("sn4", [128, S], BF16, isOutput=False)
    mask01 = nc.declare_dram_parameter("mask01", [S, S], BF16, isOutput=False)
    cst = nc.declare_dram_parameter("cst", [24, CST_W], BF16, isOutput=False)
    y = nc.declare_dram_parameter("y", [TSH, D], F32, isOutput=True)

    # internal DRAM for the chunked collectives: [dest/src, 4h*64 feat, 64 tok]
    cc_in = [nc.dram_tensor("cc_in%d" % q, [NCORES, QF, 64], BF16)
             for q in range(NQB)]
    cc_out = [nc.dram_tensor("cc_out%d" % q, [NCORES, QF, 64], BF16)
              for q in range(NQB)]

    with tile.TileContext(nc) as tc:
        _emit(nc, tc, locals(), mask_cls)
    nc.finalize()
    return nc


def _emit(nc, tc, t, mask_cls):
    xT, wqT, wkvT = t["xT"], t["wqT"], t["wkvT"]
    aA, aB, ao = t["aA"], t["aB"], t["ao"]
    bq, bk, bv, bo = t["bq"], t["bk"], t["bv"], t["bo"]
    woT, cs4, sn4, mask01, y = t["woT"], t["cs4"], t["sn4"], t["mask01"], t["y"]
    cst = t["cst"]
    cc_in, cc_out = t["cc_in"], t["cc_out"]

    import contextlib
    ctx = contextlib.ExitStack()
    with ctx:
        persist = ctx.enter_context(tc.tile_pool(name="persist", bufs=1))
        ps = ctx.enter_context(tc.tile_pool(name="ps", bufs=1, space="PSUM"))

        # ---- persistent weights, split in k-groups of 4 for early start ----
        NSP = 4
        KG = NIF // NSP
        aA_sb, aB_sb, wq_sb, wkv_sb = [], [], [], []
        for g in range(NSP):
            ksl = slice(g * KG * 128, (g + 1) * KG * 128)
            tl = persist.tile([128, KG, 128], BF16, name="aA%d" % g)
            nc.scalar.dma_start(
                out=tl, in_=aA[ksl].rearrange("(n p) f -> p n f", p=128))
            aA_sb.append(tl)
            tl = persist.tile([128, KG, 88], BF16, name="aB%d" % g)
            nc.scalar.dma_start(
                out=tl, in_=aB[ksl].rearrange("(n p) f -> p n f", p=128))
            aB_sb.append(tl)
            tl = persist.tile([128, KG, QF], BF16, name="wq%d" % g)
            nc.sync.dma_start(
                out=tl, in_=wqT[ksl].rearrange("(n p) f -> p n f", p=128))
            wq_sb.append(tl)
            tl = persist.tile([128, KG, 2 * KF], BF16, name="wkv%d" % g)
            nc.gpsimd.dma_start(
                out=tl, in_=wkvT[ksl].rearrange("(n p) f -> p n f", p=128))
            wkv_sb.append(tl)

        def A_AT(k):  # aA chain lhsT for contraction tile k
            return aA_sb[k // KG][:, k % KG, :]

        def A_BT(k):
            return aB_sb[k // KG][:, k % KG, :]

        def W_Q(k):
            return wq_sb[k // KG][:, k % KG, :]

        def W_KV(k):
            return wkv_sb[k // KG][:, k % KG, :]

        cst_sb = persist.tile([24, CST_W], BF16)
        nc.gpsimd.dma_start(out=cst_sb, in_=cst[:])
        E_A = cst_sb[:, 0:128]
        E_v = cst_sb[0:24, 128:192]
        ones24 = cst_sb[:, 192:216]
        ones8 = cst_sb[0:8, 192:200]
        E8o = cst_sb[0:8, 216:280]
        ones64 = cst_sb[0:1, 280:344]

        bq_sb = persist.tile([64, QF], BF16)
        nc.gpsimd.dma_start(out=bq_sb, in_=bq[:])
        bk_sb = persist.tile([128, KF], BF16)   # bk lives at partitions 64:128
        nc.gpsimd.dma_start(out=bk_sb[64:128, :], in_=bk[:])
        bv_sb = persist.tile([64, KF], BF16)
        nc.gpsimd.dma_start(out=bv_sb, in_=bv[:])
        bo_sb = persist.tile([64, D], BF16)
        nc.gpsimd.dma_start(out=bo_sb, in_=bo[:])
        ao_sb = persist.tile([128, NIF, 72], BF16)
        nc.scalar.dma_start(out=ao_sb,
                            in_=ao.rearrange("(n p) f -> p n f", p=128))
        cs_sb = persist.tile([128, S], BF16)
        nc.scalar.dma_start(out=cs_sb, in_=cs4[:])
        sn_sb = persist.tile([128, S], BF16)
        nc.scalar.dma_start(out=sn_sb, in_=sn4[:])

        ident_b = persist.tile([128, 128], BF16)
        make_identity(nc, ident_b)

        # attention operands (persist across phases)
        qh_sb = persist.tile([128, 2, S], BF16)   # [2 heads x 64, page, S]
        kh_sb = persist.tile([128, S], BF16)      # kv head duplicated 2x
        vtok = persist.tile([128, NKT, 65], BF16)  # token-major v + ones col
        nc.vector.memset(vtok[:, :, 64:65], 1.0)
        g_sb = persist.tile([128, NIF, TSH], BF16)  # gathered out (post-A2A)

        # ================= Phase A: QKV + LoRA + RoPE =================
        vT_all = persist.tile([64, S], BF16)      # v (feat-major) staging
        with tc.tile_pool(name="pA", bufs=1) as pA:
            for tb in range(4):
                tsl = slice(tb * 512, (tb + 1) * 512)
                xq = pA.tile([128, NIF, 512], BF16, name="xq", tag="xq",
                             bufs=3)
                if tb == 0:
                    # split first block's load so matmuls start early
                    for g in range(NSP):
                        ksl = slice(g * KG * 128, (g + 1) * KG * 128)
                        nc.sync.dma_start(
                            out=xq[:, g * KG:(g + 1) * KG, :],
                            in_=xT[ksl].rearrange(
                                "(n p) t -> p n t", p=128)[:, :, tsl])
                else:
                    nc.sync.dma_start(
                        out=xq,
                        in_=xT.rearrange("(n p) t -> p n t", p=128)[:, :, tsl])

                # ---- main projection chains ----
                hA = ps.tile([128, 512], F32, name="hA", tag="p_hA")
                hB = ps.tile([88, 512], F32, name="hB", tag="p_hB")
                q0 = ps.tile([128, 512], F32, name="q0", tag="p_q0")
                q1 = ps.tile([128, 512], F32, name="q1", tag="p_q1")
                kv = ps.tile([128, 512], F32, name="kv", tag="p_kv")
                for k in range(NIF):
                    st = k == 0
                    sp = k == NIF - 1
                    rhs = xq[:, k, :]
                    nc.tensor.matmul(hA, A_AT(k), rhs, start=st, stop=sp)
                    nc.tensor.matmul(hB, A_BT(k), rhs, start=st, stop=sp)
                for k in range(NIF):
                    rhs = xq[:, k, :]
                    st = k == 0
                    nc.tensor.matmul(q0, W_Q(k)[:, 0:128], rhs,
                                     start=st, stop=False)
                    nc.tensor.matmul(q1, W_Q(k)[:, 128:256], rhs,
                                     start=st, stop=False)
                    nc.tensor.matmul(kv, W_KV(k), rhs, start=st, stop=False)

                # ---- router softmax (q,k,v fused; no transposes) ----
                ex3 = pA.tile([24, 512], BF16, name="ex3", tag="ex3", bufs=2)
                nc.scalar.activation(ex3, hB[64:88, :], AF.Exp)
                s3 = ps.tile([24, 512], F32, name="s3", tag="p_s3")
                nc.tensor.matmul(s3, ones24, ex3, start=True, stop=True)
                rec3 = pA.tile([24, 512], BF16, name="rec3", tag="rec3",
                               bufs=2)
                with nc.allow_low_precision(reason="router softmax denom"):
                    nc.vector.reciprocal(rec3, s3)
                rw3 = pA.tile([24, 512], BF16, name="rw3", tag="rw3", bufs=2)
                nc.vector.tensor_tensor(rw3, ex3, rec3, AluOpType.mult)
                rwbA = ps.tile([128, 512], F32, name="rwbA", tag="p_rwA")
                nc.tensor.matmul(rwbA, E_A, rw3, start=True, stop=True)
                rwbV = ps.tile([64, 512], F32, name="rwbV", tag="p_rwV")
                nc.tensor.matmul(rwbV, E_v, rw3, start=True, stop=True)
                rwbA_s = pA.tile([128, 512], BF16, name="rwbA_s",
                                 tag="rwbA_s", bufs=2)
                nc.scalar.activation(rwbA_s, rwbA, AF.Copy)
                rwbV_s = pA.tile([64, 512], BF16, name="rwbV_s",
                                 tag="rwbV_s", bufs=2)
                nc.scalar.activation(rwbV_s, rwbV, AF.Copy)
                hpA = pA.tile([128, 512], BF16, name="hpA", tag="hpA", bufs=2)
                nc.vector.tensor_tensor(hpA, hA, rwbA_s, AluOpType.mult)
                hpV = pA.tile([64, 512], BF16, name="hpV", tag="hpV", bufs=2)
                nc.vector.tensor_tensor(hpV, hB[0:64, :], rwbV_s,
                                        AluOpType.mult)

                # ---- LoRA-B closes the accumulations ----
                nc.tensor.matmul(q0, bq_sb[:, 0:128], hpA[0:64, :],
                                 start=False, stop=True)
                nc.tensor.matmul(q1, bq_sb[:, 128:256], hpA[0:64, :],
                                 start=False, stop=True)
                nc.tensor.matmul(kv[0:64, :], bk_sb[64:128, :],
                                 hpA[64:128, :], start=False, stop=True,
                                 tile_position=(64, 0))
                nc.tensor.matmul(kv[64:128, :], bv_sb, hpV,
                                 start=False, stop=True,
                                 tile_position=(0, 64))

                # ---- PSUM extraction (scalar engine) ----
                qe = pA.tile([128, 512], BF16, name="qe", tag="qe", bufs=2)
                qo = pA.tile([128, 512], BF16, name="qo", tag="qo", bufs=2)
                nc.scalar.activation(qe[0:64, :], q0[0:64, :], AF.Copy)
                nc.scalar.activation(qe[64:128, :], q1[0:64, :], AF.Copy)
                nc.scalar.activation(qo[0:64, :], q0[64:128, :], AF.Copy)
                nc.scalar.activation(qo[64:128, :], q1[64:128, :], AF.Copy)
                ke = pA.tile([32, 512], BF16, name="ke", tag="ke", bufs=2)
                ko = pA.tile([32, 512], BF16, name="ko", tag="ko", bufs=2)
                nc.scalar.activation(ke, kv[0:32, :], AF.Copy)
                nc.scalar.activation(ko, kv[32:64, :], AF.Copy)
                nc.scalar.activation(vT_all[:, tsl], kv[64:128, :], AF.Copy)

                # ---- RoPE (bf16, vector engine) ----
                cs_t = cs_sb[:, tsl]
                sn_t = sn_sb[:, tsl]
                t1 = pA.tile([128, 512], BF16, name="t1", tag="t1", bufs=2)
                t2 = pA.tile([128, 512], BF16, name="t2", tag="t2", bufs=2)
                rote = pA.tile([128, 512], BF16, name="rote", tag="rote",
                               bufs=2)
                roto = pA.tile([128, 512], BF16, name="roto", tag="roto",
                               bufs=2)
                nc.vector.tensor_tensor(t1, qe, cs_t, AluOpType.mult)
                nc.vector.tensor_tensor(t2, qo, sn_t, AluOpType.mult)
                nc.vector.tensor_tensor(rote, t1, t2, AluOpType.subtract)
                nc.vector.tensor_tensor(t1, qe, sn_t, AluOpType.mult)
                nc.vector.tensor_tensor(t2, qo, cs_t, AluOpType.mult)
                nc.vector.tensor_tensor(roto, t1, t2, AluOpType.add)
                k1 = pA.tile([32, 512], BF16, name="k1", tag="k1", bufs=2)
                k2 = pA.tile([32, 512], BF16, name="k2", tag="k2", bufs=2)
                csk = cs_sb[0:32, tsl]
                snk = sn_sb[0:32, tsl]
                nc.vector.tensor_tensor(k1, ke, csk, AluOpType.mult)
                nc.vector.tensor_tensor(k2, ko, snk, AluOpType.mult)
                nc.vector.tensor_tensor(kh_sb[0:32, tsl], k1, k2,
                                        AluOpType.subtract)
                nc.vector.tensor_tensor(k1, ke, snk, AluOpType.mult)
                nc.vector.tensor_tensor(k2, ko, csk, AluOpType.mult)
                nc.vector.tensor_tensor(kh_sb[32:64, tsl], k1, k2,
                                        AluOpType.add)

                # head rearrange via SBUF->SBUF DMA (off the engines)
                nc.gpsimd.dma_start(out=kh_sb[64:128, tsl],
                                    in_=kh_sb[0:64, tsl])
                for h in range(QH):
                    page, i = h // 2, h % 2
                    nc.gpsimd.dma_start(
                        out=qh_sb[64 * i:64 * i + 32, page, tsl],
                        in_=rote[32 * h:32 * h + 32, :])
                    nc.gpsimd.dma_start(
                        out=qh_sb[64 * i + 32:64 * i + 64, page, tsl],
                        in_=roto[32 * h:32 * h + 32, :])

            # token-major v, built after the per-block pipeline drains
            for kt in range(NKT):
                v_ps = ps.tile([128, 64], BF16, name="v_ps", tag="p_s3")
                nc.tensor.transpose(v_ps,
                                    vT_all[:, 128 * kt:128 * kt + 128],
                                    ident_b[0:64, 0:64])
                nc.vector.tensor_copy(vtok[:, kt, 0:64], v_ps)

        # prefetch the full output-projection weight during attention
        wo_ctx = tc.tile_pool(name="wo_pool", bufs=4)
        wo_pool = wo_ctx.__enter__()
        wo_tiles = []
        for ob in range(4):
            osl = slice(ob * 512, (ob + 1) * 512)
            wo_sb = wo_pool.tile([128, NIF, 512], BF16, name="wo_sb",
                                 tag="wo", bufs=4)
            nc.gpsimd.dma_start(
                out=wo_sb,
                in_=woT.rearrange("(n p) f -> p n f", p=128)[:, :, osl])
            wo_tiles.append(wo_sb)

        # ============ Phase C: attention + chunked A2A ============
        SC_TAGS = ["p_q0", "p_q1", "p_hA", "p_hB"]
        OUT_TAGS = ["p_kv", "p_s3", "p_rwA", "p_rwV"]
        with tc.tile_pool(name="pC", bufs=1) as pC:
            def emit_tail(qb, outps):
                """Normalize + ship chunk qb. Emitted after the next query
                block's first score/exp round so the reciprocal latency
                hides under attention compute."""
                for h in range(QH):
                    rec = pC.tile([1, 512], BF16, name="rec%d" % h,
                                  tag="rec%d" % h, bufs=2)
                    with nc.allow_low_precision(reason="attn denom"):
                        nc.vector.reciprocal(rec, outps[h][64:65, :])
                    rb = ps.tile([64, 512], F32, name="rb", tag=SC_TAGS[h])
                    nc.tensor.matmul(rb, ones64, rec, start=True, stop=True)
                    rb_s = pC.tile([64, 512], BF16, name="rb_s%d" % h,
                                   tag="rb_s%d" % h, bufs=2)
                    nc.vector.tensor_copy(rb_s, rb)
                    o65 = pC.tile([64, 512], BF16, name="o65%d" % h,
                                  tag="o65%d" % h, bufs=2)
                    nc.vector.tensor_tensor(o65, outps[h][0:64, :], rb_s,
                                            AluOpType.mult)
                    # [64, 512] -> cc_in[qb][dest, 64h:64h+64, 0:64]
                    nc.gpsimd.dma_start(
                        out=cc_in[qb][:, 64 * h:64 * h + 64, :]
                            .rearrange("d p t -> p d t"),
                        in_=o65)
                nc.gpsimd.collective_compute(
                    "AllToAll",
                    AluOpType.bypass,
                    ins=[cc_in[qb][:]],
                    outs=[cc_out[qb][:]],
                    replica_groups=[list(range(NCORES))],
                )
                # gather this chunk into g_sb[:, :, 64qb:64qb+64]
                nc.sync.dma_start(
                    out=g_sb[:, :, 64 * qb:64 * qb + 64],
                    in_=cc_out[qb].rearrange("s (k p) t -> p (s k) t",
                                             k=2, p=128))

            pending = None
            yp_tt0 = {}
            for qb in range(NQB):
                qsl = slice(qb * 512, (qb + 1) * 512)
                active = [kt for kt in range(NKT)
                          if mask_cls[kt, qb] != M_SKIP]
                assert active, f"fully masked query block qb={qb}"
                outps = [ps.tile([65, 512], F32, name="outp%d" % h,
                                 tag=OUT_TAGS[h]) for h in range(QH)]
                for idx, kt in enumerate(active):
                    c = mask_cls[kt, qb]
                    mt = None
                    if c == M_ADD:
                        mt = pC.tile([128, 512], BF16, name="mt",
                                     tag="mt", bufs=4)
                        nc.sync.dma_start(
                            out=mt,
                            in_=mask01[128 * kt:128 * kt + 128, qsl])
                    ksl = slice(128 * kt, 128 * kt + 128)
                    prs = []
                    for h in range(QH):
                        page, i = h // 2, h % 2
                        sc = ps.tile([128, 512], F32, name="sc%d" % h,
                                     tag=SC_TAGS[h])
                        nc.tensor.matmul(sc,
                                         kh_sb[64 * i:64 * i + 64, ksl],
                                         qh_sb[64 * i:64 * i + 64, page, qsl],
                                         start=True, stop=True,
                                         tile_position=(64 * i, 0))
                        pr = pC.tile([128, 512], BF16, name="pr%d" % h,
                                     tag="pr%d" % h, bufs=2)
                        nc.scalar.activation(pr, sc, AF.Exp)
                        if mt is not None:
                            nc.vector.tensor_tensor(pr, pr, mt,
                                                    AluOpType.mult)
                        prs.append(pr)
                    if idx == 0 and pending is not None:
                        # previous block's normalization rides behind this
                        # round's score matmuls
                        emit_tail(qb - 1, pending)
                        pending = None
                    for h in range(QH):
                        nc.tensor.matmul(outps[h], vtok[:, kt, :], prs[h],
                                         start=(kt == active[0]),
                                         stop=(kt == active[-1]))
                pending = outps
            emit_tail(NQB - 1, pending)
            # o-proj token-half 0 (chunks 0/1 landed long ago) rides the
            # tensor engine under the final AllToAll
            YP0_TAGS = ["p_q0", "p_q1", "p_hA"]
            for ob in range(3):
                osl = slice(ob * 512, (ob + 1) * 512)
                yp = ps.tile([128, 512], F32, name="yp0_%d" % ob,
                             tag=YP0_TAGS[ob])
                for k in range(NIF):
                    nc.tensor.matmul(yp, g_sb[:, k, 0:128],
                                     wo_tiles[ob][:, k, :],
                                     start=(k == 0), stop=False)
                yp_tt0[ob] = yp

        # ================= Phase D: o-proj =================
        with tc.tile_pool(name="pD", bufs=1) as pD:
            ho = ps.tile([72, TSH], F32, name="ho", tag="p_hB")
            for k in range(NIF):
                nc.tensor.matmul(ho, ao_sb[:, k, :], g_sb[:, k, :],
                                 start=(k == 0), stop=(k == NIF - 1))
            exo = pD.tile([8, TSH], BF16, name="exo")
            nc.scalar.activation(exo, ho[64:72, :], AF.Exp)
            so = ps.tile([8, TSH], F32, name="so", tag="p_s3")
            nc.tensor.matmul(so, ones8, exo, start=True, stop=True)
            reco = pD.tile([8, TSH], BF16, name="reco")
            with nc.allow_low_precision(reason="o-router softmax denom"):
                nc.vector.reciprocal(reco, so)
            rwo = pD.tile([8, TSH], BF16, name="rwo")
            nc.vector.tensor_tensor(rwo, exo, reco, AluOpType.mult)
            rwbo = ps.tile([64, TSH], F32, name="rwbo", tag="p_rwA")
            nc.tensor.matmul(rwbo, E8o, rwo, start=True, stop=True)
            rwbo_s = pD.tile([64, TSH], BF16, name="rwbo_s")
            nc.vector.tensor_copy(rwbo_s, rwbo)
            hpo = pD.tile([64, TSH], BF16, name="hpo")
            nc.vector.tensor_tensor(hpo, ho[0:64, :], rwbo_s, AluOpType.mult)

            # last token-half-0 chain (bank freed by ho's readers above)
            yp = ps.tile([128, 512], F32, name="yp0_3", tag="p_hB")
            for k in range(NIF):
                nc.tensor.matmul(yp, g_sb[:, k, 0:128],
                                 wo_tiles[3][:, k, :],
                                 start=(k == 0), stop=False)
            yp_tt0[3] = yp

            for ob in range(4):
                osl = slice(ob * 512, (ob + 1) * 512)
                yp = yp_tt0[ob]
                nc.tensor.matmul(yp, hpo[:, 0:128], bo_sb[:, osl],
                                 start=False, stop=True)
                yt = pD.tile([128, 512], F32, name="yt", tag="yt", bufs=3)
                if ob % 2 == 0:
                    nc.scalar.activation(yt, yp, AF.Copy)
                else:
                    nc.vector.tensor_copy(yt, yp)
                nc.sync.dma_start(out=y[0:128, osl], in_=yt)
            for ob in range(4):
                osl = slice(ob * 512, (ob + 1) * 512)
                yp = ps.tile([128, 512], F32, name="yp1_%d" % ob,
                             tag=OUT_TAGS[ob])
                for k in range(NIF):
                    nc.tensor.matmul(yp, g_sb[:, k, 128:256],
                                     wo_tiles[ob][:, k, :],
                                     start=(k == 0), stop=False)
                nc.tensor.matmul(yp, hpo[:, 128:256], bo_sb[:, osl],
                                 start=False, stop=True)
                yt = pD.tile([128, 512], F32, name="yt", tag="yt", bufs=3)
                if ob % 2 == 0:
                    nc.scalar.activation(yt, yp, AF.Copy)
                else:
                    nc.vector.tensor_copy(yt, yp)
                nc.sync.dma_start(out=y[128:256, osl], in_=yt)
        wo_ctx.__exit__(None, None, None)


# ======================= host side =======================

_CACHE = {}


def _prep_inputs(x, mask, freqs_cos, freqs_sin, wq, wk, wv, wo,
                 lq_router, lq_A, lq_B, lk_router, lk_A, lk_B,
                 lv_router, lv_A, lv_B, lo_router, lo_A, lo_B):
    scale = 1.0 / np.sqrt(HD)
    x = _f32(np.asarray(x)).reshape(S, D)
    maskf = _f32(np.asarray(mask)).reshape(S, S)
    maskT = np.maximum(maskf, MASK_NEG).T.copy()
    mask_cls = classify_mask(maskT)
    mask01 = _bf((maskT > MASK_NEG * 0.5).astype(np.float32))

    xT = _bf(x.T)
    cs4 = _bf(np.tile(_f32(freqs_cos).T, (4, 1)))      # [128, S]
    sn4 = _bf(np.tile(_f32(freqs_sin).T, (4, 1)))
    woT = _bf(_f32(wo).T)
    ao_p = _bf(np.concatenate([_a_pack(_f32(lo_A)), _f32(lo_router).T],
                              axis=1))                 # [D, 72]
    bo_f = _bf(_b_flat(_f32(lo_B), SCALING))

    # fused LoRA-A stationaries: [D, 128] = [aq|ak], [D, 88] = [av|rq|rk|rv]
    aA_p = _bf(np.concatenate(
        [_a_pack(_f32(lq_A)), _a_pack(_f32(lk_A))], axis=1))
    aB_p = _bf(np.concatenate(
        [_a_pack(_f32(lv_A)), _f32(lq_router).T, _f32(lk_router).T,
         _f32(lv_router).T], axis=1))

    shared = dict(xT=xT, cs4=cs4, sn4=sn4, woT=woT, mask01=mask01,
                  ao=ao_p, bo=bo_f, cst=_build_cst(), aA=aA_p, aB=aB_p)

    wqf, wkf, wvf = _f32(wq), _f32(wk), _f32(wv)
    lqB, lkB, lvB = _f32(lq_B), _f32(lk_B), _f32(lv_B)

    in_maps = []
    for c in range(NCORES):
        wq_c = wqf[c * QF:(c + 1) * QF][IDX_Q] * scale
        wk_c = wkf[c * KF:(c + 1) * KF][IDX_K]
        wv_c = wvf[c * KF:(c + 1) * KF]
        bq_c = _b_flat(lqB[:, c * QF:(c + 1) * QF, :][:, IDX_Q, :],
                       SCALING * scale)
        bk_c = _b_flat(lkB[:, c * KF:(c + 1) * KF, :][:, IDX_K, :], SCALING)
        bv_c = _b_flat(lvB[:, c * KF:(c + 1) * KF, :], SCALING)
        m = dict(shared)
        m.update(wqT=_bf(wq_c.T),
                 wkvT=_bf(np.concatenate([wk_c.T, wv_c.T], axis=1)),
                 bq=_bf(bq_c), bk=_bf(bk_c), bv=_bf(bv_c))
        in_maps.append(m)
    return in_maps, mask_cls


def get_graph(mask_cls):
    key = mask_cls.tobytes()
    if key not in _CACHE:
        _CACHE[key] = build(mask_cls)
    return _CACHE[key]


def kernel(x, start_pos, mask, freqs_cos, freqs_sin, wq, wk, wv, wo,
           lq_router, lq_A, lq_B, lk_router, lk_A, lk_B,
           lv_router, lv_A, lv_B, lo_router, lo_A, lo_B,
           _trace=False):
    from concourse.bass_utils import run_bass_kernel_spmd
    in_maps, mask_cls = _prep_inputs(
        x, mask, freqs_cos, freqs_sin, wq, wk, wv, wo,
        lq_router, lq_A, lq_B, lk_router, lk_A, lk_B,
        lv_router, lv_A, lv_B, lo_router, lo_A, lo_B)
    nc = get_graph(mask_cls)
    res = run_bass_kernel_spmd(nc, in_maps, list(range(NCORES)), trace=_trace)
    # core c's y rows: group g (0..3) covers tokens [512g + 64c, 512g + 64c + 64)
    ys = np.stack([res.results[c]["y"] for c in range(NCORES)], axis=0)
    ys = ys.reshape(NCORES, 4, 64, D).transpose(1, 0, 2, 3).reshape(S, D)
    out = ys.reshape(B, S, H * HD).astype(np.float32)
    if _trace:
        return out, res
    return out


# revision 16
# speedup vs baseline: 1.3993x; 1.0945x over previous
"""Trainium2 Bass kernel for MoE-LoRA GQA attention (nn_Attention_57389353009692).

V2 strategy (8 NeuronCores, one SPMD launch):
  - Tensor-parallel over heads: core c owns q-heads 4c..4c+3 and kv-head c.
  - Phase A (per 512-token block): QKV projections (+ MoE-LoRA) with packed
    matmul chains (wk|wv fused; LoRA-A for q/k/v + all three routers fused
    into two chains of 128/88 rows), router softmax done with
    exp -> ones-matmul row-sum -> reciprocal -> broadcast-matmul (no
    transposes, no DRAM bounce), RoPE in bf16 on 128 partitions.
  - Phase C: flash-style attention per 512-query block; causal mask applied
    as a 0/1 multiply after exp (bf16); output normalized PRE-collective via
    reciprocal-of-denominator broadcast matmuls fused into the PSUM->SBUF
    cast.
  - AllToAll is chunked per query block (4 collectives) and overlaps the
    remaining attention compute. Output tokens are interleaved at
    64-granularity: core c owns tokens {t : (t//64) % 8 == c} so every chunk
    is a uniform 8-way exchange.
  - Phase D: o-projection + o-LoRA for the core's 256 tokens with the full
    (prefetched) wo.

Numerics: bf16 operands, fp32 PSUM accumulation, fp32->exp softmax without
max subtraction (scores are O(1) here; masked entries are zeroed exactly by
the 0/1 multiply). Scale 1/sqrt(64) folded into wq and q-LoRA-B on host.
RoPE trick: interleaved even/odd pairs are made contiguous by permuting
wq/wk output features on host (per 2-head "page": [h0e|h1e|h0o|h1o]).
"""

import sys

for _p in ("/opt/trn_rl_repo", "/root/.axon_site/_ro/trn_rl_repo"):
    if _p not in sys.path:
        sys.path.insert(0, _p)

import numpy as np
import ml_dtypes

import concourse.bass as bass
import concourse.tile as tile
from concourse import bacc, mybir
from concourse.masks import make_identity
from concourse.alu_op_type import AluOpType

F32 = mybir.dt.float32
BF16 = mybir.dt.bfloat16
AF = mybir.ActivationFunctionType
AX = mybir.AxisListType
BF16NP = ml_dtypes.bfloat16

B, S, D = 1, 2048, 2048
H, KVH, HD = 32, 8, 64
NREP = H // KVH
R, E = 8, 8
SCALING = 32.0 / 8.0
NCORES = 8
QH = H // NCORES          # 4 q heads per core
QF = QH * HD              # 256 q feats per core
KF = HD                   # 64 kv feats per core
TSH = S // NCORES         # 256 tokens per core for o-proj
NKT = S // 128            # 16 key tiles
NQB = S // 512            # 4 query blocks
NIF = D // 128            # 16 contraction tiles

MASK_NEG = -1e30

# mask tile classes
M_SKIP, M_ZERO, M_ADD = 0, 1, 2


def _build_perm():
    """Per-core feature permutations for rope-friendly layout."""
    idx_q = np.zeros(QF, dtype=np.int64)
    for f in range(QF):
        page, w = divmod(f, 128)
        if w < 32:
            hl, j, odd = 2 * page, w, 0
        elif w < 64:
            hl, j, odd = 2 * page + 1, w - 32, 0
        elif w < 96:
            hl, j, odd = 2 * page, w - 64, 1
        else:
            hl, j, odd = 2 * page + 1, w - 96, 1
        idx_q[f] = 64 * hl + 2 * j + odd
    idx_k = np.zeros(KF, dtype=np.int64)
    for w in range(KF):
        if w < 32:
            idx_k[w] = 2 * w
        else:
            idx_k[w] = 2 * (w - 32) + 1
    return idx_q, idx_k


IDX_Q, IDX_K = _build_perm()


def _a_pack(A):
    """[E,R,D] -> [D, 64] with col r*8+e."""
    return np.transpose(A, (1, 0, 2)).reshape(E * R, -1).T


def _b_flat(Bw, scale):
    """[E, OF, R] -> [64, OF] with row r*8+e."""
    return (np.transpose(Bw, (2, 0, 1)).reshape(E * R, -1) * scale)


def _bf(x):
    return np.ascontiguousarray(x, dtype=np.float32).astype(BF16NP)


def _f32(x):
    return np.ascontiguousarray(x, dtype=np.float32)


def classify_mask(maskT):
    """maskT: [S(k), S(q)] clamped fp32. Returns ([NKT, NQB] class map,
    [NKT, NQB] live-start-column map for M_ADD tiles).

    For an M_ADD tile, lo is the first live column, rounded down to 128;
    columns >= lo+128 must be fully live (causal staircase) -- the kernel
    then computes only [lo, 512) and masks just [lo, lo+128)."""
    cls = np.zeros((NKT, NQB), dtype=np.int64)
    los = np.zeros((NKT, NQB), dtype=np.int64)
    for kt in range(NKT):
        blk_rows = maskT[kt * 128:(kt + 1) * 128]
        for qb in range(NQB):
            blk = blk_rows[:, qb * 512:(qb + 1) * 512]
            if np.all(blk <= MASK_NEG * 0.5):
                cls[kt, qb] = M_SKIP
            elif np.all(blk == 0.0):
                cls[kt, qb] = M_ZERO
            else:
                cls[kt, qb] = M_ADD
                live = np.where((blk == 0.0).any(axis=0))[0]
                lo = (int(live[0]) // 128) * 128 if len(live) else 0
                if lo + 128 <= 512 and not np.all(blk[:, lo + 128:] == 0.0):
                    lo = 0  # not a causal staircase; keep full width
                los[kt, qb] = lo
    return cls, los


# constants tensor layout (bf16, [24, 344]):
#  [:, 0:128]   E_A: row e, col j -> 1 if (j<64 and e==j%8) or (j>=64 and e-8==j%8)
#  [:, 128:192] E_v: row e, col j -> 1 if e-16 == j%8
#  [:, 192:216] ones24: block-diag 3x(8x8 ones)
#  [0:8, 216:280] E8o: row e, col j -> 1 if e == j%8
#  [0:1, 280:344] ones64 row
CST_W = 344


def _build_cst():
    cst = np.zeros((24, CST_W), dtype=np.float32)
    for j in range(64):
        cst[j % 8, j] = 1.0           # E_A q half
        cst[8 + j % 8, 64 + j] = 1.0  # E_A k half
        cst[16 + j % 8, 128 + j] = 1.0  # E_v
        cst[j % 8, 216 + j] = 1.0     # E8o
        cst[0, 280 + j] = 1.0         # ones64
    for b in range(3):
        cst[8 * b:8 * b + 8, 192 + 8 * b:200 + 8 * b] = 1.0  # ones24
    return _bf(cst)


def build(mask_cls, mask_lo):
    """Build the SPMD Bass graph. mask_cls: [NKT, NQB] int array."""
    nc = bacc.Bacc(None, target_bir_lowering=False)

    # ---- DRAM I/O (per-core shards prepared on host) ----
    xT = nc.declare_dram_parameter("xT", [D, S], BF16, isOutput=False)
    wqT = nc.declare_dram_parameter("wqT", [D, QF], BF16, isOutput=False)
    wkvT = nc.declare_dram_parameter("wkvT", [D, 2 * KF], BF16, isOutput=False)
    aA = nc.declare_dram_parameter("aA", [D, 128], BF16, isOutput=False)
    aB = nc.declare_dram_parameter("aB", [D, 88], BF16, isOutput=False)
    ao = nc.declare_dram_parameter("ao", [D, 72], BF16, isOutput=False)
    bq = nc.declare_dram_parameter("bq", [E * R, QF], BF16, isOutput=False)
    bk = nc.declare_dram_parameter("bk", [E * R, KF], BF16, isOutput=False)
    bv = nc.declare_dram_parameter("bv", [E * R, KF], BF16, isOutput=False)
    bo = nc.declare_dram_parameter("bo", [E * R, D], BF16, isOutput=False)
    woT = nc.declare_dram_parameter("woT", [D, D], BF16, isOutput=False)
    cs4 = nc.declare_dram_parameter("cs4", [128, S], BF16, isOutput=False)
    sn4 = nc.declare_dram_parameter("sn4", [128, S], BF16, isOutput=False)
    mask01 = nc.declare_dram_parameter("mask01", [S, S], BF16, isOutput=False)
    cst = nc.declare_dram_parameter("cst", [24, CST_W], BF16, isOutput=False)
    y = nc.declare_dram_parameter("y", [TSH, D], F32, isOutput=True)

    # internal DRAM for the chunked collectives: [dest/src, 4h*64 feat, 64 tok]
    cc_in = [nc.dram_tensor("cc_in%d" % q, [NCORES, QF, 64], BF16)
             for q in range(NQB)]
    cc_out = [nc.dram_tensor("cc_out%d" % q, [NCORES, QF, 64], BF16)
              for q in range(NQB)]

    with tile.TileContext(nc) as tc:
        _emit(nc, tc, locals(), mask_cls, mask_lo)
    nc.finalize()
    return nc


def _emit(nc, tc, t, mask_cls, mask_lo):
    xT, wqT, wkvT = t["xT"], t["wqT"], t["wkvT"]
    aA, aB, ao = t["aA"], t["aB"], t["ao"]
    bq, bk, bv, bo = t["bq"], t["bk"], t["bv"], t["bo"]
    woT, cs4, sn4, mask01, y = t["woT"], t["cs4"], t["sn4"], t["mask01"], t["y"]
    cst = t["cst"]
    cc_in, cc_out = t["cc_in"], t["cc_out"]

    import contextlib
    ctx = contextlib.ExitStack()
    with ctx:
        persist = ctx.enter_context(tc.tile_pool(name="persist", bufs=1))
        ps = ctx.enter_context(tc.tile_pool(name="ps", bufs=1, space="PSUM"))

        # ---- persistent weights, split in k-groups of 4 for early start ----
        NSP = 4
        KG = NIF // NSP
        aA_sb, aB_sb, wq_sb, wkv_sb = [], [], [], []
        xq0 = persist.tile([128, NIF, 512], BF16, name="xq0")
        for g in range(NSP):
            ksl = slice(g * KG * 128, (g + 1) * KG * 128)
            tl = persist.tile([128, KG, 128], BF16, name="aA%d" % g)
            nc.scalar.dma_start(
                out=tl, in_=aA[ksl].rearrange("(n p) f -> p n f", p=128))
            aA_sb.append(tl)
            tl = persist.tile([128, KG, 88], BF16, name="aB%d" % g)
            nc.scalar.dma_start(
                out=tl, in_=aB[ksl].rearrange("(n p) f -> p n f", p=128))
            aB_sb.append(tl)
            # first token block's x rides ahead of the q weights
            nc.sync.dma_start(
                out=xq0[:, g * KG:(g + 1) * KG, :],
                in_=xT[ksl].rearrange("(n p) t -> p n t", p=128)[:, :, 0:512])
            tl = persist.tile([128, KG, QF], BF16, name="wq%d" % g)
            nc.sync.dma_start(
                out=tl, in_=wqT[ksl].rearrange("(n p) f -> p n f", p=128))
            wq_sb.append(tl)
            tl = persist.tile([128, KG, 2 * KF], BF16, name="wkv%d" % g)
            nc.gpsimd.dma_start(
                out=tl, in_=wkvT[ksl].rearrange("(n p) f -> p n f", p=128))
            wkv_sb.append(tl)

        def A_AT(k):  # aA chain lhsT for contraction tile k
            return aA_sb[k // KG][:, k % KG, :]

        def A_BT(k):
            return aB_sb[k // KG][:, k % KG, :]

        def W_Q(k):
            return wq_sb[k // KG][:, k % KG, :]

        def W_KV(k):
            return wkv_sb[k // KG][:, k % KG, :]

        cst_sb = persist.tile([24, CST_W], BF16)
        nc.gpsimd.dma_start(out=cst_sb, in_=cst[:])
        E_A = cst_sb[:, 0:128]
        E_v = cst_sb[0:24, 128:192]
        ones24 = cst_sb[:, 192:216]
        ones8 = cst_sb[0:8, 192:200]
        E8o = cst_sb[0:8, 216:280]
        ones64 = cst_sb[0:1, 280:344]

        bq_sb = persist.tile([64, QF], BF16)
        nc.gpsimd.dma_start(out=bq_sb, in_=bq[:])
        bk_sb = persist.tile([128, KF], BF16)   # bk lives at partitions 64:128
        nc.gpsimd.dma_start(out=bk_sb[64:128, :], in_=bk[:])
        bv_sb = persist.tile([64, KF], BF16)
        nc.gpsimd.dma_start(out=bv_sb, in_=bv[:])
        bo_sb = persist.tile([64, D], BF16)
        nc.gpsimd.dma_start(out=bo_sb, in_=bo[:])
        ao_sb = persist.tile([128, NIF, 72], BF16)
        nc.scalar.dma_start(out=ao_sb,
                            in_=ao.rearrange("(n p) f -> p n f", p=128))
        cs_sb = persist.tile([128, S], BF16)
        nc.scalar.dma_start(out=cs_sb, in_=cs4[:])
        sn_sb = persist.tile([128, S], BF16)
        nc.scalar.dma_start(out=sn_sb, in_=sn4[:])

        ident_b = persist.tile([128, 128], BF16)
        make_identity(nc, ident_b)

        # attention operands (persist across phases)
        qh_sb = persist.tile([128, 2, S], BF16)   # [2 heads x 64, page, S]
        kh_sb = persist.tile([128, S], BF16)      # kv head duplicated 2x
        vtok = persist.tile([128, NKT, 65], BF16)  # token-major v + ones col
        nc.vector.memset(vtok[:, :, 64:65], 1.0)
        g_sb = persist.tile([128, NIF, TSH], BF16)  # gathered out (post-A2A)

        # ================= Phase A: QKV + LoRA + RoPE =================
        vT_all = persist.tile([64, S], BF16)      # v (feat-major) staging
        with tc.tile_pool(name="pA", bufs=1) as pA:
            for tb in range(4):
                tsl = slice(tb * 512, (tb + 1) * 512)
                if tb == 0:
                    xq = xq0
                else:
                    xq = pA.tile([128, NIF, 512], BF16, name="xq", tag="xq",
                                 bufs=3)
                    nc.sync.dma_start(
                        out=xq,
                        in_=xT.rearrange("(n p) t -> p n t", p=128)[:, :, tsl])

                # ---- main projection chains ----
                hA = ps.tile([128, 512], F32, name="hA", tag="p_hA")
                hB = ps.tile([88, 512], F32, name="hB", tag="p_hB")
                q0 = ps.tile([128, 512], F32, name="q0", tag="p_q0")
                q1 = ps.tile([128, 512], F32, name="q1", tag="p_q1")
                kv = ps.tile([128, 512], F32, name="kv", tag="p_kv")
                for k in range(NIF):
                    st = k == 0
                    sp = k == NIF - 1
                    rhs = xq[:, k, :]
                    nc.tensor.matmul(hA, A_AT(k), rhs, start=st, stop=sp)
                    nc.tensor.matmul(hB, A_BT(k), rhs, start=st, stop=sp)
                for k in range(NIF):
                    rhs = xq[:, k, :]
                    st = k == 0
                    nc.tensor.matmul(q0, W_Q(k)[:, 0:128], rhs,
                                     start=st, stop=False)
                    nc.tensor.matmul(q1, W_Q(k)[:, 128:256], rhs,
                                     start=st, stop=False)
                    nc.tensor.matmul(kv, W_KV(k), rhs, start=st, stop=False)

                # ---- router softmax (q,k,v fused; no transposes) ----
                ex3 = pA.tile([24, 512], BF16, name="ex3", tag="ex3", bufs=2)
                nc.scalar.activation(ex3, hB[64:88, :], AF.Exp)
                s3 = ps.tile([24, 512], F32, name="s3", tag="p_s3")
                nc.tensor.matmul(s3, ones24, ex3, start=True, stop=True)
                rec3 = pA.tile([24, 512], BF16, name="rec3", tag="rec3",
                               bufs=2)
                with nc.allow_low_precision(reason="router softmax denom"):
                    nc.vector.reciprocal(rec3, s3)
                rw3 = pA.tile([24, 512], BF16, name="rw3", tag="rw3", bufs=2)
                nc.vector.tensor_tensor(rw3, ex3, rec3, AluOpType.mult)
                rwbA = ps.tile([128, 512], F32, name="rwbA", tag="p_rwA")
                nc.tensor.matmul(rwbA, E_A, rw3, start=True, stop=True)
                rwbV = ps.tile([64, 512], F32, name="rwbV", tag="p_rwV")
                nc.tensor.matmul(rwbV, E_v, rw3, start=True, stop=True)
                rwbA_s = pA.tile([128, 512], BF16, name="rwbA_s",
                                 tag="rwbA_s", bufs=2)
                nc.scalar.activation(rwbA_s, rwbA, AF.Copy)
                rwbV_s = pA.tile([64, 512], BF16, name="rwbV_s",
                                 tag="rwbV_s", bufs=2)
                nc.scalar.activation(rwbV_s, rwbV, AF.Copy)
                hpA = pA.tile([128, 512], BF16, name="hpA", tag="hpA", bufs=2)
                nc.vector.tensor_tensor(hpA, hA, rwbA_s, AluOpType.mult)
                hpV = pA.tile([64, 512], BF16, name="hpV", tag="hpV", bufs=2)
                nc.vector.tensor_tensor(hpV, hB[0:64, :], rwbV_s,
                                        AluOpType.mult)

                # ---- LoRA-B closes the accumulations ----
                nc.tensor.matmul(q0, bq_sb[:, 0:128], hpA[0:64, :],
                                 start=False, stop=True)
                nc.tensor.matmul(q1, bq_sb[:, 128:256], hpA[0:64, :],
                                 start=False, stop=True)
                nc.tensor.matmul(kv[0:64, :], bk_sb[64:128, :],
                                 hpA[64:128, :], start=False, stop=True,
                                 tile_position=(64, 0))
                nc.tensor.matmul(kv[64:128, :], bv_sb, hpV,
                                 start=False, stop=True,
                                 tile_position=(0, 64))

                # ---- PSUM extraction (scalar engine) ----
                qe = pA.tile([128, 512], BF16, name="qe", tag="qe", bufs=2)
                qo = pA.tile([128, 512], BF16, name="qo", tag="qo", bufs=2)
                nc.scalar.activation(qe[0:64, :], q0[0:64, :], AF.Copy)
                nc.scalar.activation(qe[64:128, :], q1[0:64, :], AF.Copy)
                nc.scalar.activation(qo[0:64, :], q0[64:128, :], AF.Copy)
                nc.scalar.activation(qo[64:128, :], q1[64:128, :], AF.Copy)
                ke = pA.tile([32, 512], BF16, name="ke", tag="ke", bufs=2)
                ko = pA.tile([32, 512], BF16, name="ko", tag="ko", bufs=2)
                nc.scalar.activation(ke, kv[0:32, :], AF.Copy)
                nc.scalar.activation(ko, kv[32:64, :], AF.Copy)
                nc.scalar.activation(vT_all[:, tsl], kv[64:128, :], AF.Copy)

                # ---- RoPE (bf16, vector engine) ----
                cs_t = cs_sb[:, tsl]
                sn_t = sn_sb[:, tsl]
                t1 = pA.tile([128, 512], BF16, name="t1", tag="t1", bufs=2)
                t2 = pA.tile([128, 512], BF16, name="t2", tag="t2", bufs=2)
                rote = pA.tile([128, 512], BF16, name="rote", tag="rote",
                               bufs=2)
                roto = pA.tile([128, 512], BF16, name="roto", tag="roto",
                               bufs=2)
                nc.vector.tensor_tensor(t1, qe, cs_t, AluOpType.mult)
                nc.vector.tensor_tensor(t2, qo, sn_t, AluOpType.mult)
                nc.vector.tensor_tensor(rote, t1, t2, AluOpType.subtract)
                nc.vector.tensor_tensor(t1, qe, sn_t, AluOpType.mult)
                nc.vector.tensor_tensor(t2, qo, cs_t, AluOpType.mult)
                nc.vector.tensor_tensor(roto, t1, t2, AluOpType.add)
                k1 = pA.tile([32, 512], BF16, name="k1", tag="k1", bufs=2)
                k2 = pA.tile([32, 512], BF16, name="k2", tag="k2", bufs=2)
                csk = cs_sb[0:32, tsl]
                snk = sn_sb[0:32, tsl]
                nc.vector.tensor_tensor(k1, ke, csk, AluOpType.mult)
                nc.vector.tensor_tensor(k2, ko, snk, AluOpType.mult)
                nc.vector.tensor_tensor(kh_sb[0:32, tsl], k1, k2,
                                        AluOpType.subtract)
                nc.vector.tensor_tensor(k1, ke, snk, AluOpType.mult)
                nc.vector.tensor_tensor(k2, ko, csk, AluOpType.mult)
                nc.vector.tensor_tensor(kh_sb[32:64, tsl], k1, k2,
                                        AluOpType.add)

                # head rearrange via SBUF->SBUF DMA (off the engines)
                nc.gpsimd.dma_start(out=kh_sb[64:128, tsl],
                                    in_=kh_sb[0:64, tsl])
                for h in range(QH):
                    page, i = h // 2, h % 2
                    nc.gpsimd.dma_start(
                        out=qh_sb[64 * i:64 * i + 32, page, tsl],
                        in_=rote[32 * h:32 * h + 32, :])
                    nc.gpsimd.dma_start(
                        out=qh_sb[64 * i + 32:64 * i + 64, page, tsl],
                        in_=roto[32 * h:32 * h + 32, :])

            # token-major v, built after the per-block pipeline drains
            for kt in range(NKT):
                v_ps = ps.tile([128, 64], BF16, name="v_ps", tag="p_s3")
                nc.tensor.transpose(v_ps,
                                    vT_all[:, 128 * kt:128 * kt + 128],
                                    ident_b[0:64, 0:64])
                nc.vector.tensor_copy(vtok[:, kt, 0:64], v_ps)

        # prefetch the full output-projection weight during attention
        wo_ctx = tc.tile_pool(name="wo_pool", bufs=4)
        wo_pool = wo_ctx.__enter__()
        wo_tiles = []
        for ob in range(4):
            osl = slice(ob * 512, (ob + 1) * 512)
            wo_sb = wo_pool.tile([128, NIF, 512], BF16, name="wo_sb",
                                 tag="wo", bufs=4)
            nc.gpsimd.dma_start(
                out=wo_sb,
                in_=woT.rearrange("(n p) f -> p n f", p=128)[:, :, osl])
            wo_tiles.append(wo_sb)

        # ============ Phase C: attention + chunked A2A ============
        SC_TAGS = ["p_q0", "p_q1", "p_hA", "p_hB"]
        OUT_TAGS = ["p_kv", "p_s3", "p_rwA", "p_rwV"]
        with tc.tile_pool(name="pC", bufs=1) as pC:
            def emit_tail(qb, outps):
                """Normalize + ship chunk qb. Emitted after the next query
                block's first score/exp round so the reciprocal latency
                hides under attention compute."""
                for h in range(QH):
                    rec = pC.tile([1, 512], BF16, name="rec%d" % h,
                                  tag="rec%d" % h, bufs=2)
                    with nc.allow_low_precision(reason="attn denom"):
                        nc.vector.reciprocal(rec, outps[h][64:65, :])
                    rb = ps.tile([64, 512], F32, name="rb", tag=SC_TAGS[h])
                    nc.tensor.matmul(rb, ones64, rec, start=True, stop=True)
                    rb_s = pC.tile([64, 512], BF16, name="rb_s%d" % h,
                                   tag="rb_s%d" % h, bufs=2)
                    nc.vector.tensor_copy(rb_s, rb)
                    o65 = pC.tile([64, 512], BF16, name="o65%d" % h,
                                  tag="o65%d" % h, bufs=2)
                    nc.vector.tensor_tensor(o65, outps[h][0:64, :], rb_s,
                                            AluOpType.mult)
                    # [64, 512] -> cc_in[qb][dest, 64h:64h+64, 0:64]
                    nc.gpsimd.dma_start(
                        out=cc_in[qb][:, 64 * h:64 * h + 64, :]
                            .rearrange("d p t -> p d t"),
                        in_=o65)
                nc.gpsimd.collective_compute(
                    "AllToAll",
                    AluOpType.bypass,
                    ins=[cc_in[qb][:]],
                    outs=[cc_out[qb][:]],
                    replica_groups=[list(range(NCORES))],
                )
                # gather this chunk into g_sb[:, :, 64qb:64qb+64]
                nc.sync.dma_start(
                    out=g_sb[:, :, 64 * qb:64 * qb + 64],
                    in_=cc_out[qb].rearrange("s (k p) t -> p (s k) t",
                                             k=2, p=128))

            pending = None
            yp_tt0 = {}
            for qb in range(NQB):
                qsl = slice(qb * 512, (qb + 1) * 512)
                active = [kt for kt in range(NKT)
                          if mask_cls[kt, qb] != M_SKIP]
                assert active, f"fully masked query block qb={qb}"
                outps = [ps.tile([65, 512], F32, name="outp%d" % h,
                                 tag=OUT_TAGS[h]) for h in range(QH)]
                for idx, kt in enumerate(active):
                    c = mask_cls[kt, qb]
                    lo = int(mask_lo[kt, qb]) if c == M_ADD else 0
                    mt = None
                    if c == M_ADD:
                        mt = pC.tile([128, 128], BF16, name="mt",
                                     tag="mt", bufs=4)
                        nc.sync.dma_start(
                            out=mt,
                            in_=mask01[128 * kt:128 * kt + 128,
                                       512 * qb + lo:512 * qb + lo + 128])
                    ksl = slice(128 * kt, 128 * kt + 128)
                    qslc = slice(512 * qb + lo, 512 * (qb + 1))
                    prs = []
                    for h in range(QH):
                        page, i = h // 2, h % 2
                        sc = ps.tile([128, 512], F32, name="sc%d" % h,
                                     tag=SC_TAGS[h])
                        nc.tensor.matmul(sc[:, lo:512],
                                         kh_sb[64 * i:64 * i + 64, ksl],
                                         qh_sb[64 * i:64 * i + 64, page,
                                               qslc],
                                         start=True, stop=True,
                                         tile_position=(64 * i, 0))
                        pr = pC.tile([128, 512], BF16, name="pr%d" % h,
                                     tag="pr%d" % h, bufs=2)
                        nc.scalar.activation(pr[:, lo:512], sc[:, lo:512],
                                             AF.Exp)
                        if mt is not None:
                            nc.vector.tensor_tensor(pr[:, lo:lo + 128],
                                                    pr[:, lo:lo + 128], mt,
                                                    AluOpType.mult)
                        prs.append(pr)
                    if idx == 0 and pending is not None:
                        # previous block's normalization rides behind this
                        # round's score matmuls
                        emit_tail(qb - 1, pending)
                        pending = None
                    for h in range(QH):
                        nc.tensor.matmul(outps[h][:, lo:512],
                                         vtok[:, kt, :],
                                         prs[h][:, lo:512],
                                         start=(kt == active[0]),
                                         stop=(kt == active[-1]))
                pending = outps
            emit_tail(NQB - 1, pending)
            # o-proj token-half 0 (chunks 0/1 landed long ago) rides the
            # tensor engine under the final AllToAll
            YP0_TAGS = ["p_q0", "p_q1", "p_hA"]
            for ob in range(3):
                osl = slice(ob * 512, (ob + 1) * 512)
                yp = ps.tile([128, 512], F32, name="yp0_%d" % ob,
                             tag=YP0_TAGS[ob])
                for k in range(NIF):
                    nc.tensor.matmul(yp, g_sb[:, k, 0:128],
                                     wo_tiles[ob][:, k, :],
                                     start=(k == 0), stop=False)
                yp_tt0[ob] = yp

        # ================= Phase D: o-proj =================
        with tc.tile_pool(name="pD", bufs=1) as pD:
            ho = ps.tile([72, TSH], F32, name="ho", tag="p_hB")
            for k in range(NIF):
                nc.tensor.matmul(ho, ao_sb[:, k, :], g_sb[:, k, :],
                                 start=(k == 0), stop=(k == NIF - 1))
            exo = pD.tile([8, TSH], BF16, name="exo")
            nc.scalar.activation(exo, ho[64:72, :], AF.Exp)
            so = ps.tile([8, TSH], F32, name="so", tag="p_s3")
            nc.tensor.matmul(so, ones8, exo, start=True, stop=True)
            reco = pD.tile([8, TSH], BF16, name="reco")
            with nc.allow_low_precision(reason="o-router softmax denom"):
                nc.vector.reciprocal(reco, so)
            rwo = pD.tile([8, TSH], BF16, name="rwo")
            nc.vector.tensor_tensor(rwo, exo, reco, AluOpType.mult)
            rwbo = ps.tile([64, TSH], F32, name="rwbo", tag="p_rwA")
            nc.tensor.matmul(rwbo, E8o, rwo, start=True, stop=True)
            rwbo_s = pD.tile([64, TSH], BF16, name="rwbo_s")
            nc.vector.tensor_copy(rwbo_s, rwbo)
            hpo = pD.tile([64, TSH], BF16, name="hpo")
            nc.vector.tensor_tensor(hpo, ho[0:64, :], rwbo_s, AluOpType.mult)

            # last token-half-0 chain (bank freed by ho's readers above)
            yp = ps.tile([128, 512], F32, name="yp0_3", tag="p_hB")
            for k in range(NIF):
                nc.tensor.matmul(yp, g_sb[:, k, 0:128],
                                 wo_tiles[3][:, k, :],
                                 start=(k == 0), stop=False)
            yp_tt0[3] = yp

            for ob in range(4):
                osl = slice(ob * 512, (ob + 1) * 512)
                yp = yp_tt0[ob]
                nc.tensor.matmul(yp, hpo[:, 0:128], bo_sb[:, osl],
                                 start=False, stop=True)
                yt = pD.tile([128, 512], F32, name="yt", tag="yt", bufs=3)
                if ob % 2 == 0:
                    nc.scalar.activation(yt, yp, AF.Copy)
                else:
                    nc.vector.tensor_copy(yt, yp)
                nc.sync.dma_start(out=y[0:128, osl], in_=yt)
            for ob in range(4):
                osl = slice(ob * 512, (ob + 1) * 512)
                yp = ps.tile([128, 512], F32, name="yp1_%d" % ob,
                             tag=OUT_TAGS[ob])
                for k in range(NIF):
                    nc.tensor.matmul(yp, g_sb[:, k, 128:256],
                                     wo_tiles[ob][:, k, :],
                                     start=(k == 0), stop=False)
                nc.tensor.matmul(yp, hpo[:, 128:256], bo_sb[:, osl],
                                 start=False, stop=True)
                yt = pD.tile([128, 512], F32, name="yt", tag="yt", bufs=3)
                if ob % 2 == 0:
                    nc.scalar.activation(yt, yp, AF.Copy)
                else:
                    nc.vector.tensor_copy(yt, yp)
                nc.sync.dma_start(out=y[128:256, osl], in_=yt)
        wo_ctx.__exit__(None, None, None)


# ======================= host side =======================

_CACHE = {}


def _prep_inputs(x, mask, freqs_cos, freqs_sin, wq, wk, wv, wo,
                 lq_router, lq_A, lq_B, lk_router, lk_A, lk_B,
                 lv_router, lv_A, lv_B, lo_router, lo_A, lo_B):
    scale = 1.0 / np.sqrt(HD)
    x = _f32(np.asarray(x)).reshape(S, D)
    maskf = _f32(np.asarray(mask)).reshape(S, S)
    maskT = np.maximum(maskf, MASK_NEG).T.copy()
    mask_cls, mask_lo = classify_mask(maskT)
    mask01 = _bf((maskT > MASK_NEG * 0.5).astype(np.float32))

    xT = _bf(x.T)
    cs4 = _bf(np.tile(_f32(freqs_cos).T, (4, 1)))      # [128, S]
    sn4 = _bf(np.tile(_f32(freqs_sin).T, (4, 1)))
    woT = _bf(_f32(wo).T)
    ao_p = _bf(np.concatenate([_a_pack(_f32(lo_A)), _f32(lo_router).T],
                              axis=1))                 # [D, 72]
    bo_f = _bf(_b_flat(_f32(lo_B), SCALING))

    # fused LoRA-A stationaries: [D, 128] = [aq|ak], [D, 88] = [av|rq|rk|rv]
    aA_p = _bf(np.concatenate(
        [_a_pack(_f32(lq_A)), _a_pack(_f32(lk_A))], axis=1))
    aB_p = _bf(np.concatenate(
        [_a_pack(_f32(lv_A)), _f32(lq_router).T, _f32(lk_router).T,
         _f32(lv_router).T], axis=1))

    shared = dict(xT=xT, cs4=cs4, sn4=sn4, woT=woT, mask01=mask01,
                  ao=ao_p, bo=bo_f, cst=_build_cst(), aA=aA_p, aB=aB_p)

    wqf, wkf, wvf = _f32(wq), _f32(wk), _f32(wv)
    lqB, lkB, lvB = _f32(lq_B), _f32(lk_B), _f32(lv_B)

    in_maps = []
    for c in range(NCORES):
        wq_c = wqf[c * QF:(c + 1) * QF][IDX_Q] * scale
        wk_c = wkf[c * KF:(c + 1) * KF][IDX_K]
        wv_c = wvf[c * KF:(c + 1) * KF]
        bq_c = _b_flat(lqB[:, c * QF:(c + 1) * QF, :][:, IDX_Q, :],
                       SCALING * scale)
        bk_c = _b_flat(lkB[:, c * KF:(c + 1) * KF, :][:, IDX_K, :], SCALING)
        bv_c = _b_flat(lvB[:, c * KF:(c + 1) * KF, :], SCALING)
        m = dict(shared)
        m.update(wqT=_bf(wq_c.T),
                 wkvT=_bf(np.concatenate([wk_c.T, wv_c.T], axis=1)),
                 bq=_bf(bq_c), bk=_bf(bk_c), bv=_bf(bv_c))
        in_maps.append(m)
    return in_maps, mask_cls, mask_lo


def get_graph(mask_cls, mask_lo):
    key = mask_cls.tobytes() + mask_lo.tobytes()
    if key not in _CACHE:
        _CACHE[key] = build(mask_cls, mask_lo)
    return _CACHE[key]


def kernel(x, start_pos, mask, freqs_cos, freqs_sin, wq, wk, wv, wo,
           lq_router, lq_A, lq_B, lk_router, lk_A, lk_B,
           lv_router, lv_A, lv_B, lo_router, lo_A, lo_B,
           _trace=False):
    from concourse.bass_utils import run_bass_kernel_spmd
    in_maps, mask_cls, mask_lo = _prep_inputs(
        x, mask, freqs_cos, freqs_sin, wq, wk, wv, wo,
        lq_router, lq_A, lq_B, lk_router, lk_A, lk_B,
        lv_router, lv_A, lv_B, lo_router, lo_A, lo_B)
    nc = get_graph(mask_cls, mask_lo)
    res = run_bass_kernel_spmd(nc, in_maps, list(range(NCORES)), trace=_trace)
    # core c's y rows: group g (0..3) covers tokens [512g + 64c, 512g + 64c + 64)
    ys = np.stack([res.results[c]["y"] for c in range(NCORES)], axis=0)
    ys = ys.reshape(NCORES, 4, 64, D).transpose(1, 0, 2, 3).reshape(S, D)
    out = ys.reshape(B, S, H * HD).astype(np.float32)
    if _trace:
        return out, res
    return out


# revision 18
# speedup vs baseline: 1.4106x; 1.0081x over previous
"""Trainium2 Bass kernel for MoE-LoRA GQA attention (nn_Attention_57389353009692).

V2 strategy (8 NeuronCores, one SPMD launch):
  - Tensor-parallel over heads: core c owns q-heads 4c..4c+3 and kv-head c.
  - Phase A (per 512-token block): QKV projections (+ MoE-LoRA) with packed
    matmul chains (wk|wv fused; LoRA-A for q/k/v + all three routers fused
    into two chains of 128/88 rows), router softmax done with
    exp -> ones-matmul row-sum -> reciprocal -> broadcast-matmul (no
    transposes, no DRAM bounce), RoPE in bf16 on 128 partitions.
  - Phase C: flash-style attention per 512-query block; causal mask applied
    as a 0/1 multiply after exp (bf16); output normalized PRE-collective via
    reciprocal-of-denominator broadcast matmuls fused into the PSUM->SBUF
    cast.
  - AllToAll is chunked per query block (4 collectives) and overlaps the
    remaining attention compute. Output tokens are interleaved at
    64-granularity: core c owns tokens {t : (t//64) % 8 == c} so every chunk
    is a uniform 8-way exchange.
  - Phase D: o-projection + o-LoRA for the core's 256 tokens with the full
    (prefetched) wo.

Numerics: bf16 operands, fp32 PSUM accumulation, fp32->exp softmax without
max subtraction (scores are O(1) here; masked entries are zeroed exactly by
the 0/1 multiply). Scale 1/sqrt(64) folded into wq and q-LoRA-B on host.
RoPE trick: interleaved even/odd pairs are made contiguous by permuting
wq/wk output features on host (per 2-head "page": [h0e|h1e|h0o|h1o]).
"""

import sys

for _p in ("/opt/trn_rl_repo", "/root/.axon_site/_ro/trn_rl_repo"):
    if _p not in sys.path:
        sys.path.insert(0, _p)

import numpy as np
import ml_dtypes

import concourse.bass as bass
import concourse.tile as tile
from concourse import bacc, mybir
from concourse.masks import make_identity
from concourse.alu_op_type import AluOpType

F32 = mybir.dt.float32
BF16 = mybir.dt.bfloat16
AF = mybir.ActivationFunctionType
AX = mybir.AxisListType
BF16NP = ml_dtypes.bfloat16

B, S, D = 1, 2048, 2048
H, KVH, HD = 32, 8, 64
NREP = H // KVH
R, E = 8, 8
SCALING = 32.0 / 8.0
NCORES = 8
QH = H // NCORES          # 4 q heads per core
QF = QH * HD              # 256 q feats per core
KF = HD                   # 64 kv feats per core
TSH = S // NCORES         # 256 tokens per core for o-proj
NKT = S // 128            # 16 key tiles
NQB = S // 512            # 4 query blocks
NIF = D // 128            # 16 contraction tiles

MASK_NEG = -1e30

# mask tile classes
M_SKIP, M_ZERO, M_ADD = 0, 1, 2


def _build_perm():
    """Per-core feature permutations for rope-friendly layout."""
    idx_q = np.zeros(QF, dtype=np.int64)
    for f in range(QF):
        page, w = divmod(f, 128)
        if w < 32:
            hl, j, odd = 2 * page, w, 0
        elif w < 64:
            hl, j, odd = 2 * page + 1, w - 32, 0
        elif w < 96:
            hl, j, odd = 2 * page, w - 64, 1
        else:
            hl, j, odd = 2 * page + 1, w - 96, 1
        idx_q[f] = 64 * hl + 2 * j + odd
    idx_k = np.zeros(KF, dtype=np.int64)
    for w in range(KF):
        if w < 32:
            idx_k[w] = 2 * w
        else:
            idx_k[w] = 2 * (w - 32) + 1
    return idx_q, idx_k


IDX_Q, IDX_K = _build_perm()


def _a_pack(A):
    """[E,R,D] -> [D, 64] with col r*8+e."""
    return np.transpose(A, (1, 0, 2)).reshape(E * R, -1).T


def _b_flat(Bw, scale):
    """[E, OF, R] -> [64, OF] with row r*8+e."""
    return (np.transpose(Bw, (2, 0, 1)).reshape(E * R, -1) * scale)


def _bf(x):
    return np.ascontiguousarray(x, dtype=np.float32).astype(BF16NP)


def _f32(x):
    return np.ascontiguousarray(x, dtype=np.float32)


def classify_mask(maskT):
    """maskT: [S(k), S(q)] clamped fp32. Returns ([NKT, NQB] class map,
    [NKT, NQB] live-start-column map for M_ADD tiles).

    For an M_ADD tile, lo is the first live column, rounded down to 128;
    columns >= lo+128 must be fully live (causal staircase) -- the kernel
    then computes only [lo, 512) and masks just [lo, lo+128)."""
    cls = np.zeros((NKT, NQB), dtype=np.int64)
    los = np.zeros((NKT, NQB), dtype=np.int64)
    for kt in range(NKT):
        blk_rows = maskT[kt * 128:(kt + 1) * 128]
        for qb in range(NQB):
            blk = blk_rows[:, qb * 512:(qb + 1) * 512]
            if np.all(blk <= MASK_NEG * 0.5):
                cls[kt, qb] = M_SKIP
            elif np.all(blk == 0.0):
                cls[kt, qb] = M_ZERO
            else:
                cls[kt, qb] = M_ADD
                live = np.where((blk == 0.0).any(axis=0))[0]
                lo = (int(live[0]) // 128) * 128 if len(live) else 0
                if lo + 128 <= 512 and not np.all(blk[:, lo + 128:] == 0.0):
                    lo = 0  # not a causal staircase; keep full width
                los[kt, qb] = lo
    return cls, los


# constants tensor layout (bf16, [24, 344]):
#  [:, 0:128]   E_A: row e, col j -> 1 if (j<64 and e==j%8) or (j>=64 and e-8==j%8)
#  [:, 128:192] E_v: row e, col j -> 1 if e-16 == j%8
#  [:, 192:216] ones24: block-diag 3x(8x8 ones)
#  [0:8, 216:280] E8o: row e, col j -> 1 if e == j%8
#  [0:1, 280:344] ones64 row
CST_W = 344


def _build_cst():
    cst = np.zeros((24, CST_W), dtype=np.float32)
    for j in range(64):
        cst[j % 8, j] = 1.0           # E_A q half
        cst[8 + j % 8, 64 + j] = 1.0  # E_A k half
        cst[16 + j % 8, 128 + j] = 1.0  # E_v
        cst[j % 8, 216 + j] = 1.0     # E8o
        cst[0, 280 + j] = 1.0         # ones64
    for b in range(3):
        cst[8 * b:8 * b + 8, 192 + 8 * b:200 + 8 * b] = 1.0  # ones24
    return _bf(cst)


def build(mask_cls, mask_lo):
    """Build the SPMD Bass graph. mask_cls: [NKT, NQB] int array."""
    nc = bacc.Bacc(None, target_bir_lowering=False)

    # ---- DRAM I/O (per-core shards prepared on host) ----
    xT = nc.declare_dram_parameter("xT", [D, S], BF16, isOutput=False)
    wqT = nc.declare_dram_parameter("wqT", [D, QF], BF16, isOutput=False)
    wkvT = nc.declare_dram_parameter("wkvT", [D, 2 * KF], BF16, isOutput=False)
    aA = nc.declare_dram_parameter("aA", [D, 128], BF16, isOutput=False)
    aB = nc.declare_dram_parameter("aB", [D, 88], BF16, isOutput=False)
    ao = nc.declare_dram_parameter("ao", [D, 72], BF16, isOutput=False)
    bq = nc.declare_dram_parameter("bq", [E * R, QF], BF16, isOutput=False)
    bk = nc.declare_dram_parameter("bk", [E * R, KF], BF16, isOutput=False)
    bv = nc.declare_dram_parameter("bv", [E * R, KF], BF16, isOutput=False)
    bo = nc.declare_dram_parameter("bo", [E * R, D], BF16, isOutput=False)
    woT = nc.declare_dram_parameter("woT", [D, D], BF16, isOutput=False)
    cs4 = nc.declare_dram_parameter("cs4", [128, S], BF16, isOutput=False)
    sn4 = nc.declare_dram_parameter("sn4", [128, S], BF16, isOutput=False)
    mask01 = nc.declare_dram_parameter("mask01", [S, S], BF16, isOutput=False)
    cst = nc.declare_dram_parameter("cst", [24, CST_W], BF16, isOutput=False)
    y = nc.declare_dram_parameter("y", [TSH, D], F32, isOutput=True)

    # internal DRAM for the chunked collectives: [dest/src, 4h*64 feat, 64 tok]
    cc_in = [nc.dram_tensor("cc_in%d" % q, [NCORES, QF, 64], BF16)
             for q in range(NQB)]
    cc_out = [nc.dram_tensor("cc_out%d" % q, [NCORES, QF, 64], BF16)
              for q in range(NQB)]

    with tile.TileContext(nc) as tc:
        _emit(nc, tc, locals(), mask_cls, mask_lo)
    nc.finalize()
    return nc


def _emit(nc, tc, t, mask_cls, mask_lo):
    xT, wqT, wkvT = t["xT"], t["wqT"], t["wkvT"]
    aA, aB, ao = t["aA"], t["aB"], t["ao"]
    bq, bk, bv, bo = t["bq"], t["bk"], t["bv"], t["bo"]
    woT, cs4, sn4, mask01, y = t["woT"], t["cs4"], t["sn4"], t["mask01"], t["y"]
    cst = t["cst"]
    cc_in, cc_out = t["cc_in"], t["cc_out"]

    import contextlib
    ctx = contextlib.ExitStack()
    with ctx:
        persist = ctx.enter_context(tc.tile_pool(name="persist", bufs=1))
        ps = ctx.enter_context(tc.tile_pool(name="ps", bufs=1, space="PSUM"))

        # ---- persistent weights, split in k-groups of 4 for early start ----
        NSP = 4
        KG = NIF // NSP
        aA_sb, aB_sb, wq_sb, wkv_sb = [], [], [], []
        xq0 = persist.tile([128, NIF, 512], BF16, name="xq0")
        for g in range(NSP):
            ksl = slice(g * KG * 128, (g + 1) * KG * 128)
            tl = persist.tile([128, KG, 128], BF16, name="aA%d" % g)
            nc.scalar.dma_start(
                out=tl, in_=aA[ksl].rearrange("(n p) f -> p n f", p=128))
            aA_sb.append(tl)
            tl = persist.tile([128, KG, 88], BF16, name="aB%d" % g)
            nc.scalar.dma_start(
                out=tl, in_=aB[ksl].rearrange("(n p) f -> p n f", p=128))
            aB_sb.append(tl)
            # first token block's x rides ahead of the q weights
            nc.sync.dma_start(
                out=xq0[:, g * KG:(g + 1) * KG, :],
                in_=xT[ksl].rearrange("(n p) t -> p n t", p=128)[:, :, 0:512])
            tl = persist.tile([128, KG, QF], BF16, name="wq%d" % g)
            nc.sync.dma_start(
                out=tl, in_=wqT[ksl].rearrange("(n p) f -> p n f", p=128))
            wq_sb.append(tl)
            tl = persist.tile([128, KG, 2 * KF], BF16, name="wkv%d" % g)
            nc.gpsimd.dma_start(
                out=tl, in_=wkvT[ksl].rearrange("(n p) f -> p n f", p=128))
            wkv_sb.append(tl)

        def A_AT(k):  # aA chain lhsT for contraction tile k
            return aA_sb[k // KG][:, k % KG, :]

        def A_BT(k):
            return aB_sb[k // KG][:, k % KG, :]

        def W_Q(k):
            return wq_sb[k // KG][:, k % KG, :]

        def W_KV(k):
            return wkv_sb[k // KG][:, k % KG, :]

        cst_sb = persist.tile([24, CST_W], BF16)
        nc.gpsimd.dma_start(out=cst_sb, in_=cst[:])
        E_A = cst_sb[:, 0:128]
        E_v = cst_sb[0:24, 128:192]
        ones24 = cst_sb[:, 192:216]
        ones8 = cst_sb[0:8, 192:200]
        E8o = cst_sb[0:8, 216:280]
        ones64 = cst_sb[0:1, 280:344]

        bq_sb = persist.tile([64, QF], BF16)
        nc.gpsimd.dma_start(out=bq_sb, in_=bq[:])
        bk_sb = persist.tile([128, KF], BF16)   # bk lives at partitions 64:128
        nc.gpsimd.dma_start(out=bk_sb[64:128, :], in_=bk[:])
        bv_sb = persist.tile([64, KF], BF16)
        nc.gpsimd.dma_start(out=bv_sb, in_=bv[:])
        bo_sb = persist.tile([64, D], BF16)
        nc.gpsimd.dma_start(out=bo_sb, in_=bo[:])
        ao_sb = persist.tile([128, NIF, 72], BF16)
        nc.scalar.dma_start(out=ao_sb,
                            in_=ao.rearrange("(n p) f -> p n f", p=128))
        cs_sb = persist.tile([128, S], BF16)
        nc.scalar.dma_start(out=cs_sb, in_=cs4[:])
        sn_sb = persist.tile([128, S], BF16)
        nc.scalar.dma_start(out=sn_sb, in_=sn4[:])

        ident_b = persist.tile([128, 128], BF16)
        make_identity(nc, ident_b)

        # attention operands (persist across phases)
        qh_sb = persist.tile([128, 2, S], BF16)   # [2 heads x 64, page, S]
        kh_sb = persist.tile([128, S], BF16)      # kv head duplicated 2x
        vtok = persist.tile([128, NKT, 65], BF16)  # token-major v + ones col
        nc.vector.memset(vtok[:, :, 64:65], 1.0)
        g_sb = persist.tile([128, NIF, TSH], BF16)  # gathered out (post-A2A)

        # ================= Phase A: QKV + LoRA + RoPE =================
        vT_all = persist.tile([64, S], BF16)      # v (feat-major) staging
        with tc.tile_pool(name="pA", bufs=1) as pA:
            for tb in range(4):
                tsl = slice(tb * 512, (tb + 1) * 512)
                if tb == 0:
                    xq = xq0
                else:
                    xq = pA.tile([128, NIF, 512], BF16, name="xq", tag="xq",
                                 bufs=3)
                    nc.sync.dma_start(
                        out=xq,
                        in_=xT.rearrange("(n p) t -> p n t", p=128)[:, :, tsl])

                # ---- main projection chains ----
                hA = ps.tile([128, 512], F32, name="hA", tag="p_hA")
                hB = ps.tile([88, 512], F32, name="hB", tag="p_hB")
                q0 = ps.tile([128, 512], F32, name="q0", tag="p_q0")
                q1 = ps.tile([128, 512], F32, name="q1", tag="p_q1")
                kv = ps.tile([128, 512], F32, name="kv", tag="p_kv")
                for k in range(NIF):
                    st = k == 0
                    sp = k == NIF - 1
                    rhs = xq[:, k, :]
                    nc.tensor.matmul(hA, A_AT(k), rhs, start=st, stop=sp)
                    nc.tensor.matmul(hB, A_BT(k), rhs, start=st, stop=sp)
                for k in range(NIF):
                    rhs = xq[:, k, :]
                    st = k == 0
                    nc.tensor.matmul(q0, W_Q(k)[:, 0:128], rhs,
                                     start=st, stop=False)
                    nc.tensor.matmul(q1, W_Q(k)[:, 128:256], rhs,
                                     start=st, stop=False)
                    nc.tensor.matmul(kv, W_KV(k), rhs, start=st, stop=False)

                # ---- router softmax (q,k,v fused; no transposes) ----
                ex3 = pA.tile([24, 512], BF16, name="ex3", tag="ex3", bufs=2)
                nc.scalar.activation(ex3, hB[64:88, :], AF.Exp)
                s3 = ps.tile([24, 512], F32, name="s3", tag="p_s3")
                nc.tensor.matmul(s3, ones24, ex3, start=True, stop=True)
                rec3 = pA.tile([24, 512], BF16, name="rec3", tag="rec3",
                               bufs=2)
                with nc.allow_low_precision(reason="router softmax denom"):
                    nc.vector.reciprocal(rec3, s3)
                rw3 = pA.tile([24, 512], BF16, name="rw3", tag="rw3", bufs=2)
                nc.vector.tensor_tensor(rw3, ex3, rec3, AluOpType.mult)
                rwbA = ps.tile([128, 512], F32, name="rwbA", tag="p_rwA")
                nc.tensor.matmul(rwbA, E_A, rw3, start=True, stop=True)
                rwbV = ps.tile([64, 512], F32, name="rwbV", tag="p_rwV")
                nc.tensor.matmul(rwbV, E_v, rw3, start=True, stop=True)
                rwbA_s = pA.tile([128, 512], BF16, name="rwbA_s",
                                 tag="rwbA_s", bufs=2)
                nc.scalar.activation(rwbA_s, rwbA, AF.Copy)
                rwbV_s = pA.tile([64, 512], BF16, name="rwbV_s",
                                 tag="rwbV_s", bufs=2)
                nc.scalar.activation(rwbV_s, rwbV, AF.Copy)
                hpA = pA.tile([128, 512], BF16, name="hpA", tag="hpA", bufs=2)
                nc.vector.tensor_tensor(hpA, hA, rwbA_s, AluOpType.mult)
                hpV = pA.tile([64, 512], BF16, name="hpV", tag="hpV", bufs=2)
                nc.vector.tensor_tensor(hpV, hB[0:64, :], rwbV_s,
                                        AluOpType.mult)

                # ---- LoRA-B closes the accumulations ----
                nc.tensor.matmul(q0, bq_sb[:, 0:128], hpA[0:64, :],
                                 start=False, stop=True)
                nc.tensor.matmul(q1, bq_sb[:, 128:256], hpA[0:64, :],
                                 start=False, stop=True)
                nc.tensor.matmul(kv[0:64, :], bk_sb[64:128, :],
                                 hpA[64:128, :], start=False, stop=True,
                                 tile_position=(64, 0))
                nc.tensor.matmul(kv[64:128, :], bv_sb, hpV,
                                 start=False, stop=True,
                                 tile_position=(0, 64))

                # ---- PSUM extraction (scalar engine) ----
                qe = pA.tile([128, 512], BF16, name="qe", tag="qe", bufs=2)
                qo = pA.tile([128, 512], BF16, name="qo", tag="qo", bufs=2)
                nc.scalar.activation(qe[0:64, :], q0[0:64, :], AF.Copy)
                nc.scalar.activation(qe[64:128, :], q1[0:64, :], AF.Copy)
                nc.scalar.activation(qo[0:64, :], q0[64:128, :], AF.Copy)
                nc.scalar.activation(qo[64:128, :], q1[64:128, :], AF.Copy)
                ke = pA.tile([32, 512], BF16, name="ke", tag="ke", bufs=2)
                ko = pA.tile([32, 512], BF16, name="ko", tag="ko", bufs=2)
                nc.scalar.activation(ke, kv[0:32, :], AF.Copy)
                nc.scalar.activation(ko, kv[32:64, :], AF.Copy)
                nc.scalar.activation(vT_all[:, tsl], kv[64:128, :], AF.Copy)

                # ---- RoPE (bf16, vector engine) ----
                cs_t = cs_sb[:, tsl]
                sn_t = sn_sb[:, tsl]
                t1 = pA.tile([128, 512], BF16, name="t1", tag="t1", bufs=2)
                t2 = pA.tile([128, 512], BF16, name="t2", tag="t2", bufs=2)
                rote = pA.tile([128, 512], BF16, name="rote", tag="rote",
                               bufs=2)
                roto = pA.tile([128, 512], BF16, name="roto", tag="roto",
                               bufs=2)
                nc.vector.tensor_tensor(t1, qe, cs_t, AluOpType.mult)
                nc.vector.tensor_tensor(t2, qo, sn_t, AluOpType.mult)
                nc.vector.tensor_tensor(rote, t1, t2, AluOpType.subtract)
                nc.vector.tensor_tensor(t1, qe, sn_t, AluOpType.mult)
                nc.vector.tensor_tensor(t2, qo, cs_t, AluOpType.mult)
                nc.vector.tensor_tensor(roto, t1, t2, AluOpType.add)
                k1 = pA.tile([32, 512], BF16, name="k1", tag="k1", bufs=2)
                k2 = pA.tile([32, 512], BF16, name="k2", tag="k2", bufs=2)
                csk = cs_sb[0:32, tsl]
                snk = sn_sb[0:32, tsl]
                nc.vector.tensor_tensor(k1, ke, csk, AluOpType.mult)
                nc.vector.tensor_tensor(k2, ko, snk, AluOpType.mult)
                nc.vector.tensor_tensor(kh_sb[0:32, tsl], k1, k2,
                                        AluOpType.subtract)
                nc.vector.tensor_tensor(k1, ke, snk, AluOpType.mult)
                nc.vector.tensor_tensor(k2, ko, csk, AluOpType.mult)
                nc.vector.tensor_tensor(kh_sb[32:64, tsl], k1, k2,
                                        AluOpType.add)

                # head rearrange via SBUF->SBUF DMA (off the engines)
                nc.gpsimd.dma_start(out=kh_sb[64:128, tsl],
                                    in_=kh_sb[0:64, tsl])
                for h in range(QH):
                    page, i = h // 2, h % 2
                    nc.gpsimd.dma_start(
                        out=qh_sb[64 * i:64 * i + 32, page, tsl],
                        in_=rote[32 * h:32 * h + 32, :])
                    nc.gpsimd.dma_start(
                        out=qh_sb[64 * i + 32:64 * i + 64, page, tsl],
                        in_=roto[32 * h:32 * h + 32, :])

            # token-major v, built after the per-block pipeline drains
            for kt in range(NKT):
                v_ps = ps.tile([128, 64], BF16, name="v_ps", tag="p_s3")
                nc.tensor.transpose(v_ps,
                                    vT_all[:, 128 * kt:128 * kt + 128],
                                    ident_b[0:64, 0:64])
                nc.vector.tensor_copy(vtok[:, kt, 0:64], v_ps)

        # prefetch the full output-projection weight during attention
        wo_ctx = tc.tile_pool(name="wo_pool", bufs=4)
        wo_pool = wo_ctx.__enter__()
        wo_tiles = []
        for ob in range(4):
            osl = slice(ob * 512, (ob + 1) * 512)
            wo_sb = wo_pool.tile([128, NIF, 512], BF16, name="wo_sb",
                                 tag="wo", bufs=4)
            nc.gpsimd.dma_start(
                out=wo_sb,
                in_=woT.rearrange("(n p) f -> p n f", p=128)[:, :, osl])
            wo_tiles.append(wo_sb)

        # ============ Phase C: attention + chunked A2A ============
        SC_TAGS = ["p_q0", "p_q1", "p_hA", "p_hB"]
        OUT_TAGS = ["p_kv", "p_s3", "p_rwA", "p_rwV"]
        with tc.tile_pool(name="pC", bufs=1) as pC:
            def emit_tail(qb, outps, gather=True):
                """Normalize + ship chunk qb. Emitted after the next query
                block's first score/exp round so the reciprocal latency
                hides under attention compute."""
                for h in range(QH):
                    dens = pC.tile([1, 512], F32, name="dens%d" % h,
                                   tag="dens%d" % h, bufs=2)
                    nc.vector.tensor_copy(dens, outps[h][64:65, :])
                    recf = pC.tile([1, 512], F32, name="recf%d" % h,
                                   tag="recf%d" % h, bufs=2)
                    nc.vector.reciprocal_approx_fast(out=recf, in_=dens)
                    rec = pC.tile([1, 512], BF16, name="rec%d" % h,
                                  tag="rec%d" % h, bufs=2)
                    nc.vector.tensor_copy(rec, recf)
                    rb = ps.tile([64, 512], F32, name="rb", tag=SC_TAGS[h])
                    nc.tensor.matmul(rb, ones64, rec, start=True, stop=True)
                    rb_s = pC.tile([64, 512], BF16, name="rb_s%d" % h,
                                   tag="rb_s%d" % h, bufs=2)
                    nc.vector.tensor_copy(rb_s, rb)
                    o65 = pC.tile([64, 512], BF16, name="o65%d" % h,
                                  tag="o65%d" % h, bufs=2)
                    nc.vector.tensor_tensor(o65, outps[h][0:64, :], rb_s,
                                            AluOpType.mult)
                    # [64, 512] -> cc_in[qb][dest, 64h:64h+64, 0:64]
                    nc.gpsimd.dma_start(
                        out=cc_in[qb][:, 64 * h:64 * h + 64, :]
                            .rearrange("d p t -> p d t"),
                        in_=o65)
                nc.gpsimd.collective_compute(
                    "AllToAll",
                    AluOpType.bypass,
                    ins=[cc_in[qb][:]],
                    outs=[cc_out[qb][:]],
                    replica_groups=[list(range(NCORES))],
                )
                if gather:
                    # gather this chunk into g_sb[:, :, 64qb:64qb+64]
                    nc.sync.dma_start(
                        out=g_sb[:, :, 64 * qb:64 * qb + 64],
                        in_=cc_out[qb].rearrange("s (k p) t -> p (s k) t",
                                                 k=2, p=128))

            pending = None
            yp_tt0 = {}
            for qb in range(NQB):
                qsl = slice(qb * 512, (qb + 1) * 512)
                active = [kt for kt in range(NKT)
                          if mask_cls[kt, qb] != M_SKIP]
                assert active, f"fully masked query block qb={qb}"
                outps = [ps.tile([65, 512], F32, name="outp%d" % h,
                                 tag=OUT_TAGS[h]) for h in range(QH)]
                for idx, kt in enumerate(active):
                    c = mask_cls[kt, qb]
                    lo = int(mask_lo[kt, qb]) if c == M_ADD else 0
                    mt = None
                    if c == M_ADD:
                        mt = pC.tile([128, 128], BF16, name="mt",
                                     tag="mt", bufs=4)
                        nc.sync.dma_start(
                            out=mt,
                            in_=mask01[128 * kt:128 * kt + 128,
                                       512 * qb + lo:512 * qb + lo + 128])
                    ksl = slice(128 * kt, 128 * kt + 128)
                    qslc = slice(512 * qb + lo, 512 * (qb + 1))
                    prs = []
                    for h in range(QH):
                        page, i = h // 2, h % 2
                        sc = ps.tile([128, 512], F32, name="sc%d" % h,
                                     tag=SC_TAGS[h])
                        nc.tensor.matmul(sc[:, lo:512],
                                         kh_sb[64 * i:64 * i + 64, ksl],
                                         qh_sb[64 * i:64 * i + 64, page,
                                               qslc],
                                         start=True, stop=True,
                                         tile_position=(64 * i, 0))
                        pr = pC.tile([128, 512], BF16, name="pr%d" % h,
                                     tag="pr%d" % h, bufs=2)
                        nc.scalar.activation(pr[:, lo:512], sc[:, lo:512],
                                             AF.Exp)
                        if mt is not None:
                            nc.vector.tensor_tensor(pr[:, lo:lo + 128],
                                                    pr[:, lo:lo + 128], mt,
                                                    AluOpType.mult)
                        prs.append(pr)
                    if idx == 0 and pending is not None:
                        # previous block's normalization rides behind this
                        # round's score matmuls
                        emit_tail(qb - 1, pending)
                        pending = None
                    for h in range(QH):
                        nc.tensor.matmul(outps[h][:, lo:512],
                                         vtok[:, kt, :],
                                         prs[h][:, lo:512],
                                         start=(kt == active[0]),
                                         stop=(kt == active[-1]))
                pending = outps
            emit_tail(NQB - 1, pending, gather=False)
            # o-proj token-half 0 (chunks 0/1 landed long ago) rides the
            # tensor engine under the final AllToAll
            YP0_TAGS = ["p_q0", "p_q1", "p_hA", "p_kv"]
            for ob in range(4):
                osl = slice(ob * 512, (ob + 1) * 512)
                yp = ps.tile([128, 512], F32, name="yp0_%d" % ob,
                             tag=YP0_TAGS[ob])
                for k in range(NIF):
                    nc.tensor.matmul(yp, g_sb[:, k, 0:128],
                                     wo_tiles[ob][:, k, :],
                                     start=(k == 0), stop=False)
                yp_tt0[ob] = yp
            nc.sync.dma_start(
                out=g_sb[:, :, 64 * (NQB - 1):64 * NQB],
                in_=cc_out[NQB - 1].rearrange("s (k p) t -> p (s k) t",
                                              k=2, p=128))

        # ================= Phase D: o-proj =================
        with tc.tile_pool(name="pD", bufs=1) as pD:
            ho = ps.tile([72, TSH], F32, name="ho", tag="p_hB")
            for k in range(NIF):
                nc.tensor.matmul(ho, ao_sb[:, k, :], g_sb[:, k, :],
                                 start=(k == 0), stop=(k == NIF - 1))
            exo = pD.tile([8, TSH], BF16, name="exo")
            nc.scalar.activation(exo, ho[64:72, :], AF.Exp)
            so = ps.tile([8, TSH], F32, name="so", tag="p_s3")
            nc.tensor.matmul(so, ones8, exo, start=True, stop=True)
            reco = pD.tile([8, TSH], BF16, name="reco")
            with nc.allow_low_precision(reason="o-router softmax denom"):
                nc.vector.reciprocal(reco, so)
            rwo = pD.tile([8, TSH], BF16, name="rwo")
            nc.vector.tensor_tensor(rwo, exo, reco, AluOpType.mult)
            rwbo = ps.tile([64, TSH], F32, name="rwbo", tag="p_rwA")
            nc.tensor.matmul(rwbo, E8o, rwo, start=True, stop=True)
            rwbo_s = pD.tile([64, TSH], BF16, name="rwbo_s")
            nc.vector.tensor_copy(rwbo_s, rwbo)
            hpo = pD.tile([64, TSH], BF16, name="hpo")
            nc.vector.tensor_tensor(hpo, ho[0:64, :], rwbo_s, AluOpType.mult)

            for ob in range(4):
                osl = slice(ob * 512, (ob + 1) * 512)
                yp = yp_tt0[ob]
                nc.tensor.matmul(yp, hpo[:, 0:128], bo_sb[:, osl],
                                 start=False, stop=True)
                yt = pD.tile([128, 512], F32, name="yt", tag="yt", bufs=3)
                if ob % 2 == 0:
                    nc.scalar.activation(yt, yp, AF.Copy)
                else:
                    nc.vector.tensor_copy(yt, yp)
                nc.sync.dma_start(out=y[0:128, osl], in_=yt)
            for ob in range(4):
                osl = slice(ob * 512, (ob + 1) * 512)
                yp = ps.tile([128, 512], F32, name="yp1_%d" % ob,
                             tag=["p_hB", "p_s3", "p_rwA", "p_rwV"][ob])
                for k in range(NIF):
                    nc.tensor.matmul(yp, g_sb[:, k, 128:256],
                                     wo_tiles[ob][:, k, :],
                                     start=(k == 0), stop=False)
                nc.tensor.matmul(yp, hpo[:, 128:256], bo_sb[:, osl],
                                 start=False, stop=True)
                yt = pD.tile([128, 512], F32, name="yt", tag="yt", bufs=3)
                if ob % 2 == 0:
                    nc.scalar.activation(yt, yp, AF.Copy)
                else:
                    nc.vector.tensor_copy(yt, yp)
                nc.sync.dma_start(out=y[128:256, osl], in_=yt)
        wo_ctx.__exit__(None, None, None)


# ======================= host side =======================

_CACHE = {}


def _prep_inputs(x, mask, freqs_cos, freqs_sin, wq, wk, wv, wo,
                 lq_router, lq_A, lq_B, lk_router, lk_A, lk_B,
                 lv_router, lv_A, lv_B, lo_router, lo_A, lo_B):
    scale = 1.0 / np.sqrt(HD)
    x = _f32(np.asarray(x)).reshape(S, D)
    maskf = _f32(np.asarray(mask)).reshape(S, S)
    maskT = np.maximum(maskf, MASK_NEG).T.copy()
    mask_cls, mask_lo = classify_mask(maskT)
    mask01 = _bf((maskT > MASK_NEG * 0.5).astype(np.float32))

    xT = _bf(x.T)
    cs4 = _bf(np.tile(_f32(freqs_cos).T, (4, 1)))      # [128, S]
    sn4 = _bf(np.tile(_f32(freqs_sin).T, (4, 1)))
    woT = _bf(_f32(wo).T)
    ao_p = _bf(np.concatenate([_a_pack(_f32(lo_A)), _f32(lo_router).T],
                              axis=1))                 # [D, 72]
    bo_f = _bf(_b_flat(_f32(lo_B), SCALING))

    # fused LoRA-A stationaries: [D, 128] = [aq|ak], [D, 88] = [av|rq|rk|rv]
    aA_p = _bf(np.concatenate(
        [_a_pack(_f32(lq_A)), _a_pack(_f32(lk_A))], axis=1))
    aB_p = _bf(np.concatenate(
        [_a_pack(_f32(lv_A)), _f32(lq_router).T, _f32(lk_router).T,
         _f32(lv_router).T], axis=1))

    shared = dict(xT=xT, cs4=cs4, sn4=sn4, woT=woT, mask01=mask01,
                  ao=ao_p, bo=bo_f, cst=_build_cst(), aA=aA_p, aB=aB_p)

    wqf, wkf, wvf = _f32(wq), _f32(wk), _f32(wv)
    lqB, lkB, lvB = _f32(lq_B), _f32(lk_B), _f32(lv_B)

    in_maps = []
    for c in range(NCORES):
        wq_c = wqf[c * QF:(c + 1) * QF][IDX_Q] * scale
        wk_c = wkf[c * KF:(c + 1) * KF][IDX_K]
        wv_c = wvf[c * KF:(c + 1) * KF]
        bq_c = _b_flat(lqB[:, c * QF:(c + 1) * QF, :][:, IDX_Q, :],
                       SCALING * scale)
        bk_c = _b_flat(lkB[:, c * KF:(c + 1) * KF, :][:, IDX_K, :], SCALING)
        bv_c = _b_flat(lvB[:, c * KF:(c + 1) * KF, :], SCALING)
        m = dict(shared)
        m.update(wqT=_bf(wq_c.T),
                 wkvT=_bf(np.concatenate([wk_c.T, wv_c.T], axis=1)),
                 bq=_bf(bq_c), bk=_bf(bk_c), bv=_bf(bv_c))
        in_maps.append(m)
    return in_maps, mask_cls, mask_lo


def get_graph(mask_cls, mask_lo):
    key = mask_cls.tobytes() + mask_lo.tobytes()
    if key not in _CACHE:
        _CACHE[key] = build(mask_cls, mask_lo)
    return _CACHE[key]


def kernel(x, start_pos, mask, freqs_cos, freqs_sin, wq, wk, wv, wo,
           lq_router, lq_A, lq_B, lk_router, lk_A, lk_B,
           lv_router, lv_A, lv_B, lo_router, lo_A, lo_B,
           _trace=False):
    from concourse.bass_utils import run_bass_kernel_spmd
    in_maps, mask_cls, mask_lo = _prep_inputs(
        x, mask, freqs_cos, freqs_sin, wq, wk, wv, wo,
        lq_router, lq_A, lq_B, lk_router, lk_A, lk_B,
        lv_router, lv_A, lv_B, lo_router, lo_A, lo_B)
    nc = get_graph(mask_cls, mask_lo)
    res = run_bass_kernel_spmd(nc, in_maps, list(range(NCORES)), trace=_trace)
    # core c's y rows: group g (0..3) covers tokens [512g + 64c, 512g + 64c + 64)
    ys = np.stack([res.results[c]["y"] for c in range(NCORES)], axis=0)
    ys = ys.reshape(NCORES, 4, 64, D).transpose(1, 0, 2, 3).reshape(S, D)
    out = ys.reshape(B, S, H * HD).astype(np.float32)
    if _trace:
        return out, res
    return out


# revision 19
# speedup vs baseline: 1.4194x; 1.0062x over previous
"""Trainium2 Bass kernel for MoE-LoRA GQA attention (nn_Attention_57389353009692).

V2 strategy (8 NeuronCores, one SPMD launch):
  - Tensor-parallel over heads: core c owns q-heads 4c..4c+3 and kv-head c.
  - Phase A (per 512-token block): QKV projections (+ MoE-LoRA) with packed
    matmul chains (wk|wv fused; LoRA-A for q/k/v + all three routers fused
    into two chains of 128/88 rows), router softmax done with
    exp -> ones-matmul row-sum -> reciprocal -> broadcast-matmul (no
    transposes, no DRAM bounce), RoPE in bf16 on 128 partitions.
  - Phase C: flash-style attention per 512-query block; causal mask applied
    as a 0/1 multiply after exp (bf16); output normalized PRE-collective via
    reciprocal-of-denominator broadcast matmuls fused into the PSUM->SBUF
    cast.
  - AllToAll is chunked per query block (4 collectives) and overlaps the
    remaining attention compute. Output tokens are interleaved at
    64-granularity: core c owns tokens {t : (t//64) % 8 == c} so every chunk
    is a uniform 8-way exchange.
  - Phase D: o-projection + o-LoRA for the core's 256 tokens with the full
    (prefetched) wo.

Numerics: bf16 operands, fp32 PSUM accumulation, fp32->exp softmax without
max subtraction (scores are O(1) here; masked entries are zeroed exactly by
the 0/1 multiply). Scale 1/sqrt(64) folded into wq and q-LoRA-B on host.
RoPE trick: interleaved even/odd pairs are made contiguous by permuting
wq/wk output features on host (per 2-head "page": [h0e|h1e|h0o|h1o]).
"""

import sys

for _p in ("/opt/trn_rl_repo", "/root/.axon_site/_ro/trn_rl_repo"):
    if _p not in sys.path:
        sys.path.insert(0, _p)

import numpy as np
import ml_dtypes

import concourse.bass as bass
import concourse.tile as tile
from concourse import bacc, mybir
from concourse.masks import make_identity
from concourse.alu_op_type import AluOpType

F32 = mybir.dt.float32
BF16 = mybir.dt.bfloat16
AF = mybir.ActivationFunctionType
AX = mybir.AxisListType
BF16NP = ml_dtypes.bfloat16

B, S, D = 1, 2048, 2048
H, KVH, HD = 32, 8, 64
NREP = H // KVH
R, E = 8, 8
SCALING = 32.0 / 8.0
NCORES = 8
QH = H // NCORES          # 4 q heads per core
QF = QH * HD              # 256 q feats per core
KF = HD                   # 64 kv feats per core
TSH = S // NCORES         # 256 tokens per core for o-proj
NKT = S // 128            # 16 key tiles
NQB = S // 512            # 4 query blocks
NIF = D // 128            # 16 contraction tiles

MASK_NEG = -1e30

# mask tile classes
M_SKIP, M_ZERO, M_ADD = 0, 1, 2


def _build_perm():
    """Per-core feature permutations for rope-friendly layout."""
    idx_q = np.zeros(QF, dtype=np.int64)
    for f in range(QF):
        page, w = divmod(f, 128)
        if w < 32:
            hl, j, odd = 2 * page, w, 0
        elif w < 64:
            hl, j, odd = 2 * page + 1, w - 32, 0
        elif w < 96:
            hl, j, odd = 2 * page, w - 64, 1
        else:
            hl, j, odd = 2 * page + 1, w - 96, 1
        idx_q[f] = 64 * hl + 2 * j + odd
    idx_k = np.zeros(KF, dtype=np.int64)
    for w in range(KF):
        if w < 32:
            idx_k[w] = 2 * w
        else:
            idx_k[w] = 2 * (w - 32) + 1
    return idx_q, idx_k


IDX_Q, IDX_K = _build_perm()


def _a_pack(A):
    """[E,R,D] -> [D, 64] with col r*8+e."""
    return np.transpose(A, (1, 0, 2)).reshape(E * R, -1).T


def _b_flat(Bw, scale):
    """[E, OF, R] -> [64, OF] with row r*8+e."""
    return (np.transpose(Bw, (2, 0, 1)).reshape(E * R, -1) * scale)


def _bf(x):
    return np.ascontiguousarray(x, dtype=np.float32).astype(BF16NP)


def _f32(x):
    return np.ascontiguousarray(x, dtype=np.float32)


def classify_mask(maskT):
    """maskT: [S(k), S(q)] clamped fp32. Returns ([NKT, NQB] class map,
    [NKT, NQB] live-start-column map for M_ADD tiles).

    For an M_ADD tile, lo is the first live column, rounded down to 128;
    columns >= lo+128 must be fully live (causal staircase) -- the kernel
    then computes only [lo, 512) and masks just [lo, lo+128)."""
    cls = np.zeros((NKT, NQB), dtype=np.int64)
    los = np.zeros((NKT, NQB), dtype=np.int64)
    for kt in range(NKT):
        blk_rows = maskT[kt * 128:(kt + 1) * 128]
        for qb in range(NQB):
            blk = blk_rows[:, qb * 512:(qb + 1) * 512]
            if np.all(blk <= MASK_NEG * 0.5):
                cls[kt, qb] = M_SKIP
            elif np.all(blk == 0.0):
                cls[kt, qb] = M_ZERO
            else:
                cls[kt, qb] = M_ADD
                live = np.where((blk == 0.0).any(axis=0))[0]
                lo = (int(live[0]) // 128) * 128 if len(live) else 0
                if lo + 128 <= 512 and not np.all(blk[:, lo + 128:] == 0.0):
                    lo = 0  # not a causal staircase; keep full width
                los[kt, qb] = lo
    return cls, los


# constants tensor layout (bf16, [24, 344]):
#  [:, 0:128]   E_A: row e, col j -> 1 if (j<64 and e==j%8) or (j>=64 and e-8==j%8)
#  [:, 128:192] E_v: row e, col j -> 1 if e-16 == j%8
#  [:, 192:216] ones24: block-diag 3x(8x8 ones)
#  [0:8, 216:280] E8o: row e, col j -> 1 if e == j%8
#  [0:1, 280:344] ones64 row
CST_W = 344


def _build_cst():
    cst = np.zeros((24, CST_W), dtype=np.float32)
    for j in range(64):
        cst[j % 8, j] = 1.0           # E_A q half
        cst[8 + j % 8, 64 + j] = 1.0  # E_A k half
        cst[16 + j % 8, 128 + j] = 1.0  # E_v
        cst[j % 8, 216 + j] = 1.0     # E8o
        cst[0, 280 + j] = 1.0         # ones64
    for b in range(3):
        cst[8 * b:8 * b + 8, 192 + 8 * b:200 + 8 * b] = 1.0  # ones24
    return _bf(cst)


def build(mask_cls, mask_lo):
    """Build the SPMD Bass graph. mask_cls: [NKT, NQB] int array."""
    nc = bacc.Bacc(None, target_bir_lowering=False)

    # ---- DRAM I/O (per-core shards prepared on host) ----
    xT = nc.declare_dram_parameter("xT", [D, S], BF16, isOutput=False)
    wqT = nc.declare_dram_parameter("wqT", [D, QF], BF16, isOutput=False)
    wkvT = nc.declare_dram_parameter("wkvT", [D, 2 * KF], BF16, isOutput=False)
    aA = nc.declare_dram_parameter("aA", [D, 128], BF16, isOutput=False)
    aB = nc.declare_dram_parameter("aB", [D, 88], BF16, isOutput=False)
    ao = nc.declare_dram_parameter("ao", [D, 72], BF16, isOutput=False)
    bq = nc.declare_dram_parameter("bq", [E * R, QF], BF16, isOutput=False)
    bk = nc.declare_dram_parameter("bk", [E * R, KF], BF16, isOutput=False)
    bv = nc.declare_dram_parameter("bv", [E * R, KF], BF16, isOutput=False)
    bo = nc.declare_dram_parameter("bo", [E * R, D], BF16, isOutput=False)
    woT = nc.declare_dram_parameter("woT", [D, D], BF16, isOutput=False)
    cs4 = nc.declare_dram_parameter("cs4", [128, S], BF16, isOutput=False)
    sn4 = nc.declare_dram_parameter("sn4", [128, S], BF16, isOutput=False)
    mask01 = nc.declare_dram_parameter("mask01", [S, S], BF16, isOutput=False)
    cst = nc.declare_dram_parameter("cst", [24, CST_W], BF16, isOutput=False)
    y = nc.declare_dram_parameter("y", [TSH, D], F32, isOutput=True)

    # internal DRAM for the chunked collectives: [dest/src, 4h*64 feat, 64 tok]
    cc_in = [nc.dram_tensor("cc_in%d" % q, [NCORES, QF, 64], BF16)
             for q in range(NQB)]
    cc_out = [nc.dram_tensor("cc_out%d" % q, [NCORES, QF, 64], BF16)
              for q in range(NQB)]

    with tile.TileContext(nc) as tc:
        _emit(nc, tc, locals(), mask_cls, mask_lo)
    nc.finalize()
    return nc


def _emit(nc, tc, t, mask_cls, mask_lo):
    xT, wqT, wkvT = t["xT"], t["wqT"], t["wkvT"]
    aA, aB, ao = t["aA"], t["aB"], t["ao"]
    bq, bk, bv, bo = t["bq"], t["bk"], t["bv"], t["bo"]
    woT, cs4, sn4, mask01, y = t["woT"], t["cs4"], t["sn4"], t["mask01"], t["y"]
    cst = t["cst"]
    cc_in, cc_out = t["cc_in"], t["cc_out"]

    import contextlib
    ctx = contextlib.ExitStack()
    with ctx:
        persist = ctx.enter_context(tc.tile_pool(name="persist", bufs=1))
        ps = ctx.enter_context(tc.tile_pool(name="ps", bufs=1, space="PSUM"))

        # ---- persistent weights, split in k-groups of 4 for early start ----
        NSP = 4
        KG = NIF // NSP
        aA_sb, aB_sb, wq_sb, wkv_sb = [], [], [], []
        xq0 = persist.tile([128, NIF, 512], BF16, name="xq0")
        for g in range(NSP):
            ksl = slice(g * KG * 128, (g + 1) * KG * 128)
            tl = persist.tile([128, KG, 128], BF16, name="aA%d" % g)
            nc.scalar.dma_start(
                out=tl, in_=aA[ksl].rearrange("(n p) f -> p n f", p=128))
            aA_sb.append(tl)
            tl = persist.tile([128, KG, 88], BF16, name="aB%d" % g)
            nc.scalar.dma_start(
                out=tl, in_=aB[ksl].rearrange("(n p) f -> p n f", p=128))
            aB_sb.append(tl)
            # first token block's x rides ahead of the q weights
            nc.sync.dma_start(
                out=xq0[:, g * KG:(g + 1) * KG, :],
                in_=xT[ksl].rearrange("(n p) t -> p n t", p=128)[:, :, 0:512])
            tl = persist.tile([128, KG, QF], BF16, name="wq%d" % g)
            nc.sync.dma_start(
                out=tl, in_=wqT[ksl].rearrange("(n p) f -> p n f", p=128))
            wq_sb.append(tl)
            tl = persist.tile([128, KG, 2 * KF], BF16, name="wkv%d" % g)
            nc.gpsimd.dma_start(
                out=tl, in_=wkvT[ksl].rearrange("(n p) f -> p n f", p=128))
            wkv_sb.append(tl)

        def A_AT(k):  # aA chain lhsT for contraction tile k
            return aA_sb[k // KG][:, k % KG, :]

        def A_BT(k):
            return aB_sb[k // KG][:, k % KG, :]

        def W_Q(k):
            return wq_sb[k // KG][:, k % KG, :]

        def W_KV(k):
            return wkv_sb[k // KG][:, k % KG, :]

        cst_sb = persist.tile([24, CST_W], BF16)
        nc.gpsimd.dma_start(out=cst_sb, in_=cst[:])
        E_A = cst_sb[:, 0:128]
        E_v = cst_sb[0:24, 128:192]
        ones24 = cst_sb[:, 192:216]
        ones8 = cst_sb[0:8, 192:200]
        E8o = cst_sb[0:8, 216:280]
        ones64 = cst_sb[0:1, 280:344]

        bq_sb = persist.tile([64, QF], BF16)
        nc.gpsimd.dma_start(out=bq_sb, in_=bq[:])
        bk_sb = persist.tile([128, KF], BF16)   # bk lives at partitions 64:128
        nc.gpsimd.dma_start(out=bk_sb[64:128, :], in_=bk[:])
        bv_sb = persist.tile([64, KF], BF16)
        nc.gpsimd.dma_start(out=bv_sb, in_=bv[:])
        bo_sb = persist.tile([64, D], BF16)
        nc.gpsimd.dma_start(out=bo_sb, in_=bo[:])
        ao_sb = persist.tile([128, NIF, 72], BF16)
        nc.scalar.dma_start(out=ao_sb,
                            in_=ao.rearrange("(n p) f -> p n f", p=128))
        cs_sb = persist.tile([128, S], BF16)
        nc.scalar.dma_start(out=cs_sb, in_=cs4[:])
        sn_sb = persist.tile([128, S], BF16)
        nc.scalar.dma_start(out=sn_sb, in_=sn4[:])

        ident_b = persist.tile([128, 128], BF16)
        make_identity(nc, ident_b)

        # attention operands (persist across phases)
        qh_sb = persist.tile([128, 2, S], BF16)   # [2 heads x 64, page, S]
        kh_sb = persist.tile([128, S], BF16)      # kv head duplicated 2x
        vtok = persist.tile([128, NKT, 65], BF16)  # token-major v + ones col
        nc.vector.memset(vtok[:, :, 64:65], 1.0)
        g_sb = persist.tile([128, NIF, TSH], BF16)  # gathered out (post-A2A)

        # ================= Phase A: QKV + LoRA + RoPE =================
        vT_all = persist.tile([64, S], BF16)      # v (feat-major) staging
        with tc.tile_pool(name="pA", bufs=1) as pA:
            for tb in range(4):
                tsl = slice(tb * 512, (tb + 1) * 512)
                if tb == 0:
                    xq = xq0
                else:
                    xq = pA.tile([128, NIF, 512], BF16, name="xq", tag="xq",
                                 bufs=3)
                    nc.sync.dma_start(
                        out=xq,
                        in_=xT.rearrange("(n p) t -> p n t", p=128)[:, :, tsl])

                # ---- main projection chains ----
                hA = ps.tile([128, 512], F32, name="hA", tag="p_hA")
                hB = ps.tile([88, 512], F32, name="hB", tag="p_hB")
                q0 = ps.tile([128, 512], F32, name="q0", tag="p_q0")
                q1 = ps.tile([128, 512], F32, name="q1", tag="p_q1")
                kv = ps.tile([128, 512], F32, name="kv", tag="p_kv")
                for k in range(NIF):
                    st = k == 0
                    sp = k == NIF - 1
                    rhs = xq[:, k, :]
                    nc.tensor.matmul(hA, A_AT(k), rhs, start=st, stop=sp)
                    nc.tensor.matmul(hB, A_BT(k), rhs, start=st, stop=sp)
                for k in range(NIF):
                    rhs = xq[:, k, :]
                    st = k == 0
                    nc.tensor.matmul(q0, W_Q(k)[:, 0:128], rhs,
                                     start=st, stop=False)
                    nc.tensor.matmul(q1, W_Q(k)[:, 128:256], rhs,
                                     start=st, stop=False)
                    nc.tensor.matmul(kv, W_KV(k), rhs, start=st, stop=False)

                # ---- router softmax (q,k,v fused; no transposes) ----
                ex3 = pA.tile([24, 512], BF16, name="ex3", tag="ex3", bufs=2)
                nc.scalar.activation(ex3, hB[64:88, :], AF.Exp)
                s3 = ps.tile([24, 512], F32, name="s3", tag="p_s3")
                nc.tensor.matmul(s3, ones24, ex3, start=True, stop=True)
                s3s = pA.tile([24, 512], F32, name="s3s", tag="s3s", bufs=2)
                nc.vector.tensor_copy(s3s, s3)
                rec3 = pA.tile([24, 512], F32, name="rec3", tag="rec3",
                               bufs=2)
                nc.vector.reciprocal_approx_fast(out=rec3, in_=s3s)
                rw3 = pA.tile([24, 512], BF16, name="rw3", tag="rw3", bufs=2)
                nc.vector.tensor_tensor(rw3, ex3, rec3, AluOpType.mult)
                rwbA = ps.tile([128, 512], F32, name="rwbA", tag="p_rwA")
                nc.tensor.matmul(rwbA, E_A, rw3, start=True, stop=True)
                rwbV = ps.tile([64, 512], F32, name="rwbV", tag="p_rwV")
                nc.tensor.matmul(rwbV, E_v, rw3, start=True, stop=True)
                rwbA_s = pA.tile([128, 512], BF16, name="rwbA_s",
                                 tag="rwbA_s", bufs=2)
                nc.scalar.activation(rwbA_s, rwbA, AF.Copy)
                rwbV_s = pA.tile([64, 512], BF16, name="rwbV_s",
                                 tag="rwbV_s", bufs=2)
                nc.scalar.activation(rwbV_s, rwbV, AF.Copy)
                hpA = pA.tile([128, 512], BF16, name="hpA", tag="hpA", bufs=2)
                nc.vector.tensor_tensor(hpA, hA, rwbA_s, AluOpType.mult)
                hpV = pA.tile([64, 512], BF16, name="hpV", tag="hpV", bufs=2)
                nc.vector.tensor_tensor(hpV, hB[0:64, :], rwbV_s,
                                        AluOpType.mult)

                # ---- LoRA-B closes the accumulations ----
                nc.tensor.matmul(q0, bq_sb[:, 0:128], hpA[0:64, :],
                                 start=False, stop=True)
                nc.tensor.matmul(q1, bq_sb[:, 128:256], hpA[0:64, :],
                                 start=False, stop=True)
                nc.tensor.matmul(kv[0:64, :], bk_sb[64:128, :],
                                 hpA[64:128, :], start=False, stop=True,
                                 tile_position=(64, 0))
                nc.tensor.matmul(kv[64:128, :], bv_sb, hpV,
                                 start=False, stop=True,
                                 tile_position=(0, 64))

                # ---- PSUM extraction (scalar engine) ----
                qe = pA.tile([128, 512], BF16, name="qe", tag="qe", bufs=2)
                qo = pA.tile([128, 512], BF16, name="qo", tag="qo", bufs=2)
                nc.scalar.activation(qe[0:64, :], q0[0:64, :], AF.Copy)
                nc.scalar.activation(qe[64:128, :], q1[0:64, :], AF.Copy)
                nc.scalar.activation(qo[0:64, :], q0[64:128, :], AF.Copy)
                nc.scalar.activation(qo[64:128, :], q1[64:128, :], AF.Copy)
                ke = pA.tile([32, 512], BF16, name="ke", tag="ke", bufs=2)
                ko = pA.tile([32, 512], BF16, name="ko", tag="ko", bufs=2)
                nc.scalar.activation(ke, kv[0:32, :], AF.Copy)
                nc.scalar.activation(ko, kv[32:64, :], AF.Copy)
                nc.scalar.activation(vT_all[:, tsl], kv[64:128, :], AF.Copy)

                # ---- RoPE (bf16, vector engine) ----
                cs_t = cs_sb[:, tsl]
                sn_t = sn_sb[:, tsl]
                t1 = pA.tile([128, 512], BF16, name="t1", tag="t1", bufs=2)
                t2 = pA.tile([128, 512], BF16, name="t2", tag="t2", bufs=2)
                rote = pA.tile([128, 512], BF16, name="rote", tag="rote",
                               bufs=2)
                roto = pA.tile([128, 512], BF16, name="roto", tag="roto",
                               bufs=2)
                nc.vector.tensor_tensor(t1, qe, cs_t, AluOpType.mult)
                nc.vector.tensor_tensor(t2, qo, sn_t, AluOpType.mult)
                nc.vector.tensor_tensor(rote, t1, t2, AluOpType.subtract)
                nc.vector.tensor_tensor(t1, qe, sn_t, AluOpType.mult)
                nc.vector.tensor_tensor(t2, qo, cs_t, AluOpType.mult)
                nc.vector.tensor_tensor(roto, t1, t2, AluOpType.add)
                k1 = pA.tile([32, 512], BF16, name="k1", tag="k1", bufs=2)
                k2 = pA.tile([32, 512], BF16, name="k2", tag="k2", bufs=2)
                csk = cs_sb[0:32, tsl]
                snk = sn_sb[0:32, tsl]
                nc.vector.tensor_tensor(k1, ke, csk, AluOpType.mult)
                nc.vector.tensor_tensor(k2, ko, snk, AluOpType.mult)
                nc.vector.tensor_tensor(kh_sb[0:32, tsl], k1, k2,
                                        AluOpType.subtract)
                nc.vector.tensor_tensor(k1, ke, snk, AluOpType.mult)
                nc.vector.tensor_tensor(k2, ko, csk, AluOpType.mult)
                nc.vector.tensor_tensor(kh_sb[32:64, tsl], k1, k2,
                                        AluOpType.add)

                # head rearrange via SBUF->SBUF DMA (off the engines)
                nc.gpsimd.dma_start(out=kh_sb[64:128, tsl],
                                    in_=kh_sb[0:64, tsl])
                for h in range(QH):
                    page, i = h // 2, h % 2
                    nc.gpsimd.dma_start(
                        out=qh_sb[64 * i:64 * i + 32, page, tsl],
                        in_=rote[32 * h:32 * h + 32, :])
                    nc.gpsimd.dma_start(
                        out=qh_sb[64 * i + 32:64 * i + 64, page, tsl],
                        in_=roto[32 * h:32 * h + 32, :])

            # token-major v, built after the per-block pipeline drains
            for kt in range(NKT):
                v_ps = ps.tile([128, 64], BF16, name="v_ps", tag="p_s3")
                nc.tensor.transpose(v_ps,
                                    vT_all[:, 128 * kt:128 * kt + 128],
                                    ident_b[0:64, 0:64])
                nc.vector.tensor_copy(vtok[:, kt, 0:64], v_ps)

        # prefetch the full output-projection weight during attention
        wo_ctx = tc.tile_pool(name="wo_pool", bufs=4)
        wo_pool = wo_ctx.__enter__()
        wo_tiles = []
        for ob in range(4):
            osl = slice(ob * 512, (ob + 1) * 512)
            wo_sb = wo_pool.tile([128, NIF, 512], BF16, name="wo_sb",
                                 tag="wo", bufs=4)
            nc.gpsimd.dma_start(
                out=wo_sb,
                in_=woT.rearrange("(n p) f -> p n f", p=128)[:, :, osl])
            wo_tiles.append(wo_sb)

        # ============ Phase C: attention + chunked A2A ============
        SC_TAGS = ["p_q0", "p_q1", "p_hA", "p_hB"]
        OUT_TAGS = ["p_kv", "p_s3", "p_rwA", "p_rwV"]
        with tc.tile_pool(name="pC", bufs=1) as pC:
            def emit_tail(qb, outps, gather=True):
                """Normalize + ship chunk qb. Emitted after the next query
                block's first score/exp round so the reciprocal latency
                hides under attention compute."""
                for h in range(QH):
                    dens = pC.tile([1, 512], F32, name="dens%d" % h,
                                   tag="dens%d" % h, bufs=2)
                    nc.vector.tensor_copy(dens, outps[h][64:65, :])
                    recf = pC.tile([1, 512], F32, name="recf%d" % h,
                                   tag="recf%d" % h, bufs=2)
                    nc.vector.reciprocal_approx_fast(out=recf, in_=dens)
                    rec = pC.tile([1, 512], BF16, name="rec%d" % h,
                                  tag="rec%d" % h, bufs=2)
                    nc.vector.tensor_copy(rec, recf)
                    rb = ps.tile([64, 512], F32, name="rb", tag=SC_TAGS[h])
                    nc.tensor.matmul(rb, ones64, rec, start=True, stop=True)
                    rb_s = pC.tile([64, 512], BF16, name="rb_s%d" % h,
                                   tag="rb_s%d" % h, bufs=2)
                    nc.vector.tensor_copy(rb_s, rb)
                    o65 = pC.tile([64, 512], BF16, name="o65%d" % h,
                                  tag="o65%d" % h, bufs=2)
                    nc.vector.tensor_tensor(o65, outps[h][0:64, :], rb_s,
                                            AluOpType.mult)
                    # [64, 512] -> cc_in[qb][dest, 64h:64h+64, 0:64]
                    nc.gpsimd.dma_start(
                        out=cc_in[qb][:, 64 * h:64 * h + 64, :]
                            .rearrange("d p t -> p d t"),
                        in_=o65)
                nc.gpsimd.collective_compute(
                    "AllToAll",
                    AluOpType.bypass,
                    ins=[cc_in[qb][:]],
                    outs=[cc_out[qb][:]],
                    replica_groups=[list(range(NCORES))],
                )
                if gather:
                    # gather this chunk into g_sb[:, :, 64qb:64qb+64]
                    nc.sync.dma_start(
                        out=g_sb[:, :, 64 * qb:64 * qb + 64],
                        in_=cc_out[qb].rearrange("s (k p) t -> p (s k) t",
                                                 k=2, p=128))

            pending = None
            yp_tt0 = {}
            for qb in range(NQB):
                qsl = slice(qb * 512, (qb + 1) * 512)
                active = [kt for kt in range(NKT)
                          if mask_cls[kt, qb] != M_SKIP]
                assert active, f"fully masked query block qb={qb}"
                outps = [ps.tile([65, 512], F32, name="outp%d" % h,
                                 tag=OUT_TAGS[h]) for h in range(QH)]
                for idx, kt in enumerate(active):
                    c = mask_cls[kt, qb]
                    lo = int(mask_lo[kt, qb]) if c == M_ADD else 0
                    mt = None
                    if c == M_ADD:
                        mt = pC.tile([128, 128], BF16, name="mt",
                                     tag="mt", bufs=4)
                        nc.sync.dma_start(
                            out=mt,
                            in_=mask01[128 * kt:128 * kt + 128,
                                       512 * qb + lo:512 * qb + lo + 128])
                    ksl = slice(128 * kt, 128 * kt + 128)
                    qslc = slice(512 * qb + lo, 512 * (qb + 1))
                    prs = []
                    for h in range(QH):
                        page, i = h // 2, h % 2
                        sc = ps.tile([128, 512], F32, name="sc%d" % h,
                                     tag=SC_TAGS[h])
                        nc.tensor.matmul(sc[:, lo:512],
                                         kh_sb[64 * i:64 * i + 64, ksl],
                                         qh_sb[64 * i:64 * i + 64, page,
                                               qslc],
                                         start=True, stop=True,
                                         tile_position=(64 * i, 0))
                        pr = pC.tile([128, 512], BF16, name="pr%d" % h,
                                     tag="pr%d" % h, bufs=2)
                        nc.scalar.activation(pr[:, lo:512], sc[:, lo:512],
                                             AF.Exp)
                        if mt is not None:
                            nc.vector.tensor_tensor(pr[:, lo:lo + 128],
                                                    pr[:, lo:lo + 128], mt,
                                                    AluOpType.mult)
                        prs.append(pr)
                    if idx == 0 and pending is not None:
                        # previous block's normalization rides behind this
                        # round's score matmuls
                        emit_tail(qb - 1, pending)
                        pending = None
                    for h in range(QH):
                        nc.tensor.matmul(outps[h][:, lo:512],
                                         vtok[:, kt, :],
                                         prs[h][:, lo:512],
                                         start=(kt == active[0]),
                                         stop=(kt == active[-1]))
                pending = outps
            emit_tail(NQB - 1, pending, gather=False)
            # o-proj token-half 0 (chunks 0/1 landed long ago) rides the
            # tensor engine under the final AllToAll
            YP0_TAGS = ["p_q0", "p_q1", "p_hA", "p_kv"]
            for ob in range(4):
                osl = slice(ob * 512, (ob + 1) * 512)
                yp = ps.tile([128, 512], F32, name="yp0_%d" % ob,
                             tag=YP0_TAGS[ob])
                for k in range(NIF):
                    nc.tensor.matmul(yp, g_sb[:, k, 0:128],
                                     wo_tiles[ob][:, k, :],
                                     start=(k == 0), stop=False)
                yp_tt0[ob] = yp

        # ================= Phase D: o-proj =================
        with tc.tile_pool(name="pD", bufs=1) as pD:
            def router_o(half, hsl):
                """o-LoRA router for one 128-token half; returns hpo half."""
                ho = ps.tile([72, 128], F32, name="ho%d" % half, tag="p_hB")
                for k in range(NIF):
                    nc.tensor.matmul(ho, ao_sb[:, k, :], g_sb[:, k, hsl],
                                     start=(k == 0), stop=(k == NIF - 1))
                exo = pD.tile([8, 128], BF16, name="exo%d" % half)
                nc.scalar.activation(exo, ho[64:72, :], AF.Exp)
                so = ps.tile([8, 128], F32, name="so%d" % half, tag="p_s3")
                nc.tensor.matmul(so, ones8, exo, start=True, stop=True)
                sos = pD.tile([8, 128], F32, name="sos%d" % half)
                nc.vector.tensor_copy(sos, so)
                reco = pD.tile([8, 128], F32, name="reco%d" % half)
                nc.vector.reciprocal_approx_fast(out=reco, in_=sos)
                rwo = pD.tile([8, 128], BF16, name="rwo%d" % half)
                nc.vector.tensor_tensor(rwo, exo, reco, AluOpType.mult)
                rwbo = ps.tile([64, 128], F32, name="rwbo%d" % half,
                               tag="p_rwA")
                nc.tensor.matmul(rwbo, E8o, rwo, start=True, stop=True)
                rwbo_s = pD.tile([64, 128], BF16, name="rwbo_s%d" % half)
                nc.vector.tensor_copy(rwbo_s, rwbo)
                hpo = pD.tile([64, 128], BF16, name="hpo%d" % half)
                nc.vector.tensor_tensor(hpo, ho[0:64, :], rwbo_s,
                                        AluOpType.mult)
                return hpo

            # token half 0: LoRA + bo closure + store, all before the final
            # gather (g_sb cols 0:128 come from chunks 0/1)
            hpo_a = router_o(0, slice(0, 128))
            for ob in range(4):
                osl = slice(ob * 512, (ob + 1) * 512)
                yp = yp_tt0[ob]
                nc.tensor.matmul(yp, hpo_a, bo_sb[:, osl],
                                 start=False, stop=True)
                yt = pD.tile([128, 512], F32, name="yt", tag="yt", bufs=3)
                if ob % 2 == 0:
                    nc.scalar.activation(yt, yp, AF.Copy)
                else:
                    nc.vector.tensor_copy(yt, yp)
                nc.sync.dma_start(out=y[0:128, osl], in_=yt)

            # final chunk's gather, then token half 1
            nc.sync.dma_start(
                out=g_sb[:, :, 64 * (NQB - 1):64 * NQB],
                in_=cc_out[NQB - 1].rearrange("s (k p) t -> p (s k) t",
                                              k=2, p=128))
            hpo_b = router_o(1, slice(128, 256))
            for ob in range(4):
                osl = slice(ob * 512, (ob + 1) * 512)
                yp = ps.tile([128, 512], F32, name="yp1_%d" % ob,
                             tag=["p_q0", "p_q1", "p_hA", "p_kv"][ob])
                for k in range(NIF):
                    nc.tensor.matmul(yp, g_sb[:, k, 128:256],
                                     wo_tiles[ob][:, k, :],
                                     start=(k == 0), stop=False)
                nc.tensor.matmul(yp, hpo_b, bo_sb[:, osl],
                                 start=False, stop=True)
                yt = pD.tile([128, 512], F32, name="yt", tag="yt", bufs=3)
                if ob % 2 == 0:
                    nc.scalar.activation(yt, yp, AF.Copy)
                else:
                    nc.vector.tensor_copy(yt, yp)
                nc.sync.dma_start(out=y[128:256, osl], in_=yt)
        wo_ctx.__exit__(None, None, None)


# ======================= host side =======================

_CACHE = {}


def _prep_inputs(x, mask, freqs_cos, freqs_sin, wq, wk, wv, wo,
                 lq_router, lq_A, lq_B, lk_router, lk_A, lk_B,
                 lv_router, lv_A, lv_B, lo_router, lo_A, lo_B):
    scale = 1.0 / np.sqrt(HD)
    x = _f32(np.asarray(x)).reshape(S, D)
    maskf = _f32(np.asarray(mask)).reshape(S, S)
    maskT = np.maximum(maskf, MASK_NEG).T.copy()
    mask_cls, mask_lo = classify_mask(maskT)
    mask01 = _bf((maskT > MASK_NEG * 0.5).astype(np.float32))

    xT = _bf(x.T)
    cs4 = _bf(np.tile(_f32(freqs_cos).T, (4, 1)))      # [128, S]
    sn4 = _bf(np.tile(_f32(freqs_sin).T, (4, 1)))
    woT = _bf(_f32(wo).T)
    ao_p = _bf(np.concatenate([_a_pack(_f32(lo_A)), _f32(lo_router).T],
                              axis=1))                 # [D, 72]
    bo_f = _bf(_b_flat(_f32(lo_B), SCALING))

    # fused LoRA-A stationaries: [D, 128] = [aq|ak], [D, 88] = [av|rq|rk|rv]
    aA_p = _bf(np.concatenate(
        [_a_pack(_f32(lq_A)), _a_pack(_f32(lk_A))], axis=1))
    aB_p = _bf(np.concatenate(
        [_a_pack(_f32(lv_A)), _f32(lq_router).T, _f32(lk_router).T,
         _f32(lv_router).T], axis=1))

    shared = dict(xT=xT, cs4=cs4, sn4=sn4, woT=woT, mask01=mask01,
                  ao=ao_p, bo=bo_f, cst=_build_cst(), aA=aA_p, aB=aB_p)

    wqf, wkf, wvf = _f32(wq), _f32(wk), _f32(wv)
    lqB, lkB, lvB = _f32(lq_B), _f32(lk_B), _f32(lv_B)

    in_maps = []
    for c in range(NCORES):
        wq_c = wqf[c * QF:(c + 1) * QF][IDX_Q] * scale
        wk_c = wkf[c * KF:(c + 1) * KF][IDX_K]
        wv_c = wvf[c * KF:(c + 1) * KF]
        bq_c = _b_flat(lqB[:, c * QF:(c + 1) * QF, :][:, IDX_Q, :],
                       SCALING * scale)
        bk_c = _b_flat(lkB[:, c * KF:(c + 1) * KF, :][:, IDX_K, :], SCALING)
        bv_c = _b_flat(lvB[:, c * KF:(c + 1) * KF, :], SCALING)
        m = dict(shared)
        m.update(wqT=_bf(wq_c.T),
                 wkvT=_bf(np.concatenate([wk_c.T, wv_c.T], axis=1)),
                 bq=_bf(bq_c), bk=_bf(bk_c), bv=_bf(bv_c))
        in_maps.append(m)
    return in_maps, mask_cls, mask_lo


def get_graph(mask_cls, mask_lo):
    key = mask_cls.tobytes() + mask_lo.tobytes()
    if key not in _CACHE:
        _CACHE[key] = build(mask_cls, mask_lo)
    return _CACHE[key]


def kernel(x, start_pos, mask, freqs_cos, freqs_sin, wq, wk, wv, wo,
           lq_router, lq_A, lq_B, lk_router, lk_A, lk_B,
           lv_router, lv_A, lv_B, lo_router, lo_A, lo_B,
           _trace=False):
    from concourse.bass_utils import run_bass_kernel_spmd
    in_maps, mask_cls, mask_lo = _prep_inputs(
        x, mask, freqs_cos, freqs_sin, wq, wk, wv, wo,
        lq_router, lq_A, lq_B, lk_router, lk_A, lk_B,
        lv_router, lv_A, lv_B, lo_router, lo_A, lo_B)
    nc = get_graph(mask_cls, mask_lo)
    res = run_bass_kernel_spmd(nc, in_maps, list(range(NCORES)), trace=_trace)
    # core c's y rows: group g (0..3) covers tokens [512g + 64c, 512g + 64c + 64)
    ys = np.stack([res.results[c]["y"] for c in range(NCORES)], axis=0)
    ys = ys.reshape(NCORES, 4, 64, D).transpose(1, 0, 2, 3).reshape(S, D)
    out = ys.reshape(B, S, H * HD).astype(np.float32)
    if _trace:
        return out, res
    return out


# revision 21
# speedup vs baseline: 1.4550x; 1.0251x over previous
"""Trainium2 Bass kernel for MoE-LoRA GQA attention (nn_Attention_57389353009692).

V2 strategy (8 NeuronCores, one SPMD launch):
  - Tensor-parallel over heads: core c owns q-heads 4c..4c+3 and kv-head c.
  - Phase A (per 512-token block): QKV projections (+ MoE-LoRA) with packed
    matmul chains (wk|wv fused; LoRA-A for q/k/v + all three routers fused
    into two chains of 128/88 rows), router softmax done with
    exp -> ones-matmul row-sum -> reciprocal -> broadcast-matmul (no
    transposes, no DRAM bounce), RoPE in bf16 on 128 partitions.
  - Phase C: flash-style attention per 512-query block; causal mask applied
    as a 0/1 multiply after exp (bf16); output normalized PRE-collective via
    reciprocal-of-denominator broadcast matmuls fused into the PSUM->SBUF
    cast.
  - AllToAll is chunked per query block (4 collectives) and overlaps the
    remaining attention compute. Output tokens are interleaved at
    64-granularity: core c owns tokens {t : (t//64) % 8 == c} so every chunk
    is a uniform 8-way exchange.
  - Phase D: o-projection + o-LoRA for the core's 256 tokens with the full
    (prefetched) wo.

Numerics: bf16 operands, fp32 PSUM accumulation, fp32->exp softmax without
max subtraction (scores are O(1) here; masked entries are zeroed exactly by
the 0/1 multiply). Scale 1/sqrt(64) folded into wq and q-LoRA-B on host.
RoPE trick: interleaved even/odd pairs are made contiguous by permuting
wq/wk output features on host (per 2-head "page": [h0e|h1e|h0o|h1o]).
"""

import sys

for _p in ("/opt/trn_rl_repo", "/root/.axon_site/_ro/trn_rl_repo"):
    if _p not in sys.path:
        sys.path.insert(0, _p)

import numpy as np
import ml_dtypes

import concourse.bass as bass
import concourse.tile as tile
from concourse import bacc, mybir
from concourse.masks import make_identity
from concourse.alu_op_type import AluOpType

F32 = mybir.dt.float32
BF16 = mybir.dt.bfloat16
AF = mybir.ActivationFunctionType
AX = mybir.AxisListType
BF16NP = ml_dtypes.bfloat16

B, S, D = 1, 2048, 2048
H, KVH, HD = 32, 8, 64
NREP = H // KVH
R, E = 8, 8
SCALING = 32.0 / 8.0
NCORES = 8
QH = H // NCORES          # 4 q heads per core
QF = QH * HD              # 256 q feats per core
KF = HD                   # 64 kv feats per core
TSH = S // NCORES         # 256 tokens per core for o-proj
NKT = S // 128            # 16 key tiles
NQB = S // 512            # 4 query blocks
NIF = D // 128            # 16 contraction tiles

MASK_NEG = -1e30

# mask tile classes
M_SKIP, M_ZERO, M_ADD = 0, 1, 2


def _build_perm():
    """Per-core feature permutations for rope-friendly layout."""
    idx_q = np.zeros(QF, dtype=np.int64)
    for f in range(QF):
        page, w = divmod(f, 128)
        if w < 32:
            hl, j, odd = 2 * page, w, 0
        elif w < 64:
            hl, j, odd = 2 * page + 1, w - 32, 0
        elif w < 96:
            hl, j, odd = 2 * page, w - 64, 1
        else:
            hl, j, odd = 2 * page + 1, w - 96, 1
        idx_q[f] = 64 * hl + 2 * j + odd
    idx_k = np.zeros(KF, dtype=np.int64)
    for w in range(KF):
        if w < 32:
            idx_k[w] = 2 * w
        else:
            idx_k[w] = 2 * (w - 32) + 1
    return idx_q, idx_k


IDX_Q, IDX_K = _build_perm()


def _a_pack(A):
    """[E,R,D] -> [D, 64] with col r*8+e."""
    return np.transpose(A, (1, 0, 2)).reshape(E * R, -1).T


def _b_flat(Bw, scale):
    """[E, OF, R] -> [64, OF] with row r*8+e."""
    return (np.transpose(Bw, (2, 0, 1)).reshape(E * R, -1) * scale)


def _bf(x):
    return np.ascontiguousarray(x, dtype=np.float32).astype(BF16NP)


def _f32(x):
    return np.ascontiguousarray(x, dtype=np.float32)


def classify_mask(maskT):
    """maskT: [S(k), S(q)] clamped fp32. Returns ([NKT, NQB] class map,
    [NKT, NQB] live-start-column map for M_ADD tiles).

    For an M_ADD tile, lo is the first live column, rounded down to 128;
    columns >= lo+128 must be fully live (causal staircase) -- the kernel
    then computes only [lo, 512) and masks just [lo, lo+128)."""
    cls = np.zeros((NKT, NQB), dtype=np.int64)
    los = np.zeros((NKT, NQB), dtype=np.int64)
    for kt in range(NKT):
        blk_rows = maskT[kt * 128:(kt + 1) * 128]
        for qb in range(NQB):
            blk = blk_rows[:, qb * 512:(qb + 1) * 512]
            if np.all(blk <= MASK_NEG * 0.5):
                cls[kt, qb] = M_SKIP
            elif np.all(blk == 0.0):
                cls[kt, qb] = M_ZERO
            else:
                cls[kt, qb] = M_ADD
                live = np.where((blk == 0.0).any(axis=0))[0]
                lo = (int(live[0]) // 128) * 128 if len(live) else 0
                if lo + 128 <= 512 and not np.all(blk[:, lo + 128:] == 0.0):
                    lo = 0  # not a causal staircase; keep full width
                los[kt, qb] = lo
    return cls, los


# constants tensor layout (bf16, [24, 344]):
#  [:, 0:128]   E_A: row e, col j -> 1 if (j<64 and e==j%8) or (j>=64 and e-8==j%8)
#  [:, 128:192] E_v: row e, col j -> 1 if e-16 == j%8
#  [:, 192:216] ones24: block-diag 3x(8x8 ones)
#  [0:8, 216:280] E8o: row e, col j -> 1 if e == j%8
#  [0:1, 280:344] ones64 row
CST_W = 344


def _build_cst():
    cst = np.zeros((24, CST_W), dtype=np.float32)
    for j in range(64):
        cst[j % 8, j] = 1.0           # E_A q half
        cst[8 + j % 8, 64 + j] = 1.0  # E_A k half
        cst[16 + j % 8, 128 + j] = 1.0  # E_v
        cst[j % 8, 216 + j] = 1.0     # E8o
        cst[0, 280 + j] = 1.0         # ones64
    for b in range(3):
        cst[8 * b:8 * b + 8, 192 + 8 * b:200 + 8 * b] = 1.0  # ones24
    return _bf(cst)


def build(mask_cls, mask_lo):
    """Build the SPMD Bass graph. mask_cls: [NKT, NQB] int array."""
    nc = bacc.Bacc(None, target_bir_lowering=False)

    # ---- DRAM I/O (per-core shards prepared on host) ----
    xT = nc.declare_dram_parameter("xT", [D, S], BF16, isOutput=False)
    wqT = nc.declare_dram_parameter("wqT", [D, QF], BF16, isOutput=False)
    wkvT = nc.declare_dram_parameter("wkvT", [D, 2 * KF], BF16, isOutput=False)
    aA = nc.declare_dram_parameter("aA", [D, 128], BF16, isOutput=False)
    aB = nc.declare_dram_parameter("aB", [D, 88], BF16, isOutput=False)
    ao = nc.declare_dram_parameter("ao", [D, 72], BF16, isOutput=False)
    bq = nc.declare_dram_parameter("bq", [E * R, QF], BF16, isOutput=False)
    bk = nc.declare_dram_parameter("bk", [E * R, KF], BF16, isOutput=False)
    bv = nc.declare_dram_parameter("bv", [E * R, KF], BF16, isOutput=False)
    bo = nc.declare_dram_parameter("bo", [E * R, D], BF16, isOutput=False)
    woT = nc.declare_dram_parameter("woT", [D, D], BF16, isOutput=False)
    cs4 = nc.declare_dram_parameter("cs4", [128, S], BF16, isOutput=False)
    sn4 = nc.declare_dram_parameter("sn4", [128, S], BF16, isOutput=False)
    mask01 = nc.declare_dram_parameter("mask01", [S, S], BF16, isOutput=False)
    cst = nc.declare_dram_parameter("cst", [24, CST_W], BF16, isOutput=False)
    y = nc.declare_dram_parameter("y", [TSH, D], F32, isOutput=True)

    # internal DRAM for the chunked collectives: [dest/src, 4h*64 feat, 64 tok]
    cc_in = [nc.dram_tensor("cc_in%d" % q, [NCORES, QF, 64], BF16)
             for q in range(NQB)]
    cc_out = [nc.dram_tensor("cc_out%d" % q, [NCORES, QF, 64], BF16)
              for q in range(NQB)]

    with tile.TileContext(nc) as tc:
        _emit(nc, tc, locals(), mask_cls, mask_lo)
    nc.finalize()
    return nc


def _emit(nc, tc, t, mask_cls, mask_lo):
    xT, wqT, wkvT = t["xT"], t["wqT"], t["wkvT"]
    aA, aB, ao = t["aA"], t["aB"], t["ao"]
    bq, bk, bv, bo = t["bq"], t["bk"], t["bv"], t["bo"]
    woT, cs4, sn4, mask01, y = t["woT"], t["cs4"], t["sn4"], t["mask01"], t["y"]
    cst = t["cst"]
    cc_in, cc_out = t["cc_in"], t["cc_out"]

    import contextlib
    ctx = contextlib.ExitStack()
    with ctx:
        persist = ctx.enter_context(tc.tile_pool(name="persist", bufs=1))
        ps = ctx.enter_context(tc.tile_pool(name="ps", bufs=1, space="PSUM"))

        # ---- persistent weights, split in k-groups of 4 for early start ----
        NSP = 4
        KG = NIF // NSP
        aA_sb, aB_sb, wq_sb, wkv_sb = [], [], [], []
        xq0 = persist.tile([128, NIF, 512], BF16, name="xq0")
        for g in range(NSP):
            ksl = slice(g * KG * 128, (g + 1) * KG * 128)
            tl = persist.tile([128, KG, 128], BF16, name="aA%d" % g)
            nc.scalar.dma_start(
                out=tl, in_=aA[ksl].rearrange("(n p) f -> p n f", p=128))
            aA_sb.append(tl)
            tl = persist.tile([128, KG, 88], BF16, name="aB%d" % g)
            nc.scalar.dma_start(
                out=tl, in_=aB[ksl].rearrange("(n p) f -> p n f", p=128))
            aB_sb.append(tl)
            # first token block's x rides ahead of the q weights
            nc.sync.dma_start(
                out=xq0[:, g * KG:(g + 1) * KG, :],
                in_=xT[ksl].rearrange("(n p) t -> p n t", p=128)[:, :, 0:512])
            tl = persist.tile([128, KG, QF], BF16, name="wq%d" % g)
            nc.sync.dma_start(
                out=tl, in_=wqT[ksl].rearrange("(n p) f -> p n f", p=128))
            wq_sb.append(tl)
            tl = persist.tile([128, KG, 2 * KF], BF16, name="wkv%d" % g)
            nc.gpsimd.dma_start(
                out=tl, in_=wkvT[ksl].rearrange("(n p) f -> p n f", p=128))
            wkv_sb.append(tl)

        def A_AT(k):  # aA chain lhsT for contraction tile k
            return aA_sb[k // KG][:, k % KG, :]

        def A_BT(k):
            return aB_sb[k // KG][:, k % KG, :]

        def W_Q(k):
            return wq_sb[k // KG][:, k % KG, :]

        def W_KV(k):
            return wkv_sb[k // KG][:, k % KG, :]

        cst_sb = persist.tile([24, CST_W], BF16)
        nc.gpsimd.dma_start(out=cst_sb, in_=cst[:])
        E_A = cst_sb[:, 0:128]
        E_v = cst_sb[0:24, 128:192]
        ones24 = cst_sb[:, 192:216]
        ones8 = cst_sb[0:8, 192:200]
        E8o = cst_sb[0:8, 216:280]
        ones64 = cst_sb[0:1, 280:344]

        bq_sb = persist.tile([64, QF], BF16)
        nc.gpsimd.dma_start(out=bq_sb, in_=bq[:])
        bk_sb = persist.tile([128, KF], BF16)   # bk lives at partitions 64:128
        nc.gpsimd.dma_start(out=bk_sb[64:128, :], in_=bk[:])
        bv_sb = persist.tile([64, KF], BF16)
        nc.gpsimd.dma_start(out=bv_sb, in_=bv[:])
        bo_sb = persist.tile([64, D], BF16)
        nc.gpsimd.dma_start(out=bo_sb, in_=bo[:])
        ao_sb = persist.tile([128, NIF, 72], BF16)
        nc.scalar.dma_start(out=ao_sb,
                            in_=ao.rearrange("(n p) f -> p n f", p=128))
        cs_sb = persist.tile([128, S], BF16)
        nc.scalar.dma_start(out=cs_sb, in_=cs4[:])
        sn_sb = persist.tile([128, S], BF16)
        nc.scalar.dma_start(out=sn_sb, in_=sn4[:])

        ident_b = persist.tile([128, 128], BF16)
        make_identity(nc, ident_b)

        # attention operands (persist across phases)
        qh_sb = persist.tile([128, 2, S], BF16)   # [2 heads x 64, page, S]
        kh_sb = persist.tile([128, S], BF16)      # kv head duplicated 2x
        vtok = persist.tile([128, NKT, 65], BF16)  # token-major v + ones col
        nc.vector.memset(vtok[:, :, 64:65], 1.0)
        g_sb = persist.tile([128, NIF, TSH], BF16)  # gathered out (post-A2A)

        # ================= Phase A: QKV + LoRA + RoPE =================
        vT_all = persist.tile([64, S], BF16)      # v (feat-major) staging
        with tc.tile_pool(name="pA", bufs=1) as pA:
            for tb in range(4):
                tsl = slice(tb * 512, (tb + 1) * 512)
                if tb == 0:
                    xq = xq0
                else:
                    xq = pA.tile([128, NIF, 512], BF16, name="xq", tag="xq",
                                 bufs=3)
                    nc.sync.dma_start(
                        out=xq,
                        in_=xT.rearrange("(n p) t -> p n t", p=128)[:, :, tsl])

                # ---- main projection chains ----
                hA = ps.tile([128, 512], F32, name="hA", tag="p_hA")
                hB = ps.tile([88, 512], F32, name="hB", tag="p_hB")
                q0 = ps.tile([128, 512], F32, name="q0", tag="p_q0")
                q1 = ps.tile([128, 512], F32, name="q1", tag="p_q1")
                kv = ps.tile([128, 512], F32, name="kv", tag="p_kv")
                for k in range(NIF):
                    st = k == 0
                    sp = k == NIF - 1
                    rhs = xq[:, k, :]
                    nc.tensor.matmul(hA, A_AT(k), rhs, start=st, stop=sp)
                    nc.tensor.matmul(hB, A_BT(k), rhs, start=st, stop=sp)
                for k in range(NIF):
                    rhs = xq[:, k, :]
                    st = k == 0
                    nc.tensor.matmul(q0, W_Q(k)[:, 0:128], rhs,
                                     start=st, stop=False)
                    nc.tensor.matmul(q1, W_Q(k)[:, 128:256], rhs,
                                     start=st, stop=False)
                    nc.tensor.matmul(kv, W_KV(k), rhs, start=st, stop=False)

                # ---- router softmax (q,k,v fused; no transposes) ----
                ex3 = pA.tile([24, 512], BF16, name="ex3", tag="ex3", bufs=2)
                nc.scalar.activation(ex3, hB[64:88, :], AF.Exp)
                s3 = ps.tile([24, 512], F32, name="s3", tag="p_s3")
                nc.tensor.matmul(s3, ones24, ex3, start=True, stop=True)
                s3s = pA.tile([24, 512], F32, name="s3s", tag="s3s", bufs=2)
                nc.vector.tensor_copy(s3s, s3)
                rec3 = pA.tile([24, 512], F32, name="rec3", tag="rec3",
                               bufs=2)
                nc.vector.reciprocal_approx_fast(out=rec3, in_=s3s)
                rw3 = pA.tile([24, 512], BF16, name="rw3", tag="rw3", bufs=2)
                nc.vector.tensor_tensor(rw3, ex3, rec3, AluOpType.mult)
                rwbA = ps.tile([128, 512], F32, name="rwbA", tag="p_rwA")
                nc.tensor.matmul(rwbA, E_A, rw3, start=True, stop=True)
                rwbV = ps.tile([64, 512], F32, name="rwbV", tag="p_rwV")
                nc.tensor.matmul(rwbV, E_v, rw3, start=True, stop=True)
                rwbA_s = pA.tile([128, 512], BF16, name="rwbA_s",
                                 tag="rwbA_s", bufs=2)
                nc.scalar.activation(rwbA_s, rwbA, AF.Copy)
                rwbV_s = pA.tile([64, 512], BF16, name="rwbV_s",
                                 tag="rwbV_s", bufs=2)
                nc.scalar.activation(rwbV_s, rwbV, AF.Copy)
                hpA = pA.tile([128, 512], BF16, name="hpA", tag="hpA", bufs=2)
                nc.vector.tensor_tensor(hpA, hA, rwbA_s, AluOpType.mult)
                hpV = pA.tile([64, 512], BF16, name="hpV", tag="hpV", bufs=2)
                nc.vector.tensor_tensor(hpV, hB[0:64, :], rwbV_s,
                                        AluOpType.mult)

                # ---- LoRA-B closes the accumulations ----
                nc.tensor.matmul(q0, bq_sb[:, 0:128], hpA[0:64, :],
                                 start=False, stop=True)
                nc.tensor.matmul(q1, bq_sb[:, 128:256], hpA[0:64, :],
                                 start=False, stop=True)
                nc.tensor.matmul(kv[0:64, :], bk_sb[64:128, :],
                                 hpA[64:128, :], start=False, stop=True,
                                 tile_position=(64, 0))
                nc.tensor.matmul(kv[64:128, :], bv_sb, hpV,
                                 start=False, stop=True,
                                 tile_position=(0, 64))

                # ---- PSUM extraction (scalar engine) ----
                qe = pA.tile([128, 512], BF16, name="qe", tag="qe", bufs=2)
                qo = pA.tile([128, 512], BF16, name="qo", tag="qo", bufs=2)
                nc.scalar.activation(qe[0:64, :], q0[0:64, :], AF.Copy)
                nc.scalar.activation(qe[64:128, :], q1[0:64, :], AF.Copy)
                nc.scalar.activation(qo[0:64, :], q0[64:128, :], AF.Copy)
                nc.scalar.activation(qo[64:128, :], q1[64:128, :], AF.Copy)
                ke = pA.tile([32, 512], BF16, name="ke", tag="ke", bufs=2)
                ko = pA.tile([32, 512], BF16, name="ko", tag="ko", bufs=2)
                nc.scalar.activation(ke, kv[0:32, :], AF.Copy)
                nc.scalar.activation(ko, kv[32:64, :], AF.Copy)
                nc.scalar.activation(vT_all[:, tsl], kv[64:128, :], AF.Copy)

                # ---- RoPE (bf16, vector engine) ----
                cs_t = cs_sb[:, tsl]
                sn_t = sn_sb[:, tsl]
                t1 = pA.tile([128, 512], BF16, name="t1", tag="t1", bufs=2)
                t2 = pA.tile([128, 512], BF16, name="t2", tag="t2", bufs=2)
                rote = pA.tile([128, 512], BF16, name="rote", tag="rote",
                               bufs=2)
                roto = pA.tile([128, 512], BF16, name="roto", tag="roto",
                               bufs=2)
                nc.vector.tensor_tensor(t1, qe, cs_t, AluOpType.mult)
                nc.vector.tensor_tensor(t2, qo, sn_t, AluOpType.mult)
                nc.vector.tensor_tensor(rote, t1, t2, AluOpType.subtract)
                nc.vector.tensor_tensor(t1, qe, sn_t, AluOpType.mult)
                nc.vector.tensor_tensor(t2, qo, cs_t, AluOpType.mult)
                nc.vector.tensor_tensor(roto, t1, t2, AluOpType.add)
                k1 = pA.tile([32, 512], BF16, name="k1", tag="k1", bufs=2)
                k2 = pA.tile([32, 512], BF16, name="k2", tag="k2", bufs=2)
                csk = cs_sb[0:32, tsl]
                snk = sn_sb[0:32, tsl]
                nc.vector.tensor_tensor(k1, ke, csk, AluOpType.mult)
                nc.vector.tensor_tensor(k2, ko, snk, AluOpType.mult)
                nc.vector.tensor_tensor(kh_sb[0:32, tsl], k1, k2,
                                        AluOpType.subtract)
                nc.vector.tensor_tensor(k1, ke, snk, AluOpType.mult)
                nc.vector.tensor_tensor(k2, ko, csk, AluOpType.mult)
                nc.vector.tensor_tensor(kh_sb[32:64, tsl], k1, k2,
                                        AluOpType.add)

                # head rearrange via SBUF->SBUF DMA (off the engines)
                nc.gpsimd.dma_start(out=kh_sb[64:128, tsl],
                                    in_=kh_sb[0:64, tsl])
                for h in range(QH):
                    page, i = h // 2, h % 2
                    nc.gpsimd.dma_start(
                        out=qh_sb[64 * i:64 * i + 32, page, tsl],
                        in_=rote[32 * h:32 * h + 32, :])
                    nc.gpsimd.dma_start(
                        out=qh_sb[64 * i + 32:64 * i + 64, page, tsl],
                        in_=roto[32 * h:32 * h + 32, :])

            # token-major v, built after the per-block pipeline drains
            for kt in range(NKT):
                v_ps = ps.tile([128, 64], BF16, name="v_ps", tag="p_s3")
                nc.tensor.transpose(v_ps,
                                    vT_all[:, 128 * kt:128 * kt + 128],
                                    ident_b[0:64, 0:64])
                nc.vector.tensor_copy(vtok[:, kt, 0:64], v_ps)

        # prefetch the full output-projection weight during attention
        wo_ctx = tc.tile_pool(name="wo_pool", bufs=4)
        wo_pool = wo_ctx.__enter__()
        wo_tiles = []
        for ob in range(4):
            osl = slice(ob * 512, (ob + 1) * 512)
            wo_sb = wo_pool.tile([128, NIF, 512], BF16, name="wo_sb",
                                 tag="wo", bufs=4)
            nc.gpsimd.dma_start(
                out=wo_sb,
                in_=woT.rearrange("(n p) f -> p n f", p=128)[:, :, osl])
            wo_tiles.append(wo_sb)

        # ============ Phase C: attention + chunked A2A ============
        SC_TAGS = ["p_q0", "p_q1", "p_hA", "p_hB"]
        OUT_TAGS = ["p_kv", "p_s3", "p_rwA", "p_rwV"]
        with tc.tile_pool(name="pC", bufs=1) as pC:
            def emit_tail(qb, outps, gather=True):
                """Normalize + ship chunk qb. Emitted after the next query
                block's first score/exp round so the reciprocal latency
                hides under attention compute."""
                for h in range(QH):
                    dens = pC.tile([1, 512], F32, name="dens%d" % h,
                                   tag="dens%d" % h, bufs=2)
                    nc.vector.tensor_copy(dens, outps[h][64:65, :])
                    recf = pC.tile([1, 512], F32, name="recf%d" % h,
                                   tag="recf%d" % h, bufs=2)
                    nc.vector.reciprocal_approx_fast(out=recf, in_=dens)
                    rec = pC.tile([1, 512], BF16, name="rec%d" % h,
                                  tag="rec%d" % h, bufs=2)
                    nc.vector.tensor_copy(rec, recf)
                    rb = ps.tile([64, 512], F32, name="rb", tag=SC_TAGS[h])
                    nc.tensor.matmul(rb, ones64, rec, start=True, stop=True)
                    rb_s = pC.tile([64, 512], BF16, name="rb_s%d" % h,
                                   tag="rb_s%d" % h, bufs=2)
                    nc.vector.tensor_copy(rb_s, rb)
                    o65 = pC.tile([64, 512], BF16, name="o65%d" % h,
                                  tag="o65%d" % h, bufs=2)
                    nc.vector.tensor_tensor(o65, outps[h][0:64, :], rb_s,
                                            AluOpType.mult)
                    # [64, 512] -> cc_in[qb][dest, 64h:64h+64, 0:64]
                    nc.gpsimd.dma_start(
                        out=cc_in[qb][:, 64 * h:64 * h + 64, :]
                            .rearrange("d p t -> p d t"),
                        in_=o65)
                nc.gpsimd.collective_compute(
                    "AllToAll",
                    AluOpType.bypass,
                    ins=[cc_in[qb][:]],
                    outs=[cc_out[qb][:]],
                    replica_groups=[list(range(NCORES))],
                )
                if gather:
                    # gather this chunk into g_sb[:, :, 64qb:64qb+64]
                    nc.sync.dma_start(
                        out=g_sb[:, :, 64 * qb:64 * qb + 64],
                        in_=cc_out[qb].rearrange("s (k p) t -> p (s k) t",
                                                 k=2, p=128))

            pending = None
            yp_tt0 = {}
            for qb in range(NQB):
                qsl = slice(qb * 512, (qb + 1) * 512)
                active = [kt for kt in range(NKT)
                          if mask_cls[kt, qb] != M_SKIP]
                assert active, f"fully masked query block qb={qb}"
                outps = [ps.tile([65, 512], F32, name="outp%d" % h,
                                 tag=OUT_TAGS[h]) for h in range(QH)]
                for idx, kt in enumerate(active):
                    c = mask_cls[kt, qb]
                    lo = int(mask_lo[kt, qb]) if c == M_ADD else 0
                    mt = None
                    if c == M_ADD:
                        mt = pC.tile([128, 128], BF16, name="mt",
                                     tag="mt", bufs=4)
                        nc.sync.dma_start(
                            out=mt,
                            in_=mask01[128 * kt:128 * kt + 128,
                                       512 * qb + lo:512 * qb + lo + 128])
                    ksl = slice(128 * kt, 128 * kt + 128)
                    qslc = slice(512 * qb + lo, 512 * (qb + 1))
                    prs = []
                    for h in range(QH):
                        page, i = h // 2, h % 2
                        sc = ps.tile([128, 512], F32, name="sc%d" % h,
                                     tag=SC_TAGS[h])
                        nc.tensor.matmul(sc[:, lo:512],
                                         kh_sb[64 * i:64 * i + 64, ksl],
                                         qh_sb[64 * i:64 * i + 64, page,
                                               qslc],
                                         start=True, stop=True,
                                         tile_position=(64 * i, 0))
                        pr = pC.tile([128, 512], BF16, name="pr%d" % h,
                                     tag="pr%d" % h, bufs=2)
                        nc.scalar.activation(pr[:, lo:512], sc[:, lo:512],
                                             AF.Exp)
                        if mt is not None:
                            nc.vector.tensor_tensor(pr[:, lo:lo + 128],
                                                    pr[:, lo:lo + 128], mt,
                                                    AluOpType.mult)
                        prs.append(pr)
                    if idx == 0 and pending is not None:
                        # previous block's normalization rides behind this
                        # round's score matmuls
                        emit_tail(qb - 1, pending)
                        pending = None
                    for h in range(QH):
                        nc.tensor.matmul(outps[h][:, lo:512],
                                         vtok[:, kt, :],
                                         prs[h][:, lo:512],
                                         start=(kt == active[0]),
                                         stop=(kt == active[-1]))
                pending = outps
            emit_tail(NQB - 1, pending, gather=False)
            # o-proj token-half 0 (chunks 0/1 landed long ago) rides the
            # tensor engine under the final AllToAll
            YP0_TAGS = ["p_q0", "p_q1", "p_hA", "p_kv"]
            for ob in range(4):
                osl = slice(ob * 512, (ob + 1) * 512)
                yp = ps.tile([128, 512], F32, name="yp0_%d" % ob,
                             tag=YP0_TAGS[ob])
                for k in range(NIF):
                    nc.tensor.matmul(yp, g_sb[:, k, 0:128],
                                     wo_tiles[ob][:, k, :],
                                     start=(k == 0), stop=False)
                yp_tt0[ob] = yp

        # ================= Phase D: o-proj =================
        with tc.tile_pool(name="pD", bufs=1) as pD:
            def router_o(half, hsl):
                """o-LoRA router for one 128-token half; returns hpo half."""
                ho = ps.tile([72, 128], F32, name="ho%d" % half, tag="p_hB")
                for k in range(NIF):
                    nc.tensor.matmul(ho, ao_sb[:, k, :], g_sb[:, k, hsl],
                                     start=(k == 0), stop=(k == NIF - 1))
                exo = pD.tile([8, 128], BF16, name="exo%d" % half)
                nc.scalar.activation(exo, ho[64:72, :], AF.Exp)
                so = ps.tile([8, 128], F32, name="so%d" % half, tag="p_s3")
                nc.tensor.matmul(so, ones8, exo, start=True, stop=True)
                sos = pD.tile([8, 128], F32, name="sos%d" % half)
                nc.vector.tensor_copy(sos, so)
                reco = pD.tile([8, 128], F32, name="reco%d" % half)
                nc.vector.reciprocal_approx_fast(out=reco, in_=sos)
                rwo = pD.tile([8, 128], BF16, name="rwo%d" % half)
                nc.vector.tensor_tensor(rwo, exo, reco, AluOpType.mult)
                rwbo = ps.tile([64, 128], F32, name="rwbo%d" % half,
                               tag="p_rwA")
                nc.tensor.matmul(rwbo, E8o, rwo, start=True, stop=True)
                rwbo_s = pD.tile([64, 128], BF16, name="rwbo_s%d" % half)
                nc.vector.tensor_copy(rwbo_s, rwbo)
                hpo = pD.tile([64, 128], BF16, name="hpo%d" % half)
                nc.vector.tensor_tensor(hpo, ho[0:64, :], rwbo_s,
                                        AluOpType.mult)
                return hpo

            # token half 0: LoRA + bo closure + store, all before the final
            # gather (g_sb cols 0:128 come from chunks 0/1)
            hpo_a = router_o(0, slice(0, 128))
            for ob in range(4):
                osl = slice(ob * 512, (ob + 1) * 512)
                yp = yp_tt0[ob]
                nc.tensor.matmul(yp, hpo_a, bo_sb[:, osl],
                                 start=False, stop=True)
                yt = pD.tile([128, 512], F32, name="yt", tag="yt", bufs=3)
                if ob % 2 == 0:
                    nc.scalar.activation(yt, yp, AF.Copy)
                else:
                    nc.vector.tensor_copy(yt, yp)
                nc.sync.dma_start(out=y[0:128, osl], in_=yt)

            # final chunk's gather, then token half 1
            nc.sync.dma_start(
                out=g_sb[:, :, 64 * (NQB - 1):64 * NQB],
                in_=cc_out[NQB - 1].rearrange("s (k p) t -> p (s k) t",
                                              k=2, p=128))
            hpo_b = router_o(1, slice(128, 256))
            for ob in range(4):
                osl = slice(ob * 512, (ob + 1) * 512)
                yp = ps.tile([128, 512], F32, name="yp1_%d" % ob,
                             tag=["p_q0", "p_q1", "p_hA", "p_kv"][ob])
                for k in range(NIF):
                    nc.tensor.matmul(yp, g_sb[:, k, 128:256],
                                     wo_tiles[ob][:, k, :],
                                     start=(k == 0), stop=False)
                nc.tensor.matmul(yp, hpo_b, bo_sb[:, osl],
                                 start=False, stop=True)
                yt = pD.tile([128, 512], F32, name="yt", tag="yt", bufs=3)
                if ob % 2 == 0:
                    nc.scalar.activation(yt, yp, AF.Copy)
                else:
                    nc.vector.tensor_copy(yt, yp)
                nc.sync.dma_start(out=y[128:256, osl], in_=yt)
        wo_ctx.__exit__(None, None, None)


# ======================= host side =======================

_CACHE = {}


def _prep_inputs(x, mask, freqs_cos, freqs_sin, wq, wk, wv, wo,
                 lq_router, lq_A, lq_B, lk_router, lk_A, lk_B,
                 lv_router, lv_A, lv_B, lo_router, lo_A, lo_B):
    scale = 1.0 / np.sqrt(HD)
    x = _f32(np.asarray(x)).reshape(S, D)
    maskf = _f32(np.asarray(mask)).reshape(S, S)
    maskT = np.maximum(maskf, MASK_NEG).T.copy()
    mask_cls, mask_lo = classify_mask(maskT)
    mask01 = _bf((maskT > MASK_NEG * 0.5).astype(np.float32))

    xT = _bf(x.T)
    cs4 = _bf(np.tile(_f32(freqs_cos).T, (4, 1)))      # [128, S]
    sn4 = _bf(np.tile(_f32(freqs_sin).T, (4, 1)))
    woT = _bf(_f32(wo).T)
    ao_p = _bf(np.concatenate([_a_pack(_f32(lo_A)), _f32(lo_router).T],
                              axis=1))                 # [D, 72]
    bo_f = _bf(_b_flat(_f32(lo_B), SCALING))

    # fused LoRA-A stationaries: [D, 128] = [aq|ak], [D, 88] = [av|rq|rk|rv]
    aA_p = _bf(np.concatenate(
        [_a_pack(_f32(lq_A)), _a_pack(_f32(lk_A))], axis=1))
    aB_p = _bf(np.concatenate(
        [_a_pack(_f32(lv_A)), _f32(lq_router).T, _f32(lk_router).T,
         _f32(lv_router).T], axis=1))

    shared = dict(xT=xT, cs4=cs4, sn4=sn4, woT=woT, mask01=mask01,
                  ao=ao_p, bo=bo_f, cst=_build_cst(), aA=aA_p, aB=aB_p)

    wqf, wkf, wvf = _f32(wq), _f32(wk), _f32(wv)
    lqB, lkB, lvB = _f32(lq_B), _f32(lk_B), _f32(lv_B)

    in_maps = []
    for c in range(NCORES):
        wq_c = wqf[c * QF:(c + 1) * QF][IDX_Q] * scale
        wk_c = wkf[c * KF:(c + 1) * KF][IDX_K]
        wv_c = wvf[c * KF:(c + 1) * KF]
        bq_c = _b_flat(lqB[:, c * QF:(c + 1) * QF, :][:, IDX_Q, :],
                       SCALING * scale)
        bk_c = _b_flat(lkB[:, c * KF:(c + 1) * KF, :][:, IDX_K, :], SCALING)
        bv_c = _b_flat(lvB[:, c * KF:(c + 1) * KF, :], SCALING)
        m = dict(shared)
        m.update(wqT=_bf(wq_c.T),
                 wkvT=_bf(np.concatenate([wk_c.T, wv_c.T], axis=1)),
                 bq=_bf(bq_c), bk=_bf(bk_c), bv=_bf(bv_c))
        in_maps.append(m)
    return in_maps, mask_cls, mask_lo


def get_graph(mask_cls, mask_lo):
    key = mask_cls.tobytes() + mask_lo.tobytes()
    if key not in _CACHE:
        _CACHE[key] = build(mask_cls, mask_lo)
    return _CACHE[key]


def kernel(x, start_pos, mask, freqs_cos, freqs_sin, wq, wk, wv, wo,
           lq_router, lq_A, lq_B, lk_router, lk_A, lk_B,
           lv_router, lv_A, lv_B, lo_router, lo_A, lo_B,
           _trace=False):
    from concourse.bass_utils import run_bass_kernel_spmd
    in_maps, mask_cls, mask_lo = _prep_inputs(
        x, mask, freqs_cos, freqs_sin, wq, wk, wv, wo,
        lq_router, lq_A, lq_B, lk_router, lk_A, lk_B,
        lv_router, lv_A, lv_B, lo_router, lo_A, lo_B)
    nc = get_graph(mask_cls, mask_lo)
    res = run_bass_kernel_spmd(nc, in_maps, list(range(NCORES)), trace=_trace)
    # core c's y rows: group g (0..3) covers tokens [512g + 64c, 512g + 64c + 64)
    ys = np.stack([res.results[c]["y"] for c in range(NCORES)], axis=0)
    ys = ys.reshape(NCORES, 4, 64, D).transpose(1, 0, 2, 3).reshape(S, D)
    out = ys.reshape(B, S, H * HD).astype(np.float32)
    if _trace:
        return out, res
    return out


# revision 23
# speedup vs baseline: 1.5058x; 1.0349x over previous
"""Trainium2 Bass kernel for MoE-LoRA GQA attention (nn_Attention_57389353009692).

V2 strategy (8 NeuronCores, one SPMD launch):
  - Tensor-parallel over heads: core c owns q-heads 4c..4c+3 and kv-head c.
  - Phase A (per 512-token block): QKV projections (+ MoE-LoRA) with packed
    matmul chains (wk|wv fused; LoRA-A for q/k/v + all three routers fused
    into two chains of 128/88 rows), router softmax done with
    exp -> ones-matmul row-sum -> reciprocal -> broadcast-matmul (no
    transposes, no DRAM bounce), RoPE in bf16 on 128 partitions.
  - Phase C: flash-style attention per 512-query block; causal mask applied
    as a 0/1 multiply after exp (bf16); output normalized PRE-collective via
    reciprocal-of-denominator broadcast matmuls fused into the PSUM->SBUF
    cast.
  - AllToAll is chunked per query block (4 collectives) and overlaps the
    remaining attention compute. Output tokens are interleaved at
    64-granularity: core c owns tokens {t : (t//64) % 8 == c} so every chunk
    is a uniform 8-way exchange.
  - Phase D: o-projection + o-LoRA for the core's 256 tokens with the full
    (prefetched) wo.

Numerics: bf16 operands, fp32 PSUM accumulation, fp32->exp softmax without
max subtraction (scores are O(1) here; masked entries are zeroed exactly by
the 0/1 multiply). Scale 1/sqrt(64) folded into wq and q-LoRA-B on host.
RoPE trick: interleaved even/odd pairs are made contiguous by permuting
wq/wk output features on host (per 2-head "page": [h0e|h1e|h0o|h1o]).
"""

import sys

for _p in ("/opt/trn_rl_repo", "/root/.axon_site/_ro/trn_rl_repo"):
    if _p not in sys.path:
        sys.path.insert(0, _p)

import numpy as np
import ml_dtypes

import concourse.bass as bass
import concourse.tile as tile
from concourse import bacc, mybir
from concourse.masks import make_identity
from concourse.alu_op_type import AluOpType

F32 = mybir.dt.float32
BF16 = mybir.dt.bfloat16
AF = mybir.ActivationFunctionType
AX = mybir.AxisListType
BF16NP = ml_dtypes.bfloat16

B, S, D = 1, 2048, 2048
H, KVH, HD = 32, 8, 64
NREP = H // KVH
R, E = 8, 8
SCALING = 32.0 / 8.0
NCORES = 8
QH = H // NCORES          # 4 q heads per core
QF = QH * HD              # 256 q feats per core
KF = HD                   # 64 kv feats per core
TSH = S // NCORES         # 256 tokens per core for o-proj
NKT = S // 128            # 16 key tiles
NQB = S // 512            # 4 query blocks
NIF = D // 128            # 16 contraction tiles

MASK_NEG = -1e30

# mask tile classes
M_SKIP, M_ZERO, M_ADD = 0, 1, 2


def _build_perm():
    """Per-core feature permutations for rope-friendly layout."""
    idx_q = np.zeros(QF, dtype=np.int64)
    for f in range(QF):
        page, w = divmod(f, 128)
        if w < 32:
            hl, j, odd = 2 * page, w, 0
        elif w < 64:
            hl, j, odd = 2 * page + 1, w - 32, 0
        elif w < 96:
            hl, j, odd = 2 * page, w - 64, 1
        else:
            hl, j, odd = 2 * page + 1, w - 96, 1
        idx_q[f] = 64 * hl + 2 * j + odd
    idx_k = np.zeros(KF, dtype=np.int64)
    for w in range(KF):
        if w < 32:
            idx_k[w] = 2 * w
        else:
            idx_k[w] = 2 * (w - 32) + 1
    return idx_q, idx_k


IDX_Q, IDX_K = _build_perm()


def _a_pack(A):
    """[E,R,D] -> [D, 64] with col r*8+e."""
    return np.transpose(A, (1, 0, 2)).reshape(E * R, -1).T


def _b_flat(Bw, scale):
    """[E, OF, R] -> [64, OF] with row r*8+e."""
    return (np.transpose(Bw, (2, 0, 1)).reshape(E * R, -1) * scale)


def _bf(x):
    return np.ascontiguousarray(x, dtype=np.float32).astype(BF16NP)


def _f32(x):
    return np.ascontiguousarray(x, dtype=np.float32)


def classify_mask(maskT):
    """maskT: [S(k), S(q)] clamped fp32. Returns ([NKT, NQB] class map,
    [NKT, NQB] live-start-column map for M_ADD tiles).

    For an M_ADD tile, lo is the first live column, rounded down to 128;
    columns >= lo+128 must be fully live (causal staircase) -- the kernel
    then computes only [lo, 512) and masks just [lo, lo+128)."""
    cls = np.zeros((NKT, NQB), dtype=np.int64)
    los = np.zeros((NKT, NQB), dtype=np.int64)
    for kt in range(NKT):
        blk_rows = maskT[kt * 128:(kt + 1) * 128]
        for qb in range(NQB):
            blk = blk_rows[:, qb * 512:(qb + 1) * 512]
            if np.all(blk <= MASK_NEG * 0.5):
                cls[kt, qb] = M_SKIP
            elif np.all(blk == 0.0):
                cls[kt, qb] = M_ZERO
            else:
                cls[kt, qb] = M_ADD
                live = np.where((blk == 0.0).any(axis=0))[0]
                lo = (int(live[0]) // 128) * 128 if len(live) else 0
                if lo + 128 <= 512 and not np.all(blk[:, lo + 128:] == 0.0):
                    lo = 0  # not a causal staircase; keep full width
                los[kt, qb] = lo
    return cls, los


# constants tensor layout (bf16, [24, 344]):
#  [:, 0:128]   E_A: row e, col j -> 1 if (j<64 and e==j%8) or (j>=64 and e-8==j%8)
#  [:, 128:192] E_v: row e, col j -> 1 if e-16 == j%8
#  [:, 192:216] ones24: block-diag 3x(8x8 ones)
#  [0:8, 216:280] E8o: row e, col j -> 1 if e == j%8
#  [0:1, 280:344] ones64 row
CST_W = 344


def _build_cst():
    cst = np.zeros((24, CST_W), dtype=np.float32)
    for j in range(64):
        cst[j % 8, j] = 1.0           # E_A q half
        cst[8 + j % 8, 64 + j] = 1.0  # E_A k half
        cst[16 + j % 8, 128 + j] = 1.0  # E_v
        cst[j % 8, 216 + j] = 1.0     # E8o
        cst[0, 280 + j] = 1.0         # ones64
    for b in range(3):
        cst[8 * b:8 * b + 8, 192 + 8 * b:200 + 8 * b] = 1.0  # ones24
    return _bf(cst)


def build(mask_cls, mask_lo):
    """Build the SPMD Bass graph. mask_cls: [NKT, NQB] int array."""
    nc = bacc.Bacc(None, target_bir_lowering=False)

    # ---- DRAM I/O (per-core shards prepared on host) ----
    xT = nc.declare_dram_parameter("xT", [D, S], BF16, isOutput=False)
    wqT = nc.declare_dram_parameter("wqT", [D, QF], BF16, isOutput=False)
    wkvT = nc.declare_dram_parameter("wkvT", [D, 2 * KF], BF16, isOutput=False)
    aA = nc.declare_dram_parameter("aA", [D, 128], BF16, isOutput=False)
    aB = nc.declare_dram_parameter("aB", [D, 88], BF16, isOutput=False)
    ao = nc.declare_dram_parameter("ao", [D, 72], BF16, isOutput=False)
    bq = nc.declare_dram_parameter("bq", [E * R, QF], BF16, isOutput=False)
    bk = nc.declare_dram_parameter("bk", [E * R, KF], BF16, isOutput=False)
    bv = nc.declare_dram_parameter("bv", [E * R, KF], BF16, isOutput=False)
    bo = nc.declare_dram_parameter("bo", [E * R, D], BF16, isOutput=False)
    woT = nc.declare_dram_parameter("woT", [D, D], BF16, isOutput=False)
    cs4 = nc.declare_dram_parameter("cs4", [128, S], BF16, isOutput=False)
    sn4 = nc.declare_dram_parameter("sn4", [128, S], BF16, isOutput=False)
    mask01 = nc.declare_dram_parameter("mask01", [S, S], BF16, isOutput=False)
    cst = nc.declare_dram_parameter("cst", [24, CST_W], BF16, isOutput=False)
    y = nc.declare_dram_parameter("y", [TSH, D], F32, isOutput=True)

    # internal DRAM for the chunked collectives: [dest/src, 4h*64 feat, 64 tok]
    cc_in = [nc.dram_tensor("cc_in%d" % q, [NCORES, QF, 64], BF16)
             for q in range(NQB)]
    cc_out = [nc.dram_tensor("cc_out%d" % q, [NCORES, QF, 64], BF16)
              for q in range(NQB)]

    with tile.TileContext(nc) as tc:
        _emit(nc, tc, locals(), mask_cls, mask_lo)
    nc.finalize()
    return nc


def _emit(nc, tc, t, mask_cls, mask_lo):
    xT, wqT, wkvT = t["xT"], t["wqT"], t["wkvT"]
    aA, aB, ao = t["aA"], t["aB"], t["ao"]
    bq, bk, bv, bo = t["bq"], t["bk"], t["bv"], t["bo"]
    woT, cs4, sn4, mask01, y = t["woT"], t["cs4"], t["sn4"], t["mask01"], t["y"]
    cst = t["cst"]
    cc_in, cc_out = t["cc_in"], t["cc_out"]

    import contextlib
    ctx = contextlib.ExitStack()
    with ctx:
        persist = ctx.enter_context(tc.tile_pool(name="persist", bufs=1))
        ps = ctx.enter_context(tc.tile_pool(name="ps", bufs=1, space="PSUM"))

        # ---- persistent weights, split in k-groups of 4 for early start ----
        NSP = 4
        KG = NIF // NSP
        aA_sb, aB_sb, wq_sb, wkv_sb = [], [], [], []
        xq0 = persist.tile([128, NIF, 512], BF16, name="xq0")
        for g in range(NSP):
            ksl = slice(g * KG * 128, (g + 1) * KG * 128)
            tl = persist.tile([128, KG, 128], BF16, name="aA%d" % g)
            nc.scalar.dma_start(
                out=tl, in_=aA[ksl].rearrange("(n p) f -> p n f", p=128))
            aA_sb.append(tl)
            tl = persist.tile([128, KG, 88], BF16, name="aB%d" % g)
            nc.gpsimd.dma_start(
                out=tl, in_=aB[ksl].rearrange("(n p) f -> p n f", p=128))
            aB_sb.append(tl)
            # first token block's x rides ahead of the q weights
            nc.sync.dma_start(
                out=xq0[:, g * KG:(g + 1) * KG, :],
                in_=xT[ksl].rearrange("(n p) t -> p n t", p=128)[:, :, 0:512])
            tl = persist.tile([128, KG, QF], BF16, name="wq%d" % g)
            nc.sync.dma_start(
                out=tl, in_=wqT[ksl].rearrange("(n p) f -> p n f", p=128))
            wq_sb.append(tl)
            tl = persist.tile([128, KG, 2 * KF], BF16, name="wkv%d" % g)
            nc.gpsimd.dma_start(
                out=tl, in_=wkvT[ksl].rearrange("(n p) f -> p n f", p=128))
            wkv_sb.append(tl)

        def A_AT(k):  # aA chain lhsT for contraction tile k
            return aA_sb[k // KG][:, k % KG, :]

        def A_BT(k):
            return aB_sb[k // KG][:, k % KG, :]

        def W_Q(k):
            return wq_sb[k // KG][:, k % KG, :]

        def W_KV(k):
            return wkv_sb[k // KG][:, k % KG, :]

        cst_sb = persist.tile([24, CST_W], BF16)
        nc.gpsimd.dma_start(out=cst_sb, in_=cst[:])
        E_A = cst_sb[:, 0:128]
        E_v = cst_sb[0:24, 128:192]
        ones24 = cst_sb[:, 192:216]
        ones8 = cst_sb[0:8, 192:200]
        E8o = cst_sb[0:8, 216:280]
        ones64 = cst_sb[0:1, 280:344]

        bq_sb = persist.tile([64, QF], BF16)
        nc.gpsimd.dma_start(out=bq_sb, in_=bq[:])
        bk_sb = persist.tile([128, KF], BF16)   # bk lives at partitions 64:128
        nc.gpsimd.dma_start(out=bk_sb[64:128, :], in_=bk[:])
        bv_sb = persist.tile([64, KF], BF16)
        nc.gpsimd.dma_start(out=bv_sb, in_=bv[:])
        bo_sb = persist.tile([64, D], BF16)
        nc.gpsimd.dma_start(out=bo_sb, in_=bo[:])
        ao_sb = persist.tile([128, NIF, 72], BF16)
        nc.scalar.dma_start(out=ao_sb,
                            in_=ao.rearrange("(n p) f -> p n f", p=128))
        cs_sb = persist.tile([128, S], BF16)
        nc.scalar.dma_start(out=cs_sb, in_=cs4[:])
        sn_sb = persist.tile([128, S], BF16)
        nc.scalar.dma_start(out=sn_sb, in_=sn4[:])

        ident_b = persist.tile([128, 128], BF16)
        make_identity(nc, ident_b)

        # attention operands (persist across phases)
        qh_sb = persist.tile([128, 2, S], BF16)   # [2 heads x 64, page, S]
        kh_sb = persist.tile([128, S], BF16)      # kv head duplicated 2x
        vtok = persist.tile([128, NKT, 65], BF16)  # token-major v + ones col
        nc.vector.memset(vtok[:, :, 64:65], 1.0)
        g_sb = persist.tile([128, NIF, TSH], BF16)  # gathered out (post-A2A)

        # ================= Phase A: QKV + LoRA + RoPE =================
        vT_all = persist.tile([64, S], BF16)      # v (feat-major) staging
        with tc.tile_pool(name="pA", bufs=1) as pA:
            for tb in range(4):
                tsl = slice(tb * 512, (tb + 1) * 512)
                if tb == 0:
                    xq = xq0
                else:
                    xq = pA.tile([128, NIF, 512], BF16, name="xq", tag="xq",
                                 bufs=3)
                    nc.sync.dma_start(
                        out=xq,
                        in_=xT.rearrange("(n p) t -> p n t", p=128)[:, :, tsl])

                # ---- main projection chains ----
                hA = ps.tile([128, 512], F32, name="hA", tag="p_hA")
                hB = ps.tile([88, 512], F32, name="hB", tag="p_hB")
                q0 = ps.tile([128, 512], F32, name="q0", tag="p_q0")
                q1 = ps.tile([128, 512], F32, name="q1", tag="p_q1")
                kv = ps.tile([128, 512], F32, name="kv", tag="p_kv")
                for k in range(NIF):
                    st = k == 0
                    sp = k == NIF - 1
                    rhs = xq[:, k, :]
                    nc.tensor.matmul(hA, A_AT(k), rhs, start=st, stop=sp)
                    nc.tensor.matmul(hB, A_BT(k), rhs, start=st, stop=sp)
                for k in range(NIF):
                    rhs = xq[:, k, :]
                    st = k == 0
                    nc.tensor.matmul(q0, W_Q(k)[:, 0:128], rhs,
                                     start=st, stop=False)
                    nc.tensor.matmul(q1, W_Q(k)[:, 128:256], rhs,
                                     start=st, stop=False)
                    nc.tensor.matmul(kv, W_KV(k), rhs, start=st, stop=False)

                # ---- router softmax (q,k,v fused; no transposes) ----
                ex3 = pA.tile([24, 512], BF16, name="ex3", tag="ex3", bufs=2)
                nc.scalar.activation(ex3, hB[64:88, :], AF.Exp)
                s3 = ps.tile([24, 512], F32, name="s3", tag="p_s3")
                nc.tensor.matmul(s3, ones24, ex3, start=True, stop=True)
                s3s = pA.tile([24, 512], F32, name="s3s", tag="s3s", bufs=2)
                nc.vector.tensor_copy(s3s, s3)
                rec3 = pA.tile([24, 512], F32, name="rec3", tag="rec3",
                               bufs=2)
                nc.vector.reciprocal_approx_fast(out=rec3, in_=s3s)
                rw3 = pA.tile([24, 512], BF16, name="rw3", tag="rw3", bufs=2)
                nc.vector.tensor_tensor(rw3, ex3, rec3, AluOpType.mult)
                rwbA = ps.tile([128, 512], F32, name="rwbA", tag="p_rwA")
                nc.tensor.matmul(rwbA, E_A, rw3, start=True, stop=True)
                rwbV = ps.tile([64, 512], F32, name="rwbV", tag="p_rwV")
                nc.tensor.matmul(rwbV, E_v, rw3, start=True, stop=True)
                rwbA_s = pA.tile([128, 512], BF16, name="rwbA_s",
                                 tag="rwbA_s", bufs=2)
                nc.scalar.activation(rwbA_s, rwbA, AF.Copy)
                rwbV_s = pA.tile([64, 512], BF16, name="rwbV_s",
                                 tag="rwbV_s", bufs=2)
                nc.scalar.activation(rwbV_s, rwbV, AF.Copy)
                hpA = pA.tile([128, 512], BF16, name="hpA", tag="hpA", bufs=2)
                nc.vector.tensor_tensor(hpA, hA, rwbA_s, AluOpType.mult)
                hpV = pA.tile([64, 512], BF16, name="hpV", tag="hpV", bufs=2)
                nc.vector.tensor_tensor(hpV, hB[0:64, :], rwbV_s,
                                        AluOpType.mult)

                # ---- LoRA-B closes the accumulations ----
                nc.tensor.matmul(q0, bq_sb[:, 0:128], hpA[0:64, :],
                                 start=False, stop=True)
                nc.tensor.matmul(q1, bq_sb[:, 128:256], hpA[0:64, :],
                                 start=False, stop=True)
                nc.tensor.matmul(kv[0:64, :], bk_sb[64:128, :],
                                 hpA[64:128, :], start=False, stop=True,
                                 tile_position=(64, 0))
                nc.tensor.matmul(kv[64:128, :], bv_sb, hpV,
                                 start=False, stop=True,
                                 tile_position=(0, 64))

                # ---- PSUM extraction (scalar engine) ----
                qe = pA.tile([128, 512], BF16, name="qe", tag="qe", bufs=2)
                qo = pA.tile([128, 512], BF16, name="qo", tag="qo", bufs=2)
                nc.scalar.activation(qe[0:64, :], q0[0:64, :], AF.Copy)
                nc.scalar.activation(qe[64:128, :], q1[0:64, :], AF.Copy)
                nc.scalar.activation(qo[0:64, :], q0[64:128, :], AF.Copy)
                nc.scalar.activation(qo[64:128, :], q1[64:128, :], AF.Copy)
                ke = pA.tile([32, 512], BF16, name="ke", tag="ke", bufs=2)
                ko = pA.tile([32, 512], BF16, name="ko", tag="ko", bufs=2)
                nc.scalar.activation(ke, kv[0:32, :], AF.Copy)
                nc.scalar.activation(ko, kv[32:64, :], AF.Copy)
                nc.scalar.activation(vT_all[:, tsl], kv[64:128, :], AF.Copy)

                # ---- RoPE (bf16, vector engine) ----
                cs_t = cs_sb[:, tsl]
                sn_t = sn_sb[:, tsl]
                t1 = pA.tile([128, 512], BF16, name="t1", tag="t1", bufs=2)
                t2 = pA.tile([128, 512], BF16, name="t2", tag="t2", bufs=2)
                rote = pA.tile([128, 512], BF16, name="rote", tag="rote",
                               bufs=2)
                roto = pA.tile([128, 512], BF16, name="roto", tag="roto",
                               bufs=2)
                nc.vector.tensor_tensor(t1, qe, cs_t, AluOpType.mult)
                nc.vector.tensor_tensor(t2, qo, sn_t, AluOpType.mult)
                nc.vector.tensor_tensor(rote, t1, t2, AluOpType.subtract)
                nc.vector.tensor_tensor(t1, qe, sn_t, AluOpType.mult)
                nc.vector.tensor_tensor(t2, qo, cs_t, AluOpType.mult)
                nc.vector.tensor_tensor(roto, t1, t2, AluOpType.add)
                k1 = pA.tile([32, 512], BF16, name="k1", tag="k1", bufs=2)
                k2 = pA.tile([32, 512], BF16, name="k2", tag="k2", bufs=2)
                csk = cs_sb[0:32, tsl]
                snk = sn_sb[0:32, tsl]
                nc.vector.tensor_tensor(k1, ke, csk, AluOpType.mult)
                nc.vector.tensor_tensor(k2, ko, snk, AluOpType.mult)
                nc.vector.tensor_tensor(kh_sb[0:32, tsl], k1, k2,
                                        AluOpType.subtract)
                nc.vector.tensor_tensor(k1, ke, snk, AluOpType.mult)
                nc.vector.tensor_tensor(k2, ko, csk, AluOpType.mult)
                nc.vector.tensor_tensor(kh_sb[32:64, tsl], k1, k2,
                                        AluOpType.add)

                # head rearrange via SBUF->SBUF DMA (off the engines)
                nc.gpsimd.dma_start(out=kh_sb[64:128, tsl],
                                    in_=kh_sb[0:64, tsl])
                for h in range(QH):
                    page, i = h // 2, h % 2
                    nc.gpsimd.dma_start(
                        out=qh_sb[64 * i:64 * i + 32, page, tsl],
                        in_=rote[32 * h:32 * h + 32, :])
                    nc.gpsimd.dma_start(
                        out=qh_sb[64 * i + 32:64 * i + 64, page, tsl],
                        in_=roto[32 * h:32 * h + 32, :])

            # token-major v, built after the per-block pipeline drains
            for kt in range(NKT):
                v_ps = ps.tile([128, 64], BF16, name="v_ps", tag="p_s3")
                nc.tensor.transpose(v_ps,
                                    vT_all[:, 128 * kt:128 * kt + 128],
                                    ident_b[0:64, 0:64])
                nc.vector.tensor_copy(vtok[:, kt, 0:64], v_ps)

        # prefetch the full output-projection weight during attention
        wo_ctx = tc.tile_pool(name="wo_pool", bufs=4)
        wo_pool = wo_ctx.__enter__()
        wo_tiles = []
        for ob in range(4):
            osl = slice(ob * 512, (ob + 1) * 512)
            wo_sb = wo_pool.tile([128, NIF, 512], BF16, name="wo_sb",
                                 tag="wo", bufs=4)
            nc.gpsimd.dma_start(
                out=wo_sb,
                in_=woT.rearrange("(n p) f -> p n f", p=128)[:, :, osl])
            wo_tiles.append(wo_sb)

        # ============ Phase C: attention + chunked A2A ============
        SC_TAGS = ["p_q0", "p_q1", "p_hA", "p_hB"]
        OUT_TAGS = ["p_kv", "p_s3", "p_rwA", "p_rwV"]
        with tc.tile_pool(name="pC", bufs=1) as pC:
            def emit_tail(qb, outps, gather=True):
                """Normalize + ship chunk qb. Emitted after the next query
                block's first score/exp round so the reciprocal latency
                hides under attention compute."""
                for h in range(QH):
                    dens = pC.tile([1, 512], F32, name="dens%d" % h,
                                   tag="dens%d" % h, bufs=2)
                    nc.vector.tensor_copy(dens, outps[h][64:65, :])
                    recf = pC.tile([1, 512], F32, name="recf%d" % h,
                                   tag="recf%d" % h, bufs=2)
                    nc.vector.reciprocal_approx_fast(out=recf, in_=dens)
                    rec = pC.tile([1, 512], BF16, name="rec%d" % h,
                                  tag="rec%d" % h, bufs=2)
                    nc.vector.tensor_copy(rec, recf)
                    rb = ps.tile([64, 512], F32, name="rb", tag=SC_TAGS[h])
                    nc.tensor.matmul(rb, ones64, rec, start=True, stop=True)
                    rb_s = pC.tile([64, 512], BF16, name="rb_s%d" % h,
                                   tag="rb_s%d" % h, bufs=2)
                    nc.vector.tensor_copy(rb_s, rb)
                    o65 = pC.tile([64, 512], BF16, name="o65%d" % h,
                                  tag="o65%d" % h, bufs=2)
                    nc.vector.tensor_tensor(o65, outps[h][0:64, :], rb_s,
                                            AluOpType.mult)
                    # [64, 512] -> cc_in[qb][dest, 64h:64h+64, 0:64]
                    nc.gpsimd.dma_start(
                        out=cc_in[qb][:, 64 * h:64 * h + 64, :]
                            .rearrange("d p t -> p d t"),
                        in_=o65)
                nc.gpsimd.collective_compute(
                    "AllToAll",
                    AluOpType.bypass,
                    ins=[cc_in[qb][:]],
                    outs=[cc_out[qb][:]],
                    replica_groups=[list(range(NCORES))],
                )
                if gather:
                    # gather this chunk into g_sb[:, :, 64qb:64qb+64]
                    nc.sync.dma_start(
                        out=g_sb[:, :, 64 * qb:64 * qb + 64],
                        in_=cc_out[qb].rearrange("s (k p) t -> p (s k) t",
                                                 k=2, p=128))

            pending = None
            yp_tt0 = {}
            for qb in range(NQB):
                qsl = slice(qb * 512, (qb + 1) * 512)
                active = [kt for kt in range(NKT)
                          if mask_cls[kt, qb] != M_SKIP]
                assert active, f"fully masked query block qb={qb}"
                outps = [ps.tile([65, 512], F32, name="outp%d" % h,
                                 tag=OUT_TAGS[h]) for h in range(QH)]
                for idx, kt in enumerate(active):
                    c = mask_cls[kt, qb]
                    lo = int(mask_lo[kt, qb]) if c == M_ADD else 0
                    mt = None
                    if c == M_ADD:
                        mt = pC.tile([128, 128], BF16, name="mt",
                                     tag="mt", bufs=4)
                        nc.sync.dma_start(
                            out=mt,
                            in_=mask01[128 * kt:128 * kt + 128,
                                       512 * qb + lo:512 * qb + lo + 128])
                    ksl = slice(128 * kt, 128 * kt + 128)
                    qslc = slice(512 * qb + lo, 512 * (qb + 1))
                    prs = []
                    for h in range(QH):
                        page, i = h // 2, h % 2
                        sc = ps.tile([128, 512], F32, name="sc%d" % h,
                                     tag=SC_TAGS[h])
                        nc.tensor.matmul(sc[:, lo:512],
                                         kh_sb[64 * i:64 * i + 64, ksl],
                                         qh_sb[64 * i:64 * i + 64, page,
                                               qslc],
                                         start=True, stop=True,
                                         tile_position=(64 * i, 0))
                        pr = pC.tile([128, 512], BF16, name="pr%d" % h,
                                     tag="pr%d" % h, bufs=2)
                        nc.scalar.activation(pr[:, lo:512], sc[:, lo:512],
                                             AF.Exp)
                        if mt is not None:
                            nc.vector.tensor_tensor(pr[:, lo:lo + 128],
                                                    pr[:, lo:lo + 128], mt,
                                                    AluOpType.mult)
                        prs.append(pr)
                    if idx == 0 and pending is not None:
                        # previous block's normalization rides behind this
                        # round's score matmuls
                        emit_tail(qb - 1, pending)
                        pending = None
                    for h in range(QH):
                        nc.tensor.matmul(outps[h][:, lo:512],
                                         vtok[:, kt, :],
                                         prs[h][:, lo:512],
                                         start=(kt == active[0]),
                                         stop=(kt == active[-1]))
                pending = outps
            emit_tail(NQB - 1, pending, gather=False)
            # o-proj token-half 0 (chunks 0/1 landed long ago) rides the
            # tensor engine under the final AllToAll
            YP0_TAGS = ["p_q0", "p_q1", "p_hA", "p_kv"]
            for ob in range(4):
                osl = slice(ob * 512, (ob + 1) * 512)
                yp = ps.tile([128, 512], F32, name="yp0_%d" % ob,
                             tag=YP0_TAGS[ob])
                for k in range(NIF):
                    nc.tensor.matmul(yp, g_sb[:, k, 0:128],
                                     wo_tiles[ob][:, k, :],
                                     start=(k == 0), stop=False)
                yp_tt0[ob] = yp

        # ================= Phase D: o-proj =================
        with tc.tile_pool(name="pD", bufs=1) as pD:
            def router_o(half, hsl):
                """o-LoRA router for one 128-token half; returns hpo half."""
                ho = ps.tile([72, 128], F32, name="ho%d" % half, tag="p_hB")
                for k in range(NIF):
                    nc.tensor.matmul(ho, ao_sb[:, k, :], g_sb[:, k, hsl],
                                     start=(k == 0), stop=(k == NIF - 1))
                exo = pD.tile([8, 128], BF16, name="exo%d" % half)
                nc.scalar.activation(exo, ho[64:72, :], AF.Exp)
                so = ps.tile([8, 128], F32, name="so%d" % half, tag="p_s3")
                nc.tensor.matmul(so, ones8, exo, start=True, stop=True)
                sos = pD.tile([8, 128], F32, name="sos%d" % half)
                nc.vector.tensor_copy(sos, so)
                reco = pD.tile([8, 128], F32, name="reco%d" % half)
                nc.vector.reciprocal_approx_fast(out=reco, in_=sos)
                rwo = pD.tile([8, 128], BF16, name="rwo%d" % half)
                nc.vector.tensor_tensor(rwo, exo, reco, AluOpType.mult)
                rwbo = ps.tile([64, 128], F32, name="rwbo%d" % half,
                               tag="p_rwA")
                nc.tensor.matmul(rwbo, E8o, rwo, start=True, stop=True)
                rwbo_s = pD.tile([64, 128], BF16, name="rwbo_s%d" % half)
                nc.vector.tensor_copy(rwbo_s, rwbo)
                hpo = pD.tile([64, 128], BF16, name="hpo%d" % half)
                nc.vector.tensor_tensor(hpo, ho[0:64, :], rwbo_s,
                                        AluOpType.mult)
                return hpo

            # token half 0: LoRA + bo closure + store, all before the final
            # gather (g_sb cols 0:128 come from chunks 0/1)
            hpo_a = router_o(0, slice(0, 128))
            for ob in range(4):
                osl = slice(ob * 512, (ob + 1) * 512)
                yp = yp_tt0[ob]
                nc.tensor.matmul(yp, hpo_a, bo_sb[:, osl],
                                 start=False, stop=True)
                yt = pD.tile([128, 512], F32, name="yt", tag="yt", bufs=4)
                nc.scalar.activation(yt[:, 0:256], yp[:, 0:256], AF.Copy)
                nc.vector.tensor_copy(yt[:, 256:512], yp[:, 256:512])
                nc.sync.dma_start(out=y[0:128, osl], in_=yt)

            # final chunk's gather, then token half 1
            nc.sync.dma_start(
                out=g_sb[:, :, 64 * (NQB - 1):64 * NQB],
                in_=cc_out[NQB - 1].rearrange("s (k p) t -> p (s k) t",
                                              k=2, p=128))
            hpo_b = router_o(1, slice(128, 256))
            for ob in range(4):
                osl = slice(ob * 512, (ob + 1) * 512)
                yp = ps.tile([128, 512], F32, name="yp1_%d" % ob,
                             tag=["p_q0", "p_q1", "p_hA", "p_kv"][ob])
                for k in range(NIF):
                    nc.tensor.matmul(yp, g_sb[:, k, 128:256],
                                     wo_tiles[ob][:, k, :],
                                     start=(k == 0), stop=False)
                nc.tensor.matmul(yp, hpo_b, bo_sb[:, osl],
                                 start=False, stop=True)
                yt = pD.tile([128, 512], F32, name="yt", tag="yt", bufs=4)
                nc.scalar.activation(yt[:, 0:256], yp[:, 0:256], AF.Copy)
                nc.vector.tensor_copy(yt[:, 256:512], yp[:, 256:512])
                nc.sync.dma_start(out=y[128:256, osl], in_=yt)
        wo_ctx.__exit__(None, None, None)


# ======================= host side =======================

_CACHE = {}


def _prep_inputs(x, mask, freqs_cos, freqs_sin, wq, wk, wv, wo,
                 lq_router, lq_A, lq_B, lk_router, lk_A, lk_B,
                 lv_router, lv_A, lv_B, lo_router, lo_A, lo_B):
    scale = 1.0 / np.sqrt(HD)
    x = _f32(np.asarray(x)).reshape(S, D)
    maskf = _f32(np.asarray(mask)).reshape(S, S)
    maskT = np.maximum(maskf, MASK_NEG).T.copy()
    mask_cls, mask_lo = classify_mask(maskT)
    mask01 = _bf((maskT > MASK_NEG * 0.5).astype(np.float32))

    xT = _bf(x.T)
    cs4 = _bf(np.tile(_f32(freqs_cos).T, (4, 1)))      # [128, S]
    sn4 = _bf(np.tile(_f32(freqs_sin).T, (4, 1)))
    woT = _bf(_f32(wo).T)
    ao_p = _bf(np.concatenate([_a_pack(_f32(lo_A)), _f32(lo_router).T],
                              axis=1))                 # [D, 72]
    bo_f = _bf(_b_flat(_f32(lo_B), SCALING))

    # fused LoRA-A stationaries: [D, 128] = [aq|ak], [D, 88] = [av|rq|rk|rv]
    aA_p = _bf(np.concatenate(
        [_a_pack(_f32(lq_A)), _a_pack(_f32(lk_A))], axis=1))
    aB_p = _bf(np.concatenate(
        [_a_pack(_f32(lv_A)), _f32(lq_router).T, _f32(lk_router).T,
         _f32(lv_router).T], axis=1))

    shared = dict(xT=xT, cs4=cs4, sn4=sn4, woT=woT, mask01=mask01,
                  ao=ao_p, bo=bo_f, cst=_build_cst(), aA=aA_p, aB=aB_p)

    wqf, wkf, wvf = _f32(wq), _f32(wk), _f32(wv)
    lqB, lkB, lvB = _f32(lq_B), _f32(lk_B), _f32(lv_B)

    in_maps = []
    for c in range(NCORES):
        wq_c = wqf[c * QF:(c + 1) * QF][IDX_Q] * scale
        wk_c = wkf[c * KF:(c + 1) * KF][IDX_K]
        wv_c = wvf[c * KF:(c + 1) * KF]
        bq_c = _b_flat(lqB[:, c * QF:(c + 1) * QF, :][:, IDX_Q, :],
                       SCALING * scale)
        bk_c = _b_flat(lkB[:, c * KF:(c + 1) * KF, :][:, IDX_K, :], SCALING)
        bv_c = _b_flat(lvB[:, c * KF:(c + 1) * KF, :], SCALING)
        m = dict(shared)
        m.update(wqT=_bf(wq_c.T),
                 wkvT=_bf(np.concatenate([wk_c.T, wv_c.T], axis=1)),
                 bq=_bf(bq_c), bk=_bf(bk_c), bv=_bf(bv_c))
        in_maps.append(m)
    return in_maps, mask_cls, mask_lo


def get_graph(mask_cls, mask_lo):
    key = mask_cls.tobytes() + mask_lo.tobytes()
    if key not in _CACHE:
        _CACHE[key] = build(mask_cls, mask_lo)
    return _CACHE[key]


def kernel(x, start_pos, mask, freqs_cos, freqs_sin, wq, wk, wv, wo,
           lq_router, lq_A, lq_B, lk_router, lk_A, lk_B,
           lv_router, lv_A, lv_B, lo_router, lo_A, lo_B,
           _trace=False):
    from concourse.bass_utils import run_bass_kernel_spmd
    in_maps, mask_cls, mask_lo = _prep_inputs(
        x, mask, freqs_cos, freqs_sin, wq, wk, wv, wo,
        lq_router, lq_A, lq_B, lk_router, lk_A, lk_B,
        lv_router, lv_A, lv_B, lo_router, lo_A, lo_B)
    nc = get_graph(mask_cls, mask_lo)
    res = run_bass_kernel_spmd(nc, in_maps, list(range(NCORES)), trace=_trace)
    # core c's y rows: group g (0..3) covers tokens [512g + 64c, 512g + 64c + 64)
    ys = np.stack([res.results[c]["y"] for c in range(NCORES)], axis=0)
    ys = ys.reshape(NCORES, 4, 64, D).transpose(1, 0, 2, 3).reshape(S, D)
    out = ys.reshape(B, S, H * HD).astype(np.float32)
    if _trace:
        return out, res
    return out
